# revision 1
# baseline (speedup 1.0000x reference)
"""Expert-parallel MoE (top-2 of 8 experts, SwiGLU) on 8 Trainium2 NeuronCores.

Sharding: one expert per core (W1/W3/W2 sharded on the expert axis), router
replicated. Each core, fully on-device:
  1. Routing: logitsT = Wr.T @ x.T (fp32 PE matmul), PE-transpose to [tok, 8],
     top-2 + softmax -> per-token combine weight c for this core's expert.
  2. Compaction: cross-partition prefix sum (strictly-upper-triangular ones
     matmul) assigns every selected token a dense slot; an indirect-DMA
     scatter writes (token_id, c) pairs into a DRAM slot table.
  3. Dispatch: read the token ids back, indirect-DMA gather the selected rows
     of x (bf16), PE-transpose them to put H on partitions.
  4. Expert FFN: gate/up/down matmuls in bf16 with fp32 PSUM accumulation,
     SwiGLU, scale by c, emit yT [H, CAP] fp32 plus the slot table.
Host: out[idx_e] += yt_e.T accumulated over the 8 cores (the unshard step for
expert-parallel sharding). Unfilled slots carry c = 0 so they contribute 0.
"""
import sys

sys.path.insert(0, "/opt/trn_rl_repo")

from contextlib import ExitStack

import ml_dtypes
import numpy as np

import concourse.bacc as bacc
import concourse.bass as bass
import concourse.mybir as mybir
from concourse.bass_utils import run_bass_kernel_spmd
from concourse.masks import make_identity, make_upper_triangular
from concourse.tile import TileContext

F32 = mybir.dt.float32
BF16 = mybir.dt.bfloat16
I32 = mybir.dt.int32
AF = mybir.ActivationFunctionType
OP = mybir.AluOpType

P = 128
B, S, H, I_DIM, E, TOP_K = 1, 2048, 1024, 2048, 8, 2
NTT = S // P        # 16 token tiles
NKH = H // P        # 8 k-tiles over H
NKI = I_DIM // P    # 16 i-tiles
CAP = 640           # per-expert token capacity (multiple of 128)
NCT = CAP // P
TRASH = CAP         # trash row of the (CAP+1)-row slot table
BIG = 3.0e38
N_CORES = 8

CHUNKS = [(0, 512), (512, 128)]   # token chunks for the expert matmuls
ROUTE_CHUNK = 512
N_ROUTE_CHUNKS = S // ROUTE_CHUNK


def build_program():
    nc = bacc.Bacc("TRN2", target_bir_lowering=False, debug=False,
                   num_devices=N_CORES)

    xt = nc.dram_tensor("xt", [H, S], F32, kind="ExternalInput")
    xbf = nc.dram_tensor("xbf", [S, H], BF16, kind="ExternalInput")
    wr = nc.dram_tensor("wr", [H, E], F32, kind="ExternalInput")
    br = nc.dram_tensor("br", [1, E], F32, kind="ExternalInput")
    oh = nc.dram_tensor("oh", [1, E], F32, kind="ExternalInput")
    w1 = nc.dram_tensor("w1", [H, I_DIM], BF16, kind="ExternalInput")
    w3 = nc.dram_tensor("w3", [H, I_DIM], BF16, kind="ExternalInput")
    w2 = nc.dram_tensor("w2", [I_DIM, H], BF16, kind="ExternalInput")
    # slot table of (token_id, c) fp32 pairs; runtime pre-zeroes outputs
    idxw = nc.dram_tensor("idxw", [CAP + 1, 2], F32, kind="ExternalOutput")
    yt = nc.dram_tensor("yt", [H, CAP], F32, kind="ExternalOutput")

    with TileContext(nc) as tc, ExitStack() as ctx:
        const = ctx.enter_context(tc.tile_pool(name="const", bufs=1))
        route = ctx.enter_context(tc.tile_pool(name="route", bufs=1))
        xtch_pool = ctx.enter_context(tc.tile_pool(name="xtch", bufs=2))
        scr = ctx.enter_context(tc.tile_pool(name="scr", bufs=4))
        disp = ctx.enter_context(tc.tile_pool(name="disp", bufs=1))
        wpool = ctx.enter_context(tc.tile_pool(name="wpool", bufs=1))
        xgt_pool = ctx.enter_context(tc.tile_pool(name="xgt", bufs=1))
        xg_pool = ctx.enter_context(tc.tile_pool(name="xg", bufs=3))
        ht_pool = ctx.enter_context(tc.tile_pool(name="ht", bufs=1))
        mm_pool = ctx.enter_context(tc.tile_pool(name="mm", bufs=3))

        # ---- constants ----
        id_f32 = const.tile([P, P], F32, tag="idf")
        make_identity(nc, id_f32[:])
        id_bf = const.tile([P, P], BF16, tag="idb")
        make_identity(nc, id_bf[:])
        u128 = const.tile([P, P], F32, tag="u128")  # strictly-upper ones
        make_upper_triangular(nc, u128[:], val=1.0, diag=False)
        ones_col = const.tile([1, P], F32, tag="ones")
        nc.vector.memset(ones_col[:], 1.0)
        ones128 = const.tile([P, 1], F32, tag="ones128")
        nc.vector.memset(ones128[:], 1.0)
        zeros16 = const.tile([1, NTT], F32, tag="z16")
        nc.vector.memset(zeros16[:], 0.0)
        br_bc = const.tile([P, E], F32, tag="brbc")
        nc.sync.dma_start(out=br_bc[:], in_=br[:].to_broadcast((P, E)))
        oh_bc = const.tile([P, E], F32, tag="ohbc")
        nc.sync.dma_start(out=oh_bc[:], in_=oh[:].to_broadcast((P, E)))
        wr_sb = [const.tile([P, E], F32, tag=f"wr{k}", name=f"wr_sb{k}")
                 for k in range(NKH)]
        for k in range(NKH):
            nc.sync.dma_start(out=wr_sb[k][:], in_=wr[k * P:(k + 1) * P, :])

        with tc.tile_pool(name="psr", bufs=2, space="PSUM") as psr:
            # ---- routing: logitsT [E, S] = Wr.T @ xT ----
            logT = route.tile([E, S], F32, tag="logT")
            for ch in range(N_ROUTE_CHUNKS):
                c0 = ch * ROUTE_CHUNK
                lps = psr.tile([E, ROUTE_CHUNK], F32, tag="lps")
                xts = []
                for k in range(NKH):
                    t = xtch_pool.tile([P, ROUTE_CHUNK], F32, tag=f"xtch{k}",
                                       name=f"xtch_{ch}_{k}")
                    nc.sync.dma_start(
                        out=t[:], in_=xt[k * P:(k + 1) * P, c0:c0 + ROUTE_CHUNK])
                    xts.append(t)
                for k in range(NKH):
                    nc.tensor.matmul(
                        out=lps[:], lhsT=wr_sb[k][:], rhs=xts[k][:],
                        start=(k == 0), stop=(k == NKH - 1))
                nc.vector.tensor_copy(out=logT[:, c0:c0 + ROUTE_CHUNK], in_=lps[:])

            # ---- per token tile: transpose logits, top-2 + softmax ----
            cm_all = disp.tile([P, NTT], F32, tag="cm")
            sel_all = disp.tile([P, NTT], F32, tag="sel")
            tokf = disp.tile([P, NTT], F32, tag="tokf")
            toki = scr.tile([P, NTT], I32, tag="toki")
            nc.gpsimd.iota(toki[:], pattern=[[P, NTT]], base=0,
                           channel_multiplier=1)
            nc.vector.tensor_copy(out=tokf[:], in_=toki[:])

            for t in range(NTT):
                tr = psr.tile([P, E], F32, tag="tr")
                nc.tensor.transpose(
                    out=tr[:], in_=logT[:E, t * P:(t + 1) * P],
                    identity=id_f32[:E, :E])
                l = scr.tile([P, E], F32, tag="l")
                nc.vector.tensor_tensor(out=l[:], in0=tr[:], in1=br_bc[:],
                                        op=OP.add)
                m1 = scr.tile([P, 1], F32, tag="m1")
                nc.vector.tensor_reduce(
                    out=m1[:], in_=l[:], axis=mybir.AxisListType.X, op=OP.max)
                mask1 = scr.tile([P, E], F32, tag="mask1")
                nc.vector.tensor_scalar(
                    out=mask1[:], in0=l[:], scalar1=m1[:], scalar2=None,
                    op0=OP.is_equal)
                neg = scr.tile([P, E], F32, tag="neg")
                nc.vector.tensor_scalar(
                    out=neg[:], in0=mask1[:], scalar1=BIG, scalar2=None,
                    op0=OP.mult)
                l2 = scr.tile([P, E], F32, tag="l2")
                nc.vector.tensor_sub(l2[:], l[:], neg[:])
                m2 = scr.tile([P, 1], F32, tag="m2")
                nc.vector.tensor_reduce(
                    out=m2[:], in_=l2[:], axis=mybir.AxisListType.X, op=OP.max)
                mask2 = scr.tile([P, E], F32, tag="mask2")
                nc.vector.tensor_scalar(
                    out=mask2[:], in0=l2[:], scalar1=m2[:], scalar2=None,
                    op0=OP.is_equal)
                d = scr.tile([P, 1], F32, tag="d")
                nc.vector.tensor_sub(d[:], m2[:], m1[:])
                ed = scr.tile([P, 1], F32, tag="ed")
                nc.scalar.activation(out=ed[:], in_=d[:], func=AF.Exp)
                den = scr.tile([P, 1], F32, tag="den")
                nc.vector.tensor_scalar_add(den[:], ed[:], 1.0)
                w1c = scr.tile([P, 1], F32, tag="w1c")
                nc.vector.reciprocal(w1c[:], den[:])
                w2c = scr.tile([P, 1], F32, tag="w2c")
                nc.vector.tensor_mul(w2c[:], ed[:], w1c[:])
                c1 = scr.tile([P, E], F32, tag="c1")
                nc.vector.tensor_scalar(
                    out=c1[:], in0=mask1[:], scalar1=w1c[:], scalar2=None,
                    op0=OP.mult)
                c2 = scr.tile([P, E], F32, tag="c2")
                nc.vector.tensor_scalar(
                    out=c2[:], in0=mask2[:], scalar1=w2c[:], scalar2=None,
                    op0=OP.mult)
                call = scr.tile([P, E], F32, tag="call")
                nc.vector.tensor_add(call[:], c1[:], c2[:])
                cm8 = scr.tile([P, E], F32, tag="cm8")
                nc.vector.tensor_mul(cm8[:], call[:], oh_bc[:])
                nc.vector.tensor_reduce(
                    out=cm_all[:, t:t + 1], in_=cm8[:],
                    axis=mybir.AxisListType.X, op=OP.add)
                nc.vector.tensor_scalar(
                    out=sel_all[:, t:t + 1], in0=cm_all[:, t:t + 1],
                    scalar1=0.0, scalar2=None, op0=OP.is_gt)

            # ---- compaction: dense slot per selected token ----
            excl_ps = psr.tile([P, NTT], F32, tag="excl", bufs=1)
            nc.tensor.matmul(
                out=excl_ps[:], lhsT=u128[:], rhs=sel_all[:], start=True,
                stop=True)
            excl = disp.tile([P, NTT], F32, tag="exclsb")
            nc.vector.tensor_copy(out=excl[:], in_=excl_ps[:])
            tot_ps = psr.tile([1, NTT], F32, tag="totps", bufs=1)
            nc.tensor.matmul(
                out=tot_ps[:], lhsT=ones128[:], rhs=sel_all[:], start=True,
                stop=True)
            incl = disp.tile([1, NTT], F32, tag="incl")
            nc.vector.tensor_tensor_scan(
                out=incl[:], data0=tot_ps[:], data1=zeros16[:], initial=0.0,
                op0=OP.add, op1=OP.add)
            offs = disp.tile([1, NTT], F32, tag="offs")
            nc.vector.tensor_sub(offs[:], incl[:], tot_ps[:])
            offs_ps = psr.tile([P, NTT], F32, tag="offsps", bufs=1)
            nc.tensor.matmul(
                out=offs_ps[:], lhsT=ones_col[:], rhs=offs[:], start=True,
                stop=True)
            slot = disp.tile([P, NTT], F32, tag="slot")
            nc.vector.tensor_tensor(
                out=slot[:], in0=excl[:], in1=offs_ps[:], op=OP.add)
            # unselected tokens -> TRASH row
            nc.vector.tensor_scalar_sub(slot[:], slot[:], float(TRASH))
            nc.vector.tensor_mul(slot[:], slot[:], sel_all[:])
            nc.vector.tensor_scalar_add(slot[:], slot[:], float(TRASH))
            slot_i = disp.tile([P, NTT], I32, tag="sloti")
            nc.vector.tensor_copy(out=slot_i[:], in_=slot[:])

            # (token_id, c) pairs, interleaved for an 8-byte-per-row scatter
            pair = disp.tile([P, 2 * NTT], F32, tag="pair")
            for t in range(NTT):
                nc.vector.tensor_copy(
                    out=pair[:, 2 * t:2 * t + 1], in_=tokf[:, t:t + 1])
                nc.vector.tensor_copy(
                    out=pair[:, 2 * t + 1:2 * t + 2], in_=cm_all[:, t:t + 1])
            for t in range(NTT):
                nc.gpsimd.indirect_dma_start(
                    out=idxw[:],
                    out_offset=bass.IndirectOffsetOnAxis(
                        ap=slot_i[:, t:t + 1], axis=0),
                    in_=pair[:, 2 * t:2 * t + 2],
                    in_offset=None,
                    bounds_check=TRASH,
                    oob_is_err=False)

        # ---- dispatch: gather selected x rows, transpose to [H, CAP] ----
        with tc.tile_pool(name="psd", bufs=2, space="PSUM") as psd:
            w_row = disp.tile([1, CAP], F32, tag="wrow")
            nc.sync.dma_start(out=w_row[:], in_=idxw[0:CAP, 1:2])
            wbc_sb = disp.tile([P, CAP], F32, tag="wbc")
            for c0, n in CHUNKS:
                wps = psd.tile([P, 512], F32, tag="wbcps", bufs=1)
                nc.tensor.matmul(
                    out=wps[:, :n], lhsT=ones_col[:], rhs=w_row[:, c0:c0 + n],
                    start=True, stop=True)
                nc.vector.tensor_copy(out=wbc_sb[:, c0:c0 + n], in_=wps[:, :n])

            xgt = [xgt_pool.tile([P, CAP], BF16, tag=f"xgt{k}", name=f"xgt{k}")
                   for k in range(NKH)]
            for ct in range(NCT):
                idx_f = scr.tile([P, 1], F32, tag="idxf")
                nc.sync.dma_start(
                    out=idx_f[:], in_=idxw[ct * P:(ct + 1) * P, 0:1])
                idx_i = scr.tile([P, 1], I32, tag="idxi")
                nc.vector.tensor_copy(out=idx_i[:], in_=idx_f[:])
                xg = xg_pool.tile([P, H], BF16, tag="xg")
                nc.gpsimd.indirect_dma_start(
                    out=xg[:],
                    out_offset=None,
                    in_=xbf[:],
                    in_offset=bass.IndirectOffsetOnAxis(ap=idx_i[:, 0:1], axis=0))
                for k in range(NKH):
                    tps = psd.tile([P, P], BF16, tag="xtr")
                    nc.tensor.transpose(
                        out=tps[:], in_=xg[:, k * P:(k + 1) * P],
                        identity=id_bf[:])
                    nc.vector.tensor_copy(
                        out=xgt[k][:, ct * P:(ct + 1) * P], in_=tps[:])

        # ---- expert weights (resident in SBUF) ----
        w1_sb = [wpool.tile([P, I_DIM], BF16, tag=f"w1_{k}", name=f"w1sb{k}")
                 for k in range(NKH)]
        w3_sb = [wpool.tile([P, I_DIM], BF16, tag=f"w3_{k}", name=f"w3sb{k}")
                 for k in range(NKH)]
        for k in range(NKH):
            nc.sync.dma_start(out=w1_sb[k][:], in_=w1[k * P:(k + 1) * P, :])
            nc.sync.dma_start(out=w3_sb[k][:], in_=w3[k * P:(k + 1) * P, :])
        w2_sb = [wpool.tile([P, H], BF16, tag=f"w2_{k}", name=f"w2sb{k}")
                 for k in range(NKI)]
        for k in range(NKI):
            nc.sync.dma_start(out=w2_sb[k][:], in_=w2[k * P:(k + 1) * P, :])

        # ---- expert FFN: gate/up + SwiGLU -> hT, down -> yT ----
        with tc.tile_pool(name="psm", bufs=2, space="PSUM") as psm:
            hts = [ht_pool.tile([P, CAP], BF16, tag=f"ht{i}", name=f"ht{i}")
                   for i in range(NKI)]
            for it in range(NKI):
                i0 = it * P
                for c0, n in CHUNKS:
                    gps = psm.tile([P, 512], F32, tag="gate")
                    ups = psm.tile([P, 512], F32, tag="up")
                    for k in range(NKH):
                        nc.tensor.matmul(
                            out=gps[:, :n], lhsT=w1_sb[k][:, i0:i0 + P],
                            rhs=xgt[k][:, c0:c0 + n],
                            start=(k == 0), stop=(k == NKH - 1))
                    for k in range(NKH):
                        nc.tensor.matmul(
                            out=ups[:, :n], lhsT=w3_sb[k][:, i0:i0 + P],
                            rhs=xgt[k][:, c0:c0 + n],
                            start=(k == 0), stop=(k == NKH - 1))
                    sl = mm_pool.tile([P, 512], BF16, tag="silu")
                    nc.scalar.activation(out=sl[:, :n], in_=gps[:, :n],
                                         func=AF.Sigmoid)
                    tmp = mm_pool.tile([P, 512], BF16, tag="sgate")
                    nc.vector.tensor_tensor(
                        out=tmp[:, :n], in0=sl[:, :n], in1=gps[:, :n],
                        op=OP.mult)
                    nc.vector.tensor_tensor(
                        out=hts[it][:, c0:c0 + n], in0=tmp[:, :n],
                        in1=ups[:, :n], op=OP.mult)
            for ht_i in range(NKH):
                h0 = ht_i * P
                for c0, n in CHUNKS:
                    yps = psm.tile([P, 512], F32, tag="y")
                    for k in range(NKI):
                        nc.tensor.matmul(
                            out=yps[:, :n], lhsT=w2_sb[k][:, h0:h0 + P],
                            rhs=hts[k][:, c0:c0 + n],
                            start=(k == 0), stop=(k == NKI - 1))
                    ysb = mm_pool.tile([P, 512], F32, tag="ysb")
                    nc.vector.tensor_tensor(
                        out=ysb[:, :n], in0=yps[:, :n],
                        in1=wbc_sb[:, c0:c0 + n], op=OP.mult)
                    nc.sync.dma_start(
                        out=yt[h0:h0 + P, c0:c0 + n], in_=ysb[:, :n])

    nc.compile()
    return nc


_NC_CACHE = None


def _get_program():
    global _NC_CACHE
    if _NC_CACHE is None:
        _NC_CACHE = build_program()
    return _NC_CACHE


def _prepare_in_maps(x, Wr, br, W1, W3, W2):
    x2d = np.ascontiguousarray(np.asarray(x, dtype=np.float32).reshape(S, H))
    xt = np.ascontiguousarray(x2d.T)
    xbf = x2d.astype(ml_dtypes.bfloat16)
    wr_np = np.ascontiguousarray(np.asarray(Wr, dtype=np.float32))
    br_np = np.asarray(br, dtype=np.float32).reshape(1, E)
    W1 = np.asarray(W1, dtype=np.float32)
    W3 = np.asarray(W3, dtype=np.float32)
    W2 = np.asarray(W2, dtype=np.float32)
    in_maps = []
    for e in range(N_CORES):
        oh_np = np.zeros((1, E), np.float32)
        oh_np[0, e] = 1.0
        in_maps.append({
            "xt": xt,
            "xbf": xbf,
            "wr": wr_np,
            "br": br_np,
            "oh": oh_np,
            "w1": W1[e].astype(ml_dtypes.bfloat16),
            "w3": W3[e].astype(ml_dtypes.bfloat16),
            "w2": W2[e].astype(ml_dtypes.bfloat16),
        })
    return in_maps


def _combine(results):
    out = np.zeros((S + 1, H), np.float32)
    for e in range(N_CORES):
        idxw = np.asarray(results[e]["idxw"])
        yt = np.asarray(results[e]["yt"])
        idx = idxw[:CAP, 0].astype(np.int64)
        np.add.at(out, idx, yt[:, :CAP].T)
    return out[:S].reshape(B, S, H)


def run_on_device(inputs, trace=False, trace_cores=None):
    """Run the SPMD program; returns (full_output, BassKernelResults)."""
    nc = _get_program()
    in_maps = _prepare_in_maps(**inputs)
    kwargs = {}
    if trace:
        try:
            import types

            if "antenv.axon_hooks" not in sys.modules:
                from trn_agent_boot.trn_boot import _ntff_profile_via_ctypes

                hook = _ntff_profile_via_ctypes("/opt/axon/libaxon_pjrt.so")
                mod = types.ModuleType("antenv.axon_hooks")
                mod._hook = hook
                mod.get_axon_ntff_profile_hook = lambda: mod._hook

                def _set(h):
                    mod._hook = h

                mod.set_axon_ntff_profile_hook = _set
                sys.modules["antenv.axon_hooks"] = mod
                import antenv

                antenv.axon_hooks = mod
        except Exception as exc:  # profiling unavailable -> run untraced
            print(f"trace hook install failed: {exc}", file=sys.stderr)
        kwargs = dict(trace=True,
                      trace_cores=trace_cores or list(range(N_CORES)))
    res = run_bass_kernel_spmd(nc, in_maps, list(range(N_CORES)), **kwargs)
    return _combine(res.results), res


def kernel(x, Wr, br, W1, W3, W2):
    out, _ = run_on_device(dict(x=x, Wr=Wr, br=br, W1=W1, W3=W3, W2=W2))
    return out


# revision 11
# speedup vs baseline: 1.8141x; 1.8141x over previous
"""Expert-parallel MoE (top-2 of 8 experts, SwiGLU) on 8 Trainium2 NeuronCores.

Sharding: one expert per core (W1/W3/W2 sharded on the expert axis), router
replicated. Each core, fully on-device:
  1. Routing: logitsT = Wr.T @ x.T (fp32 PE matmul), PE-transpose to [tok, 8],
     top-2 + softmax -> per-token combine weight c for this core's expert.
  2. Compaction: cross-partition prefix sum (strictly-upper-triangular ones
     matmul) assigns every selected token a dense slot; an indirect-DMA
     scatter writes (token_id, c) pairs into a DRAM slot table.
  3. Dispatch: read the token ids back, indirect-DMA gather the selected rows
     of x (bf16), PE-transpose them to put H on partitions.
  4. Expert FFN: gate/up/down matmuls in bf16 with fp32 PSUM accumulation,
     SwiGLU, scale by c, emit yT [H, CAP] fp32 plus the slot table.
Host: out[idx_e] += yt_e.T accumulated over the 8 cores (the unshard step for
expert-parallel sharding). Unfilled slots carry c = 0 so they contribute 0.
"""
import sys

sys.path.insert(0, "/opt/trn_rl_repo")

from contextlib import ExitStack

import ml_dtypes
import numpy as np

import concourse.bacc as bacc
import concourse.bass as bass
import concourse.mybir as mybir
from concourse.bass_utils import run_bass_kernel_spmd
from concourse.masks import make_identity, make_upper_triangular
from concourse.tile import TileContext

F32 = mybir.dt.float32
BF16 = mybir.dt.bfloat16
I32 = mybir.dt.int32
AF = mybir.ActivationFunctionType
OP = mybir.AluOpType

P = 128
B, S, H, I_DIM, E, TOP_K = 1, 2048, 1024, 2048, 8, 2
NTT = S // P        # 16 token tiles
NKH = H // P        # 8 k-tiles over H
NKI = I_DIM // P    # 16 i-tiles
CAP = 640           # per-expert token capacity (multiple of 128)
NCT = CAP // P
TRASH = CAP         # trash row of the (CAP+1)-row slot table
BIG = 3.0e38
N_CORES = 8

CHUNKS = [(0, 512), (512, 128)]   # token chunks for the expert matmuls
ROUTE_CHUNK = 256
N_ROUTE_CHUNKS = S // ROUTE_CHUNK


def build_program():
    nc = bacc.Bacc("TRN2", target_bir_lowering=False, debug=False,
                   num_devices=N_CORES)

    xt = nc.dram_tensor("xt", [H, S], F32, kind="ExternalInput")
    xbf = nc.dram_tensor("xbf", [S, H], BF16, kind="ExternalInput")
    wr = nc.dram_tensor("wr", [H, E], F32, kind="ExternalInput")
    br = nc.dram_tensor("br", [1, E], F32, kind="ExternalInput")
    oh = nc.dram_tensor("oh", [1, E], F32, kind="ExternalInput")
    w1 = nc.dram_tensor("w1", [H, I_DIM], BF16, kind="ExternalInput")
    w3 = nc.dram_tensor("w3", [H, I_DIM], BF16, kind="ExternalInput")
    w2 = nc.dram_tensor("w2", [I_DIM, H], BF16, kind="ExternalInput")
    # slot table: row 0 = token ids, row 1 = combine weights, per slot
    idxw = nc.dram_tensor("idxw", [2, CAP], F32, kind="ExternalOutput")
    yt = nc.dram_tensor("yt", [H, CAP], F32, kind="ExternalOutput")

    with TileContext(nc) as tc, ExitStack() as ctx:
        const = ctx.enter_context(tc.tile_pool(name="const", bufs=1))
        route = ctx.enter_context(tc.tile_pool(name="route", bufs=1))
        xtch_pool = ctx.enter_context(tc.tile_pool(name="xtch", bufs=2))
        scr = ctx.enter_context(tc.tile_pool(name="scr", bufs=4))
        disp = ctx.enter_context(tc.tile_pool(name="disp", bufs=1))
        wpool = ctx.enter_context(tc.tile_pool(name="wpool", bufs=1))
        xgt_pool = ctx.enter_context(tc.tile_pool(name="xgt", bufs=1))
        xg_pool = ctx.enter_context(tc.tile_pool(name="xg", bufs=3))
        ht_pool = ctx.enter_context(tc.tile_pool(name="ht", bufs=1))
        mm_pool = ctx.enter_context(tc.tile_pool(name="mm", bufs=2))

        # ---- constants ----
        id_f32 = const.tile([P, P], F32, tag="idf")
        make_identity(nc, id_f32[:])
        id_bf = const.tile([P, P], BF16, tag="idb")
        make_identity(nc, id_bf[:])
        u128 = const.tile([P, P], F32, tag="u128")  # strictly-upper ones
        make_upper_triangular(nc, u128[:], val=1.0, diag=False)
        ones_col = const.tile([1, P], F32, tag="ones")
        nc.vector.memset(ones_col[:], 1.0)
        ones128 = const.tile([P, 1], F32, tag="ones128")
        nc.vector.memset(ones128[:], 1.0)
        zeros16 = const.tile([1, NTT], F32, tag="z16")
        nc.vector.memset(zeros16[:], 0.0)
        iota640 = const.tile([P, CAP], F32, tag="iota640")
        ii = mm_pool.tile([P, CAP], I32, tag="iotai", bufs=1)
        nc.gpsimd.iota(ii[:], pattern=[[1, CAP]], base=0, channel_multiplier=0)
        nc.vector.tensor_copy(out=iota640[:], in_=ii[:])
        row1sel = const.tile([2, P], F32, tag="row1sel")
        nc.vector.memset(row1sel[:], 1.0)
        nc.vector.memset(row1sel[0:1, :], 0.0)
        br_bc = const.tile([P, E], F32, tag="brbc")
        nc.sync.dma_start(out=br_bc[:], in_=br[:].to_broadcast((P, E)))
        oh_bc = const.tile([P, E], F32, tag="ohbc")
        nc.sync.dma_start(out=oh_bc[:], in_=oh[:].to_broadcast((P, E)))
        wr_sb = [const.tile([P, E], F32, tag=f"wr{k}", name=f"wr_sb{k}")
                 for k in range(NKH)]
        for k in range(NKH):
            nc.sync.dma_start(out=wr_sb[k][:], in_=wr[k * P:(k + 1) * P, :])

        with tc.tile_pool(name="psr", bufs=2, space="PSUM") as psr:
            # ---- routing: logits [tok, E] per token tile, all 16 tiles
            # accumulated into one [P, NTT*E] PSUM bank. Each tile's 8-matmul
            # k-group closes before the next opens (PE runs in order), so the
            # bank-wide has_written clear on each start is harmless.
            tiles_per_chunk = ROUTE_CHUNK // P
            lt_ps = psr.tile([P, NTT * E], F32, tag="ltps", bufs=1)
            for ch in range(N_ROUTE_CHUNKS):
                c0 = ch * ROUTE_CHUNK
                xts = []
                for k in range(NKH):
                    t = xtch_pool.tile([P, ROUTE_CHUNK], F32, tag=f"xtch{k}",
                                       name=f"xtch_{ch}_{k}")
                    nc.sync.dma_start(
                        out=t[:], in_=xt[k * P:(k + 1) * P, c0:c0 + ROUTE_CHUNK])
                    xts.append(t)
                for tt in range(tiles_per_chunk):
                    t = ch * tiles_per_chunk + tt
                    for k in range(NKH):
                        nc.tensor.matmul(
                            out=lt_ps[:, t * E:(t + 1) * E],
                            lhsT=xts[k][:, tt * P:(tt + 1) * P],
                            rhs=wr_sb[k][:],
                            start=(k == 0), stop=(k == NKH - 1),
                            skip_group_check=True)

            # ---- top-2 + softmax, all 16 token tiles at once ----
            l_all = disp.tile([P, NTT * E], F32, tag="lall")
            nc.vector.tensor_tensor(
                out=l_all[:].rearrange("p (t e) -> p t e", e=E),
                in0=lt_ps[:].rearrange("p (t e) -> p t e", e=E),
                in1=br_bc[:].rearrange("p e -> p () e").to_broadcast((P, NTT, E)),
                op=OP.add)

            cm_all = disp.tile([P, NTT], F32, tag="cm")
            sel_all = disp.tile([P, NTT], F32, tag="sel")
            tokf = disp.tile([P, NTT], F32, tag="tokf")
            toki = scr.tile([P, NTT], I32, tag="toki")
            nc.gpsimd.iota(toki[:], pattern=[[P, NTT]], base=0,
                           channel_multiplier=1)
            nc.vector.tensor_copy(out=tokf[:], in_=toki[:])

            def bcast(ap):  # [P, NTT] -> [P, NTT, E] stride-0 view
                return ap.to_broadcast((P, NTT, E))

            l3 = l_all[:].rearrange("p (t e) -> p t e", e=E)
            m1 = scr.tile([P, NTT], F32, tag="m1")
            nc.vector.tensor_reduce(
                out=m1[:], in_=l3, axis=mybir.AxisListType.X, op=OP.max)
            mask1 = scr.tile([P, NTT * E], F32, tag="mask1")
            mask1_3 = mask1[:].rearrange("p (t e) -> p t e", e=E)
            nc.vector.tensor_tensor(
                out=mask1_3, in0=l3, in1=bcast(m1[:]), op=OP.is_equal)
            l2 = scr.tile([P, NTT * E], F32, tag="l2")
            l2_3 = l2[:].rearrange("p (t e) -> p t e", e=E)
            nc.vector.tensor_scalar(
                out=l2[:], in0=mask1[:], scalar1=-BIG, scalar2=None,
                op0=OP.mult)
            nc.vector.tensor_add(l2[:], l2[:], l_all[:])
            m2 = scr.tile([P, NTT], F32, tag="m2")
            nc.vector.tensor_reduce(
                out=m2[:], in_=l2_3, axis=mybir.AxisListType.X, op=OP.max)
            mask2 = scr.tile([P, NTT * E], F32, tag="mask2")
            mask2_3 = mask2[:].rearrange("p (t e) -> p t e", e=E)
            nc.vector.tensor_tensor(
                out=mask2_3, in0=l2_3, in1=bcast(m2[:]), op=OP.is_equal)
            d = scr.tile([P, NTT], F32, tag="d")
            nc.vector.tensor_sub(d[:], m2[:], m1[:])
            ed = scr.tile([P, NTT], F32, tag="ed")
            nc.scalar.activation(out=ed[:], in_=d[:], func=AF.Exp)
            den = scr.tile([P, NTT], F32, tag="den")
            nc.vector.tensor_scalar_add(den[:], ed[:], 1.0)
            w1c = scr.tile([P, NTT], F32, tag="w1c")
            nc.vector.reciprocal(w1c[:], den[:])
            w2c = scr.tile([P, NTT], F32, tag="w2c")
            nc.vector.tensor_mul(w2c[:], ed[:], w1c[:])
            # c[p,t,e] = mask1*w1 + mask2*w2; then pick this core's expert
            call = scr.tile([P, NTT * E], F32, tag="call")
            call_3 = call[:].rearrange("p (t e) -> p t e", e=E)
            nc.vector.tensor_tensor(
                out=call_3, in0=mask1_3, in1=bcast(w1c[:]), op=OP.mult)
            c2t = scr.tile([P, NTT * E], F32, tag="c2t")
            c2_3 = c2t[:].rearrange("p (t e) -> p t e", e=E)
            nc.vector.tensor_tensor(
                out=c2_3, in0=mask2_3, in1=bcast(w2c[:]), op=OP.mult)
            nc.vector.tensor_add(call[:], call[:], c2t[:])
            cm8 = scr.tile([P, NTT * E], F32, tag="cm8")
            cm8_3 = cm8[:].rearrange("p (t e) -> p t e", e=E)
            nc.vector.tensor_tensor(
                out=cm8_3, in0=call_3,
                in1=oh_bc[:].rearrange("p e -> p () e").to_broadcast((P, NTT, E)),
                op=OP.mult)
            nc.vector.tensor_reduce(
                out=cm_all[:], in_=cm8_3, axis=mybir.AxisListType.X, op=OP.add)
            nc.vector.tensor_scalar(
                out=sel_all[:], in0=cm_all[:], scalar1=0.0, scalar2=None,
                op0=OP.is_gt)

            # ---- compaction: dense slot per selected token ----
            excl_ps = psr.tile([P, NTT], F32, tag="excl", bufs=1)
            nc.tensor.matmul(
                out=excl_ps[:], lhsT=u128[:], rhs=sel_all[:], start=True,
                stop=True)
            excl = disp.tile([P, NTT], F32, tag="exclsb")
            nc.vector.tensor_copy(out=excl[:], in_=excl_ps[:])
            tot_ps = psr.tile([1, NTT], F32, tag="totps", bufs=1)
            nc.tensor.matmul(
                out=tot_ps[:], lhsT=ones128[:], rhs=sel_all[:], start=True,
                stop=True)
            incl = disp.tile([1, NTT], F32, tag="incl")
            nc.vector.tensor_tensor_scan(
                out=incl[:], data0=tot_ps[:], data1=zeros16[:], initial=0.0,
                op0=OP.add, op1=OP.add)
            offs = disp.tile([1, NTT], F32, tag="offs")
            nc.vector.tensor_sub(offs[:], incl[:], tot_ps[:])
            offs_ps = psr.tile([P, NTT], F32, tag="offsps", bufs=1)
            nc.tensor.matmul(
                out=offs_ps[:], lhsT=ones_col[:], rhs=offs[:], start=True,
                stop=True)
            slot = disp.tile([P, NTT], F32, tag="slot")
            nc.vector.tensor_tensor(
                out=slot[:], in0=excl[:], in1=offs_ps[:], op=OP.add)
            # unselected tokens -> far past any real slot
            nc.vector.tensor_scalar_sub(slot[:], slot[:], float(TRASH))
            nc.vector.tensor_mul(slot[:], slot[:], sel_all[:])
            nc.vector.tensor_scalar_add(slot[:], slot[:], float(TRASH))

            # (token_id, c) interleaved pairs
            pair = disp.tile([P, 2 * NTT], F32, tag="pair")
            pair3 = pair[:].rearrange("p (t two) -> p t two", two=2)
            nc.vector.tensor_copy(
                out=pair3[:, :, 0:1],
                in_=tokf[:].rearrange("p t -> p t ()"))
            nc.vector.tensor_copy(
                out=pair3[:, :, 1:2],
                in_=cm_all[:].rearrange("p t -> p t ()"))

            # ---- on-chip inverse permutation via one-hot matmuls ----
            # cmp_t[p, s] = (slot[p, t] == s); peT[2, s] += pair[:,t].T @ cmp_t
            # Exactly one token matches each filled slot, so the sums are
            # single-term and exact; unfilled slots come out (0, 0).
            pe_parts = []
            for c0, n in CHUNKS:
                pe_parts.append(psr.tile(
                    [2, 512], F32, tag=f"pe{c0}", bufs=1, name=f"pe_ps{c0}"))
            for t in range(NTT):
                cmp = scr.tile([P, CAP], F32, tag="cmp", bufs=2)
                nc.vector.tensor_tensor(
                    out=cmp[:], in0=slot[:, t:t + 1].to_broadcast((P, CAP)),
                    in1=iota640[:], op=OP.is_equal)
                for ci, (c0, n) in enumerate(CHUNKS):
                    nc.tensor.matmul(
                        out=pe_parts[ci][:, :n], lhsT=pair[:, 2 * t:2 * t + 2],
                        rhs=cmp[:, c0:c0 + n],
                        start=(t == 0), stop=(t == NTT - 1))
            pe_sb = disp.tile([2, CAP], F32, tag="pesb")
            for ci, (c0, n) in enumerate(CHUNKS):
                nc.vector.tensor_copy(
                    out=pe_sb[:, c0:c0 + n], in_=pe_parts[ci][:, :n])
            # ship the slot table to the host: idxw[0,:]=token ids, [1,:]=c
            nc.sync.dma_start(out=idxw[:], in_=pe_sb[:])

        # ---- dispatch: gather selected x rows, transpose to [H, CAP] ----
        with tc.tile_pool(name="psd", bufs=2, space="PSUM") as psd:
            # broadcast c over partitions: wbc[p, s] = pe_sb[1, s]
            wbc_sb = disp.tile([P, CAP], F32, tag="wbc")
            for c0, n in CHUNKS:
                wps = psd.tile([P, 512], F32, tag="wbcps", bufs=1)
                nc.tensor.matmul(
                    out=wps[:, :n], lhsT=row1sel[:], rhs=pe_sb[:, c0:c0 + n],
                    start=True, stop=True)
                nc.vector.tensor_copy(out=wbc_sb[:, c0:c0 + n], in_=wps[:, :n])

            xgt = [xgt_pool.tile([P, CAP], BF16, tag=f"xgt{k}", name=f"xgt{k}")
                   for k in range(NKH)]
            for ct in range(NCT):
                # idx per capacity tile: transpose pe_sb[:, ct*P:+P] -> [P, 2]
                trp = psd.tile([P, 2], F32, tag="idxtr")
                nc.tensor.matmul(
                    out=trp[:], lhsT=pe_sb[:2, ct * P:(ct + 1) * P],
                    rhs=id_f32[:2, :2],
                    is_transpose=True, start=True, stop=True)
                idx_i = scr.tile([P, 1], I32, tag="idxi")
                nc.vector.tensor_copy(out=idx_i[:], in_=trp[:, 0:1])
                xg = xg_pool.tile([P, H], BF16, tag="xg")
                nc.gpsimd.indirect_dma_start(
                    out=xg[:],
                    out_offset=None,
                    in_=xbf[:],
                    in_offset=bass.IndirectOffsetOnAxis(ap=idx_i[:, 0:1], axis=0))
                for k in range(NKH):
                    tps = psd.tile([P, P], BF16, tag="xtr")
                    nc.tensor.transpose(
                        out=tps[:], in_=xg[:, k * P:(k + 1) * P],
                        identity=id_bf[:])
                    nc.vector.tensor_copy(
                        out=xgt[k][:, ct * P:(ct + 1) * P], in_=tps[:])

        # ---- expert weights (resident in SBUF) ----
        w1_sb = [wpool.tile([P, I_DIM], BF16, tag=f"w1_{k}", name=f"w1sb{k}")
                 for k in range(NKH)]
        w3_sb = [wpool.tile([P, I_DIM], BF16, tag=f"w3_{k}", name=f"w3sb{k}")
                 for k in range(NKH)]
        for k in range(NKH):
            nc.scalar.dma_start(out=w1_sb[k][:], in_=w1[k * P:(k + 1) * P, :])
            nc.scalar.dma_start(out=w3_sb[k][:], in_=w3[k * P:(k + 1) * P, :])
        w2_sb = [wpool.tile([P, H], BF16, tag=f"w2_{k}", name=f"w2sb{k}")
                 for k in range(NKI)]
        for k in range(NKI):
            nc.scalar.dma_start(out=w2_sb[k][:], in_=w2[k * P:(k + 1) * P, :])

        # ---- expert FFN: gate/up + SwiGLU -> hT, down -> yT ----
        with tc.tile_pool(name="psm", bufs=2, space="PSUM") as psm:
            hts = [ht_pool.tile([P, CAP], BF16, tag=f"ht{i}", name=f"ht{i}")
                   for i in range(NKI)]
            for it in range(NKI):
                i0 = it * P
                for c0, n in CHUNKS:
                    gps = psm.tile([P, 512], F32, tag="gate")
                    ups = psm.tile([P, 512], F32, tag="up")
                    for k in range(NKH):
                        nc.tensor.matmul(
                            out=gps[:, :n], lhsT=w1_sb[k][:, i0:i0 + P],
                            rhs=xgt[k][:, c0:c0 + n],
                            start=(k == 0), stop=(k == NKH - 1))
                    for k in range(NKH):
                        nc.tensor.matmul(
                            out=ups[:, :n], lhsT=w3_sb[k][:, i0:i0 + P],
                            rhs=xgt[k][:, c0:c0 + n],
                            start=(k == 0), stop=(k == NKH - 1))
                    sl = mm_pool.tile([P, 512], BF16, tag="silu")
                    nc.scalar.activation(out=sl[:, :n], in_=gps[:, :n],
                                         func=AF.Sigmoid)
                    tmp = mm_pool.tile([P, 512], BF16, tag="sgate")
                    nc.vector.tensor_tensor(
                        out=tmp[:, :n], in0=sl[:, :n], in1=gps[:, :n],
                        op=OP.mult)
                    nc.vector.tensor_tensor(
                        out=hts[it][:, c0:c0 + n], in0=tmp[:, :n],
                        in1=ups[:, :n], op=OP.mult)
            for ht_i in range(NKH):
                h0 = ht_i * P
                for c0, n in CHUNKS:
                    yps = psm.tile([P, 512], F32, tag="y")
                    for k in range(NKI):
                        nc.tensor.matmul(
                            out=yps[:, :n], lhsT=w2_sb[k][:, h0:h0 + P],
                            rhs=hts[k][:, c0:c0 + n],
                            start=(k == 0), stop=(k == NKI - 1))
                    ysb = mm_pool.tile([P, 512], F32, tag="ysb")
                    nc.vector.tensor_tensor(
                        out=ysb[:, :n], in0=yps[:, :n],
                        in1=wbc_sb[:, c0:c0 + n], op=OP.mult)
                    nc.sync.dma_start(
                        out=yt[h0:h0 + P, c0:c0 + n], in_=ysb[:, :n])

    nc.compile()
    return nc


_NC_CACHE = None


def _get_program():
    global _NC_CACHE
    if _NC_CACHE is None:
        _NC_CACHE = build_program()
    return _NC_CACHE


def _prepare_in_maps(x, Wr, br, W1, W3, W2):
    x2d = np.ascontiguousarray(np.asarray(x, dtype=np.float32).reshape(S, H))
    xt = np.ascontiguousarray(x2d.T)
    xbf = x2d.astype(ml_dtypes.bfloat16)
    wr_np = np.ascontiguousarray(np.asarray(Wr, dtype=np.float32))
    br_np = np.asarray(br, dtype=np.float32).reshape(1, E)
    W1 = np.asarray(W1, dtype=np.float32)
    W3 = np.asarray(W3, dtype=np.float32)
    W2 = np.asarray(W2, dtype=np.float32)
    in_maps = []
    for e in range(N_CORES):
        oh_np = np.zeros((1, E), np.float32)
        oh_np[0, e] = 1.0
        in_maps.append({
            "xt": xt,
            "xbf": xbf,
            "wr": wr_np,
            "br": br_np,
            "oh": oh_np,
            "w1": W1[e].astype(ml_dtypes.bfloat16),
            "w3": W3[e].astype(ml_dtypes.bfloat16),
            "w2": W2[e].astype(ml_dtypes.bfloat16),
        })
    return in_maps


def _combine(results):
    out = np.zeros((S, H), np.float32)
    for e in range(N_CORES):
        idxw = np.asarray(results[e]["idxw"])
        yt = np.asarray(results[e]["yt"])
        idx = idxw[0, :].astype(np.int64)
        np.add.at(out, idx, yt[:, :CAP].T)
    return out.reshape(B, S, H)


def run_on_device(inputs, trace=False, trace_cores=None):
    """Run the SPMD program; returns (full_output, BassKernelResults)."""
    nc = _get_program()
    in_maps = _prepare_in_maps(**inputs)
    kwargs = {}
    if trace:
        try:
            import types

            if "antenv.axon_hooks" not in sys.modules:
                from trn_agent_boot.trn_boot import _ntff_profile_via_ctypes

                hook = _ntff_profile_via_ctypes("/opt/axon/libaxon_pjrt.so")
                mod = types.ModuleType("antenv.axon_hooks")
                mod._hook = hook
                mod.get_axon_ntff_profile_hook = lambda: mod._hook

                def _set(h):
                    mod._hook = h

                mod.set_axon_ntff_profile_hook = _set
                sys.modules["antenv.axon_hooks"] = mod
                import antenv

                antenv.axon_hooks = mod
        except Exception as exc:  # profiling unavailable -> run untraced
            print(f"trace hook install failed: {exc}", file=sys.stderr)
        kwargs = dict(trace=True,
                      trace_cores=trace_cores or list(range(N_CORES)))
    res = run_bass_kernel_spmd(nc, in_maps, list(range(N_CORES)), **kwargs)
    return _combine(res.results), res


def kernel(x, Wr, br, W1, W3, W2):
    out, _ = run_on_device(dict(x=x, Wr=Wr, br=br, W1=W1, W3=W3, W2=W2))
    return out


# revision 13
# speedup vs baseline: 1.8285x; 1.0080x over previous
"""Expert-parallel MoE (top-2 of 8 experts, SwiGLU) on 8 Trainium2 NeuronCores.

Sharding: one expert per core (W1/W3/W2 sharded on the expert axis), router
replicated. Each core, fully on-device:
  1. Routing: logitsT = Wr.T @ x.T (fp32 PE matmul), PE-transpose to [tok, 8],
     top-2 + softmax -> per-token combine weight c for this core's expert.
  2. Compaction: cross-partition prefix sum (strictly-upper-triangular ones
     matmul) assigns every selected token a dense slot; an indirect-DMA
     scatter writes (token_id, c) pairs into a DRAM slot table.
  3. Dispatch: read the token ids back, indirect-DMA gather the selected rows
     of x (bf16), PE-transpose them to put H on partitions.
  4. Expert FFN: gate/up/down matmuls in bf16 with fp32 PSUM accumulation,
     SwiGLU, scale by c, emit yT [H, CAP] fp32 plus the slot table.
Host: out[idx_e] += yt_e.T accumulated over the 8 cores (the unshard step for
expert-parallel sharding). Unfilled slots carry c = 0 so they contribute 0.
"""
import sys

sys.path.insert(0, "/opt/trn_rl_repo")

from contextlib import ExitStack

import ml_dtypes
import numpy as np

import concourse.bacc as bacc
import concourse.bass as bass
import concourse.mybir as mybir
from concourse.bass_utils import run_bass_kernel_spmd
from concourse.masks import make_identity, make_upper_triangular
from concourse.tile import TileContext

F32 = mybir.dt.float32
BF16 = mybir.dt.bfloat16
I32 = mybir.dt.int32
AF = mybir.ActivationFunctionType
OP = mybir.AluOpType

P = 128
B, S, H, I_DIM, E, TOP_K = 1, 2048, 1024, 2048, 8, 2
NTT = S // P        # 16 token tiles
NKH = H // P        # 8 k-tiles over H
NKI = I_DIM // P    # 16 i-tiles
CAP = 640           # per-expert token capacity (multiple of 128)
NCT = CAP // P
TRASH = CAP         # trash row of the (CAP+1)-row slot table
BIG = 3.0e38
N_CORES = 8

CHUNKS = [(0, 512), (512, 128)]   # token chunks for the expert matmuls
ROUTE_CHUNK = 256
N_ROUTE_CHUNKS = S // ROUTE_CHUNK


def build_program():
    nc = bacc.Bacc("TRN2", target_bir_lowering=False, debug=False,
                   num_devices=N_CORES)

    xt = nc.dram_tensor("xt", [H, S], F32, kind="ExternalInput")
    xbf = nc.dram_tensor("xbf", [S, H], BF16, kind="ExternalInput")
    wr = nc.dram_tensor("wr", [H, E], F32, kind="ExternalInput")
    br = nc.dram_tensor("br", [1, E], F32, kind="ExternalInput")
    oh = nc.dram_tensor("oh", [1, E], F32, kind="ExternalInput")
    w1 = nc.dram_tensor("w1", [H, I_DIM], BF16, kind="ExternalInput")
    w3 = nc.dram_tensor("w3", [H, I_DIM], BF16, kind="ExternalInput")
    w2 = nc.dram_tensor("w2", [I_DIM, H], BF16, kind="ExternalInput")
    # slot table: row 0 = token ids, row 1 = combine weights, per slot
    idxw = nc.dram_tensor("idxw", [2, CAP], F32, kind="ExternalOutput")
    yt = nc.dram_tensor("yt", [H, CAP], F32, kind="ExternalOutput")

    with TileContext(nc) as tc, ExitStack() as ctx:
        const = ctx.enter_context(tc.tile_pool(name="const", bufs=1))
        route = ctx.enter_context(tc.tile_pool(name="route", bufs=1))
        xtch_pool = ctx.enter_context(tc.tile_pool(name="xtch", bufs=2))
        scr = ctx.enter_context(tc.tile_pool(name="scr", bufs=4))
        disp = ctx.enter_context(tc.tile_pool(name="disp", bufs=1))
        wpool = ctx.enter_context(tc.tile_pool(name="wpool", bufs=1))
        xgt_pool = ctx.enter_context(tc.tile_pool(name="xgt", bufs=1))
        xg_pool = ctx.enter_context(tc.tile_pool(name="xg", bufs=3))
        ht_pool = ctx.enter_context(tc.tile_pool(name="ht", bufs=1))
        mm_pool = ctx.enter_context(tc.tile_pool(name="mm", bufs=2))

        # ---- constants ----
        id_f32 = const.tile([P, P], F32, tag="idf")
        make_identity(nc, id_f32[:])
        id_bf = const.tile([P, P], BF16, tag="idb")
        make_identity(nc, id_bf[:])
        u128 = const.tile([P, P], F32, tag="u128")  # strictly-upper ones
        make_upper_triangular(nc, u128[:], val=1.0, diag=False)
        ones_col = const.tile([1, P], F32, tag="ones")
        nc.vector.memset(ones_col[:], 1.0)
        ones128 = const.tile([P, 1], F32, tag="ones128")
        nc.vector.memset(ones128[:], 1.0)
        zeros16 = const.tile([1, NTT], F32, tag="z16")
        nc.vector.memset(zeros16[:], 0.0)
        iota640 = const.tile([P, CAP], F32, tag="iota640")
        ii = mm_pool.tile([P, CAP], I32, tag="iotai", bufs=1)
        nc.gpsimd.iota(ii[:], pattern=[[1, CAP]], base=0, channel_multiplier=0)
        nc.vector.tensor_copy(out=iota640[:], in_=ii[:])
        row1sel = const.tile([2, P], F32, tag="row1sel")
        nc.vector.memset(row1sel[:], 1.0)
        nc.vector.memset(row1sel[0:1, :], 0.0)
        br_bc = const.tile([P, E], F32, tag="brbc")
        nc.sync.dma_start(out=br_bc[:], in_=br[:].to_broadcast((P, E)))
        oh_bc = const.tile([P, E], F32, tag="ohbc")
        nc.sync.dma_start(out=oh_bc[:], in_=oh[:].to_broadcast((P, E)))
        wr_sb = [const.tile([P, E], F32, tag=f"wr{k}", name=f"wr_sb{k}")
                 for k in range(NKH)]
        for k in range(NKH):
            nc.sync.dma_start(out=wr_sb[k][:], in_=wr[k * P:(k + 1) * P, :])

        with tc.tile_pool(name="psr", bufs=2, space="PSUM") as psr:
            # PE warmup: keep TensorE busy from t=0 so HAM un-throttles before
            # the routing matmuls start (they wait ~15us for the first DMAs).
            warm_ps = psr.tile([P, P], BF16, tag="excl", bufs=1)
            for _ in range(160):
                nc.tensor.transpose(
                    out=warm_ps[:], in_=id_bf[:], identity=id_bf[:])

            # ---- routing: logitsT [E, S] = Wr.T @ xT (Wr stationary: the
            # 8-column weight load is nearly free; fp32 cost rides the moving
            # rows) ----
            logT = route.tile([E, S], F32, tag="logT")
            for ch in range(N_ROUTE_CHUNKS):
                c0 = ch * ROUTE_CHUNK
                lps = psr.tile([E, ROUTE_CHUNK], F32, tag="lps")
                xts = []
                for k in range(NKH):
                    t = xtch_pool.tile([P, ROUTE_CHUNK], F32, tag=f"xtch{k}",
                                       name=f"xtch_{ch}_{k}")
                    nc.sync.dma_start(
                        out=t[:], in_=xt[k * P:(k + 1) * P, c0:c0 + ROUTE_CHUNK])
                    xts.append(t)
                for k in range(NKH):
                    nc.tensor.matmul(
                        out=lps[:], lhsT=wr_sb[k][:], rhs=xts[k][:],
                        start=(k == 0), stop=(k == NKH - 1))
                nc.vector.tensor_copy(out=logT[:, c0:c0 + ROUTE_CHUNK], in_=lps[:])

            # ---- top-2 + softmax, all 16 token tiles at once ----
            # transpose each [E, 128] logit block into one [P, NTT*E] PSUM
            # bank (disjoint columns; single-shot groups, safe in order)
            trps = psr.tile([P, NTT * E], F32, tag="trps", bufs=1)
            for t in range(NTT):
                nc.tensor.matmul(
                    out=trps[:, t * E:(t + 1) * E],
                    lhsT=logT[:E, t * P:(t + 1) * P],
                    rhs=id_f32[:E, :E],
                    is_transpose=True, start=True, stop=True,
                    skip_group_check=True)
            l_all = disp.tile([P, NTT * E], F32, tag="lall")
            nc.vector.tensor_tensor(
                out=l_all[:].rearrange("p (t e) -> p t e", e=E),
                in0=trps[:].rearrange("p (t e) -> p t e", e=E),
                in1=br_bc[:].rearrange("p e -> p () e").to_broadcast((P, NTT, E)),
                op=OP.add)

            cm_all = disp.tile([P, NTT], F32, tag="cm")
            sel_all = disp.tile([P, NTT], F32, tag="sel")
            tokf = disp.tile([P, NTT], F32, tag="tokf")
            toki = scr.tile([P, NTT], I32, tag="toki")
            nc.gpsimd.iota(toki[:], pattern=[[P, NTT]], base=0,
                           channel_multiplier=1)
            nc.vector.tensor_copy(out=tokf[:], in_=toki[:])

            def bcast(ap):  # [P, NTT] -> [P, NTT, E] stride-0 view
                return ap.to_broadcast((P, NTT, E))

            l3 = l_all[:].rearrange("p (t e) -> p t e", e=E)
            m1 = scr.tile([P, NTT], F32, tag="m1")
            nc.vector.tensor_reduce(
                out=m1[:], in_=l3, axis=mybir.AxisListType.X, op=OP.max)
            mask1 = scr.tile([P, NTT * E], F32, tag="mask1")
            mask1_3 = mask1[:].rearrange("p (t e) -> p t e", e=E)
            nc.vector.tensor_tensor(
                out=mask1_3, in0=l3, in1=bcast(m1[:]), op=OP.is_equal)
            l2 = scr.tile([P, NTT * E], F32, tag="l2")
            l2_3 = l2[:].rearrange("p (t e) -> p t e", e=E)
            nc.vector.tensor_scalar(
                out=l2[:], in0=mask1[:], scalar1=-BIG, scalar2=None,
                op0=OP.mult)
            nc.vector.tensor_add(l2[:], l2[:], l_all[:])
            m2 = scr.tile([P, NTT], F32, tag="m2")
            nc.vector.tensor_reduce(
                out=m2[:], in_=l2_3, axis=mybir.AxisListType.X, op=OP.max)
            mask2 = scr.tile([P, NTT * E], F32, tag="mask2")
            mask2_3 = mask2[:].rearrange("p (t e) -> p t e", e=E)
            nc.vector.tensor_tensor(
                out=mask2_3, in0=l2_3, in1=bcast(m2[:]), op=OP.is_equal)
            d = scr.tile([P, NTT], F32, tag="d")
            nc.vector.tensor_sub(d[:], m2[:], m1[:])
            ed = scr.tile([P, NTT], F32, tag="ed")
            nc.scalar.activation(out=ed[:], in_=d[:], func=AF.Exp)
            den = scr.tile([P, NTT], F32, tag="den")
            nc.vector.tensor_scalar_add(den[:], ed[:], 1.0)
            w1c = scr.tile([P, NTT], F32, tag="w1c")
            nc.vector.reciprocal(w1c[:], den[:])
            w2c = scr.tile([P, NTT], F32, tag="w2c")
            nc.vector.tensor_mul(w2c[:], ed[:], w1c[:])
            # c[p,t,e] = mask1*w1 + mask2*w2; then pick this core's expert
            call = scr.tile([P, NTT * E], F32, tag="call")
            call_3 = call[:].rearrange("p (t e) -> p t e", e=E)
            nc.vector.tensor_tensor(
                out=call_3, in0=mask1_3, in1=bcast(w1c[:]), op=OP.mult)
            c2t = scr.tile([P, NTT * E], F32, tag="c2t")
            c2_3 = c2t[:].rearrange("p (t e) -> p t e", e=E)
            nc.vector.tensor_tensor(
                out=c2_3, in0=mask2_3, in1=bcast(w2c[:]), op=OP.mult)
            nc.vector.tensor_add(call[:], call[:], c2t[:])
            cm8 = scr.tile([P, NTT * E], F32, tag="cm8")
            cm8_3 = cm8[:].rearrange("p (t e) -> p t e", e=E)
            nc.vector.tensor_tensor(
                out=cm8_3, in0=call_3,
                in1=oh_bc[:].rearrange("p e -> p () e").to_broadcast((P, NTT, E)),
                op=OP.mult)
            nc.vector.tensor_reduce(
                out=cm_all[:], in_=cm8_3, axis=mybir.AxisListType.X, op=OP.add)
            nc.vector.tensor_scalar(
                out=sel_all[:], in0=cm_all[:], scalar1=0.0, scalar2=None,
                op0=OP.is_gt)

            # ---- compaction: dense slot per selected token ----
            excl_ps = psr.tile([P, NTT], F32, tag="excl", bufs=1)
            nc.tensor.matmul(
                out=excl_ps[:], lhsT=u128[:], rhs=sel_all[:], start=True,
                stop=True)
            excl = disp.tile([P, NTT], F32, tag="exclsb")
            nc.vector.tensor_copy(out=excl[:], in_=excl_ps[:])
            tot_ps = psr.tile([1, NTT], F32, tag="totps", bufs=1)
            nc.tensor.matmul(
                out=tot_ps[:], lhsT=ones128[:], rhs=sel_all[:], start=True,
                stop=True)
            incl = disp.tile([1, NTT], F32, tag="incl")
            nc.vector.tensor_tensor_scan(
                out=incl[:], data0=tot_ps[:], data1=zeros16[:], initial=0.0,
                op0=OP.add, op1=OP.add)
            offs = disp.tile([1, NTT], F32, tag="offs")
            nc.vector.tensor_sub(offs[:], incl[:], tot_ps[:])
            offs_ps = psr.tile([P, NTT], F32, tag="offsps", bufs=1)
            nc.tensor.matmul(
                out=offs_ps[:], lhsT=ones_col[:], rhs=offs[:], start=True,
                stop=True)
            slot = disp.tile([P, NTT], F32, tag="slot")
            nc.vector.tensor_tensor(
                out=slot[:], in0=excl[:], in1=offs_ps[:], op=OP.add)
            # unselected tokens -> far past any real slot
            nc.vector.tensor_scalar_sub(slot[:], slot[:], float(TRASH))
            nc.vector.tensor_mul(slot[:], slot[:], sel_all[:])
            nc.vector.tensor_scalar_add(slot[:], slot[:], float(TRASH))

            # (token_id, c) interleaved pairs
            pair = disp.tile([P, 2 * NTT], F32, tag="pair")
            pair3 = pair[:].rearrange("p (t two) -> p t two", two=2)
            nc.vector.tensor_copy(
                out=pair3[:, :, 0:1],
                in_=tokf[:].rearrange("p t -> p t ()"))
            nc.vector.tensor_copy(
                out=pair3[:, :, 1:2],
                in_=cm_all[:].rearrange("p t -> p t ()"))

            # ---- on-chip inverse permutation via one-hot matmuls ----
            # cmp_t[p, s] = (slot[p, t] == s); peT[2, s] += pair[:,t].T @ cmp_t
            # Exactly one token matches each filled slot, so the sums are
            # single-term and exact; unfilled slots come out (0, 0).
            pe_parts = []
            for c0, n in CHUNKS:
                pe_parts.append(psr.tile(
                    [2, 512], F32, tag=f"pe{c0}", bufs=1, name=f"pe_ps{c0}"))
            for t in range(NTT):
                cmp = scr.tile([P, CAP], F32, tag="cmp", bufs=2)
                nc.vector.tensor_tensor(
                    out=cmp[:], in0=slot[:, t:t + 1].to_broadcast((P, CAP)),
                    in1=iota640[:], op=OP.is_equal)
                for ci, (c0, n) in enumerate(CHUNKS):
                    nc.tensor.matmul(
                        out=pe_parts[ci][:, :n], lhsT=pair[:, 2 * t:2 * t + 2],
                        rhs=cmp[:, c0:c0 + n],
                        start=(t == 0), stop=(t == NTT - 1))
            pe_sb = disp.tile([2, CAP], F32, tag="pesb")
            for ci, (c0, n) in enumerate(CHUNKS):
                nc.vector.tensor_copy(
                    out=pe_sb[:, c0:c0 + n], in_=pe_parts[ci][:, :n])
            # ship the slot table to the host: idxw[0,:]=token ids, [1,:]=c
            nc.sync.dma_start(out=idxw[:], in_=pe_sb[:])

        # ---- dispatch: gather selected x rows, transpose to [H, CAP] ----
        with tc.tile_pool(name="psd", bufs=2, space="PSUM") as psd:
            # broadcast c over partitions: wbc[p, s] = pe_sb[1, s]
            wbc_sb = disp.tile([P, CAP], F32, tag="wbc")
            for c0, n in CHUNKS:
                wps = psd.tile([P, 512], F32, tag="wbcps", bufs=1)
                nc.tensor.matmul(
                    out=wps[:, :n], lhsT=row1sel[:], rhs=pe_sb[:, c0:c0 + n],
                    start=True, stop=True)
                nc.vector.tensor_copy(out=wbc_sb[:, c0:c0 + n], in_=wps[:, :n])

            xgt = [xgt_pool.tile([P, CAP], BF16, tag=f"xgt{k}", name=f"xgt{k}")
                   for k in range(NKH)]
            for ct in range(NCT):
                # idx per capacity tile: transpose pe_sb[:, ct*P:+P] -> [P, 2]
                trp = psd.tile([P, 2], F32, tag="idxtr")
                nc.tensor.matmul(
                    out=trp[:], lhsT=pe_sb[:2, ct * P:(ct + 1) * P],
                    rhs=id_f32[:2, :2],
                    is_transpose=True, start=True, stop=True)
                idx_i = scr.tile([P, 1], I32, tag="idxi")
                nc.vector.tensor_copy(out=idx_i[:], in_=trp[:, 0:1])
                xg = xg_pool.tile([P, H], BF16, tag="xg")
                nc.gpsimd.indirect_dma_start(
                    out=xg[:],
                    out_offset=None,
                    in_=xbf[:],
                    in_offset=bass.IndirectOffsetOnAxis(ap=idx_i[:, 0:1], axis=0))
                for k in range(NKH):
                    tps = psd.tile([P, P], BF16, tag="xtr")
                    nc.tensor.transpose(
                        out=tps[:], in_=xg[:, k * P:(k + 1) * P],
                        identity=id_bf[:])
                    nc.vector.tensor_copy(
                        out=xgt[k][:, ct * P:(ct + 1) * P], in_=tps[:])

        # ---- expert weights (resident in SBUF) ----
        w1_sb = [wpool.tile([P, I_DIM], BF16, tag=f"w1_{k}", name=f"w1sb{k}")
                 for k in range(NKH)]
        w3_sb = [wpool.tile([P, I_DIM], BF16, tag=f"w3_{k}", name=f"w3sb{k}")
                 for k in range(NKH)]
        for k in range(NKH):
            nc.scalar.dma_start(out=w1_sb[k][:], in_=w1[k * P:(k + 1) * P, :])
            nc.scalar.dma_start(out=w3_sb[k][:], in_=w3[k * P:(k + 1) * P, :])
        w2_sb = [wpool.tile([P, H], BF16, tag=f"w2_{k}", name=f"w2sb{k}")
                 for k in range(NKI)]
        for k in range(NKI):
            nc.scalar.dma_start(out=w2_sb[k][:], in_=w2[k * P:(k + 1) * P, :])

        # ---- expert FFN: gate/up + SwiGLU -> hT, down -> yT ----
        with tc.tile_pool(name="psm", bufs=2, space="PSUM") as psm:
            hts = [ht_pool.tile([P, CAP], BF16, tag=f"ht{i}", name=f"ht{i}")
                   for i in range(NKI)]
            for it in range(NKI):
                i0 = it * P
                for c0, n in CHUNKS:
                    gps = psm.tile([P, 512], F32, tag="gate")
                    ups = psm.tile([P, 512], F32, tag="up")
                    for k in range(NKH):
                        nc.tensor.matmul(
                            out=gps[:, :n], lhsT=w1_sb[k][:, i0:i0 + P],
                            rhs=xgt[k][:, c0:c0 + n],
                            start=(k == 0), stop=(k == NKH - 1))
                    for k in range(NKH):
                        nc.tensor.matmul(
                            out=ups[:, :n], lhsT=w3_sb[k][:, i0:i0 + P],
                            rhs=xgt[k][:, c0:c0 + n],
                            start=(k == 0), stop=(k == NKH - 1))
                    sl = mm_pool.tile([P, 512], BF16, tag="silu")
                    nc.scalar.activation(out=sl[:, :n], in_=gps[:, :n],
                                         func=AF.Sigmoid)
                    tmp = mm_pool.tile([P, 512], BF16, tag="sgate")
                    nc.vector.tensor_tensor(
                        out=tmp[:, :n], in0=sl[:, :n], in1=gps[:, :n],
                        op=OP.mult)
                    nc.vector.tensor_tensor(
                        out=hts[it][:, c0:c0 + n], in0=tmp[:, :n],
                        in1=ups[:, :n], op=OP.mult)
            for ht_i in range(NKH):
                h0 = ht_i * P
                for c0, n in CHUNKS:
                    yps = psm.tile([P, 512], F32, tag="y")
                    for k in range(NKI):
                        nc.tensor.matmul(
                            out=yps[:, :n], lhsT=w2_sb[k][:, h0:h0 + P],
                            rhs=hts[k][:, c0:c0 + n],
                            start=(k == 0), stop=(k == NKI - 1))
                    ysb = mm_pool.tile([P, 512], F32, tag="ysb")
                    nc.vector.tensor_tensor(
                        out=ysb[:, :n], in0=yps[:, :n],
                        in1=wbc_sb[:, c0:c0 + n], op=OP.mult)
                    nc.sync.dma_start(
                        out=yt[h0:h0 + P, c0:c0 + n], in_=ysb[:, :n])

    nc.compile()
    return nc


_NC_CACHE = None


def _get_program():
    global _NC_CACHE
    if _NC_CACHE is None:
        _NC_CACHE = build_program()
    return _NC_CACHE


def _prepare_in_maps(x, Wr, br, W1, W3, W2):
    x2d = np.ascontiguousarray(np.asarray(x, dtype=np.float32).reshape(S, H))
    xt = np.ascontiguousarray(x2d.T)
    xbf = x2d.astype(ml_dtypes.bfloat16)
    wr_np = np.ascontiguousarray(np.asarray(Wr, dtype=np.float32))
    br_np = np.asarray(br, dtype=np.float32).reshape(1, E)
    W1 = np.asarray(W1, dtype=np.float32)
    W3 = np.asarray(W3, dtype=np.float32)
    W2 = np.asarray(W2, dtype=np.float32)
    in_maps = []
    for e in range(N_CORES):
        oh_np = np.zeros((1, E), np.float32)
        oh_np[0, e] = 1.0
        in_maps.append({
            "xt": xt,
            "xbf": xbf,
            "wr": wr_np,
            "br": br_np,
            "oh": oh_np,
            "w1": W1[e].astype(ml_dtypes.bfloat16),
            "w3": W3[e].astype(ml_dtypes.bfloat16),
            "w2": W2[e].astype(ml_dtypes.bfloat16),
        })
    return in_maps


def _combine(results):
    out = np.zeros((S, H), np.float32)
    for e in range(N_CORES):
        idxw = np.asarray(results[e]["idxw"])
        yt = np.asarray(results[e]["yt"])
        idx = idxw[0, :].astype(np.int64)
        np.add.at(out, idx, yt[:, :CAP].T)
    return out.reshape(B, S, H)


def run_on_device(inputs, trace=False, trace_cores=None):
    """Run the SPMD program; returns (full_output, BassKernelResults)."""
    nc = _get_program()
    in_maps = _prepare_in_maps(**inputs)
    kwargs = {}
    if trace:
        try:
            import types

            if "antenv.axon_hooks" not in sys.modules:
                from trn_agent_boot.trn_boot import _ntff_profile_via_ctypes

                hook = _ntff_profile_via_ctypes("/opt/axon/libaxon_pjrt.so")
                mod = types.ModuleType("antenv.axon_hooks")
                mod._hook = hook
                mod.get_axon_ntff_profile_hook = lambda: mod._hook

                def _set(h):
                    mod._hook = h

                mod.set_axon_ntff_profile_hook = _set
                sys.modules["antenv.axon_hooks"] = mod
                import antenv

                antenv.axon_hooks = mod
        except Exception as exc:  # profiling unavailable -> run untraced
            print(f"trace hook install failed: {exc}", file=sys.stderr)
        kwargs = dict(trace=True,
                      trace_cores=trace_cores or list(range(N_CORES)))
    res = run_bass_kernel_spmd(nc, in_maps, list(range(N_CORES)), **kwargs)
    return _combine(res.results), res


def kernel(x, Wr, br, W1, W3, W2):
    out, _ = run_on_device(dict(x=x, Wr=Wr, br=br, W1=W1, W3=W3, W2=W2))
    return out


# revision 14
# speedup vs baseline: 2.0025x; 1.0951x over previous
"""Expert-parallel MoE (top-2 of 8 experts, SwiGLU) on 8 Trainium2 NeuronCores.

Sharding: one expert per core (W1/W3/W2 sharded on the expert axis), router
replicated. Each core, fully on-device:
  1. Routing: logitsT = Wr.T @ x.T (fp32 PE matmul), PE-transpose to [tok, 8],
     top-2 + softmax -> per-token combine weight c for this core's expert.
  2. Compaction: cross-partition prefix sum (strictly-upper-triangular ones
     matmul) assigns every selected token a dense slot; an indirect-DMA
     scatter writes (token_id, c) pairs into a DRAM slot table.
  3. Dispatch: read the token ids back, indirect-DMA gather the selected rows
     of x (bf16), PE-transpose them to put H on partitions.
  4. Expert FFN: gate/up/down matmuls in bf16 with fp32 PSUM accumulation,
     SwiGLU, scale by c, emit yT [H, CAP] fp32 plus the slot table.
Host: out[idx_e] += yt_e.T accumulated over the 8 cores (the unshard step for
expert-parallel sharding). Unfilled slots carry c = 0 so they contribute 0.
"""
import sys

sys.path.insert(0, "/opt/trn_rl_repo")

from contextlib import ExitStack

import ml_dtypes
import numpy as np

import concourse.bacc as bacc
import concourse.bass as bass
import concourse.mybir as mybir
from concourse.bass_utils import run_bass_kernel_spmd
from concourse.masks import make_identity, make_upper_triangular
from concourse.tile import TileContext

F32 = mybir.dt.float32
BF16 = mybir.dt.bfloat16
I32 = mybir.dt.int32
AF = mybir.ActivationFunctionType
OP = mybir.AluOpType

P = 128
B, S, H, I_DIM, E, TOP_K = 1, 2048, 1024, 2048, 8, 2
NTT = S // P        # 16 token tiles
NKH = H // P        # 8 k-tiles over H
NKI = I_DIM // P    # 16 i-tiles
CAP = 640           # per-expert token capacity (multiple of 128)
NCT = CAP // P
TRASH = CAP         # trash row of the (CAP+1)-row slot table
BIG = 3.0e38
N_CORES = 8

CHUNKS = [(0, 512), (512, 128)]   # token chunks for the expert matmuls
ROUTE_CHUNK = 256
N_ROUTE_CHUNKS = S // ROUTE_CHUNK


def build_program():
    nc = bacc.Bacc("TRN2", target_bir_lowering=False, debug=False,
                   num_devices=N_CORES)

    xt = nc.dram_tensor("xt", [H, S], F32, kind="ExternalInput")
    xbf = nc.dram_tensor("xbf", [S, H], BF16, kind="ExternalInput")
    wr = nc.dram_tensor("wr", [H, E], F32, kind="ExternalInput")
    br = nc.dram_tensor("br", [1, E], F32, kind="ExternalInput")
    oh = nc.dram_tensor("oh", [1, E], F32, kind="ExternalInput")
    w1 = nc.dram_tensor("w1", [H, I_DIM], BF16, kind="ExternalInput")
    w3 = nc.dram_tensor("w3", [H, I_DIM], BF16, kind="ExternalInput")
    w2 = nc.dram_tensor("w2", [I_DIM, H], BF16, kind="ExternalInput")
    # slot table: row 0 = token ids, row 1 = combine weights, per slot
    idxw = nc.dram_tensor("idxw", [2, CAP], F32, kind="ExternalOutput")
    yt = nc.dram_tensor("yt", [H, CAP], F32, kind="ExternalOutput")

    with TileContext(nc) as tc, ExitStack() as ctx:
        const = ctx.enter_context(tc.tile_pool(name="const", bufs=1))
        route = ctx.enter_context(tc.tile_pool(name="route", bufs=1))
        xtch_pool = ctx.enter_context(tc.tile_pool(name="xtch", bufs=2))
        scr = ctx.enter_context(tc.tile_pool(name="scr", bufs=4))
        disp = ctx.enter_context(tc.tile_pool(name="disp", bufs=1))
        wpool = ctx.enter_context(tc.tile_pool(name="wpool", bufs=1))
        xgt_pool = ctx.enter_context(tc.tile_pool(name="xgt", bufs=1))
        xg_pool = ctx.enter_context(tc.tile_pool(name="xg", bufs=3))
        ht_pool = ctx.enter_context(tc.tile_pool(name="ht", bufs=1))
        mm_pool = ctx.enter_context(tc.tile_pool(name="mm", bufs=2))

        # ---- constants ----
        id_f32 = const.tile([P, P], F32, tag="idf")
        make_identity(nc, id_f32[:])
        id_bf = const.tile([P, P], BF16, tag="idb")
        make_identity(nc, id_bf[:])
        u128 = const.tile([P, P], F32, tag="u128")  # strictly-upper ones
        make_upper_triangular(nc, u128[:], val=1.0, diag=False)
        ones_col = const.tile([1, P], F32, tag="ones")
        nc.vector.memset(ones_col[:], 1.0)
        ones128 = const.tile([P, 1], F32, tag="ones128")
        nc.vector.memset(ones128[:], 1.0)
        zeros16 = const.tile([1, NTT], F32, tag="z16")
        nc.vector.memset(zeros16[:], 0.0)
        iota640 = const.tile([P, CAP], F32, tag="iota640")
        ii = mm_pool.tile([P, CAP], I32, tag="iotai", bufs=1)
        nc.gpsimd.iota(ii[:], pattern=[[1, CAP]], base=0, channel_multiplier=0)
        nc.vector.tensor_copy(out=iota640[:], in_=ii[:])
        row1sel = const.tile([2, P], F32, tag="row1sel")
        nc.vector.memset(row1sel[:], 1.0)
        nc.vector.memset(row1sel[0:1, :], 0.0)
        br_bc = const.tile([P, E], F32, tag="brbc")
        nc.sync.dma_start(out=br_bc[:], in_=br[:].to_broadcast((P, E)))
        oh_bc = const.tile([P, E], F32, tag="ohbc")
        nc.sync.dma_start(out=oh_bc[:], in_=oh[:].to_broadcast((P, E)))
        wr_sb = [const.tile([P, E], F32, tag=f"wr{k}", name=f"wr_sb{k}")
                 for k in range(NKH)]
        for k in range(NKH):
            nc.sync.dma_start(out=wr_sb[k][:], in_=wr[k * P:(k + 1) * P, :])

        x_dma_insts = []
        with tc.tile_pool(name="psr", bufs=2, space="PSUM") as psr:
            # PE warmup: keep TensorE busy from t=0 so HAM un-throttles before
            # the routing matmuls start (they wait ~15us for the first DMAs).
            warm_ps = psr.tile([P, P], BF16, tag="excl", bufs=1)
            for _ in range(160):
                nc.tensor.transpose(
                    out=warm_ps[:], in_=id_bf[:], identity=id_bf[:])

            # ---- routing: logitsT [E, S] = Wr.T @ xT (Wr stationary: the
            # 8-column weight load is nearly free; fp32 cost rides the moving
            # rows) ----
            logT = route.tile([E, S], F32, tag="logT")
            for ch in range(N_ROUTE_CHUNKS):
                c0 = ch * ROUTE_CHUNK
                lps = psr.tile([E, ROUTE_CHUNK], F32, tag="lps")
                xts = []
                for k in range(NKH):
                    t = xtch_pool.tile([P, ROUTE_CHUNK], F32, tag=f"xtch{k}",
                                       name=f"xtch_{ch}_{k}")
                    xdma = nc.sync.dma_start(
                        out=t[:], in_=xt[k * P:(k + 1) * P, c0:c0 + ROUTE_CHUNK])
                    x_dma_insts.append(xdma)
                    xts.append(t)
                for k in range(NKH):
                    nc.tensor.matmul(
                        out=lps[:], lhsT=wr_sb[k][:], rhs=xts[k][:],
                        start=(k == 0), stop=(k == NKH - 1))
                nc.vector.tensor_copy(out=logT[:, c0:c0 + ROUTE_CHUNK], in_=lps[:])

            # ---- top-2 + softmax, all 16 token tiles at once ----
            # transpose each [E, 128] logit block into one [P, NTT*E] PSUM
            # bank (disjoint columns; single-shot groups, safe in order)
            trps = psr.tile([P, NTT * E], F32, tag="trps", bufs=1)
            for t in range(NTT):
                nc.tensor.matmul(
                    out=trps[:, t * E:(t + 1) * E],
                    lhsT=logT[:E, t * P:(t + 1) * P],
                    rhs=id_f32[:E, :E],
                    is_transpose=True, start=True, stop=True,
                    skip_group_check=True)
            l_all = disp.tile([P, NTT * E], F32, tag="lall")
            nc.vector.tensor_tensor(
                out=l_all[:].rearrange("p (t e) -> p t e", e=E),
                in0=trps[:].rearrange("p (t e) -> p t e", e=E),
                in1=br_bc[:].rearrange("p e -> p () e").to_broadcast((P, NTT, E)),
                op=OP.add)

            cm_all = disp.tile([P, NTT], F32, tag="cm")
            sel_all = disp.tile([P, NTT], F32, tag="sel")
            tokf = disp.tile([P, NTT], F32, tag="tokf")
            toki = scr.tile([P, NTT], I32, tag="toki")
            nc.gpsimd.iota(toki[:], pattern=[[P, NTT]], base=0,
                           channel_multiplier=1)
            nc.vector.tensor_copy(out=tokf[:], in_=toki[:])

            def bcast(ap):  # [P, NTT] -> [P, NTT, E] stride-0 view
                return ap.to_broadcast((P, NTT, E))

            l3 = l_all[:].rearrange("p (t e) -> p t e", e=E)
            m1 = scr.tile([P, NTT], F32, tag="m1")
            nc.vector.tensor_reduce(
                out=m1[:], in_=l3, axis=mybir.AxisListType.X, op=OP.max)
            mask1 = scr.tile([P, NTT * E], F32, tag="mask1")
            mask1_3 = mask1[:].rearrange("p (t e) -> p t e", e=E)
            nc.vector.tensor_tensor(
                out=mask1_3, in0=l3, in1=bcast(m1[:]), op=OP.is_equal)
            l2 = scr.tile([P, NTT * E], F32, tag="l2")
            l2_3 = l2[:].rearrange("p (t e) -> p t e", e=E)
            nc.vector.tensor_scalar(
                out=l2[:], in0=mask1[:], scalar1=-BIG, scalar2=None,
                op0=OP.mult)
            nc.vector.tensor_add(l2[:], l2[:], l_all[:])
            m2 = scr.tile([P, NTT], F32, tag="m2")
            nc.vector.tensor_reduce(
                out=m2[:], in_=l2_3, axis=mybir.AxisListType.X, op=OP.max)
            mask2 = scr.tile([P, NTT * E], F32, tag="mask2")
            mask2_3 = mask2[:].rearrange("p (t e) -> p t e", e=E)
            nc.vector.tensor_tensor(
                out=mask2_3, in0=l2_3, in1=bcast(m2[:]), op=OP.is_equal)
            d = scr.tile([P, NTT], F32, tag="d")
            nc.vector.tensor_sub(d[:], m2[:], m1[:])
            ed = scr.tile([P, NTT], F32, tag="ed")
            nc.scalar.activation(out=ed[:], in_=d[:], func=AF.Exp)
            den = scr.tile([P, NTT], F32, tag="den")
            nc.vector.tensor_scalar_add(den[:], ed[:], 1.0)
            w1c = scr.tile([P, NTT], F32, tag="w1c")
            nc.vector.reciprocal(w1c[:], den[:])
            w2c = scr.tile([P, NTT], F32, tag="w2c")
            nc.vector.tensor_mul(w2c[:], ed[:], w1c[:])
            # c[p,t,e] = mask1*w1 + mask2*w2; then pick this core's expert
            call = scr.tile([P, NTT * E], F32, tag="call")
            call_3 = call[:].rearrange("p (t e) -> p t e", e=E)
            nc.vector.tensor_tensor(
                out=call_3, in0=mask1_3, in1=bcast(w1c[:]), op=OP.mult)
            c2t = scr.tile([P, NTT * E], F32, tag="c2t")
            c2_3 = c2t[:].rearrange("p (t e) -> p t e", e=E)
            nc.vector.tensor_tensor(
                out=c2_3, in0=mask2_3, in1=bcast(w2c[:]), op=OP.mult)
            nc.vector.tensor_add(call[:], call[:], c2t[:])
            cm8 = scr.tile([P, NTT * E], F32, tag="cm8")
            cm8_3 = cm8[:].rearrange("p (t e) -> p t e", e=E)
            nc.vector.tensor_tensor(
                out=cm8_3, in0=call_3,
                in1=oh_bc[:].rearrange("p e -> p () e").to_broadcast((P, NTT, E)),
                op=OP.mult)
            nc.vector.tensor_reduce(
                out=cm_all[:], in_=cm8_3, axis=mybir.AxisListType.X, op=OP.add)
            nc.vector.tensor_scalar(
                out=sel_all[:], in0=cm_all[:], scalar1=0.0, scalar2=None,
                op0=OP.is_gt)

            # ---- compaction: dense slot per selected token ----
            excl_ps = psr.tile([P, NTT], F32, tag="excl", bufs=1)
            nc.tensor.matmul(
                out=excl_ps[:], lhsT=u128[:], rhs=sel_all[:], start=True,
                stop=True)
            excl = disp.tile([P, NTT], F32, tag="exclsb")
            nc.vector.tensor_copy(out=excl[:], in_=excl_ps[:])
            tot_ps = psr.tile([1, NTT], F32, tag="totps", bufs=1)
            nc.tensor.matmul(
                out=tot_ps[:], lhsT=ones128[:], rhs=sel_all[:], start=True,
                stop=True)
            incl = disp.tile([1, NTT], F32, tag="incl")
            nc.vector.tensor_tensor_scan(
                out=incl[:], data0=tot_ps[:], data1=zeros16[:], initial=0.0,
                op0=OP.add, op1=OP.add)
            offs = disp.tile([1, NTT], F32, tag="offs")
            nc.vector.tensor_sub(offs[:], incl[:], tot_ps[:])
            offs_ps = psr.tile([P, NTT], F32, tag="offsps", bufs=1)
            nc.tensor.matmul(
                out=offs_ps[:], lhsT=ones_col[:], rhs=offs[:], start=True,
                stop=True)
            slot = disp.tile([P, NTT], F32, tag="slot")
            nc.vector.tensor_tensor(
                out=slot[:], in0=excl[:], in1=offs_ps[:], op=OP.add)
            # unselected tokens -> far past any real slot
            nc.vector.tensor_scalar_sub(slot[:], slot[:], float(TRASH))
            nc.vector.tensor_mul(slot[:], slot[:], sel_all[:])
            nc.vector.tensor_scalar_add(slot[:], slot[:], float(TRASH))

            # (token_id, c) interleaved pairs
            pair = disp.tile([P, 2 * NTT], F32, tag="pair")
            pair3 = pair[:].rearrange("p (t two) -> p t two", two=2)
            nc.vector.tensor_copy(
                out=pair3[:, :, 0:1],
                in_=tokf[:].rearrange("p t -> p t ()"))
            nc.vector.tensor_copy(
                out=pair3[:, :, 1:2],
                in_=cm_all[:].rearrange("p t -> p t ()"))

            # ---- on-chip inverse permutation via one-hot matmuls ----
            # cmp_t[p, s] = (slot[p, t] == s); peT[2, s] += pair[:,t].T @ cmp_t
            # Exactly one token matches each filled slot, so the sums are
            # single-term and exact; unfilled slots come out (0, 0).
            pe_parts = []
            for c0, n in CHUNKS:
                pe_parts.append(psr.tile(
                    [2, 512], F32, tag=f"pe{c0}", bufs=1, name=f"pe_ps{c0}"))
            for t in range(NTT):
                cmp = scr.tile([P, CAP], F32, tag="cmp", bufs=2)
                nc.vector.tensor_tensor(
                    out=cmp[:], in0=slot[:, t:t + 1].to_broadcast((P, CAP)),
                    in1=iota640[:], op=OP.is_equal)
                for ci, (c0, n) in enumerate(CHUNKS):
                    nc.tensor.matmul(
                        out=pe_parts[ci][:, :n], lhsT=pair[:, 2 * t:2 * t + 2],
                        rhs=cmp[:, c0:c0 + n],
                        start=(t == 0), stop=(t == NTT - 1))
            pe_sb = disp.tile([2, CAP], F32, tag="pesb")
            for ci, (c0, n) in enumerate(CHUNKS):
                nc.vector.tensor_copy(
                    out=pe_sb[:, c0:c0 + n], in_=pe_parts[ci][:, :n])
            # ship the slot table to the host: idxw[0,:]=token ids, [1,:]=c
            nc.sync.dma_start(out=idxw[:], in_=pe_sb[:])

        # ---- dispatch: gather selected x rows, transpose to [H, CAP] ----
        with tc.tile_pool(name="psd", bufs=2, space="PSUM") as psd:
            # broadcast c over partitions: wbc[p, s] = pe_sb[1, s]
            wbc_sb = disp.tile([P, CAP], F32, tag="wbc")
            for c0, n in CHUNKS:
                wps = psd.tile([P, 512], F32, tag="wbcps", bufs=1)
                nc.tensor.matmul(
                    out=wps[:, :n], lhsT=row1sel[:], rhs=pe_sb[:, c0:c0 + n],
                    start=True, stop=True)
                nc.vector.tensor_copy(out=wbc_sb[:, c0:c0 + n], in_=wps[:, :n])

            xgt = [xgt_pool.tile([P, CAP], BF16, tag=f"xgt{k}", name=f"xgt{k}")
                   for k in range(NKH)]
            for ct in range(NCT):
                # idx per capacity tile: transpose pe_sb[:, ct*P:+P] -> [P, 2]
                trp = psd.tile([P, 2], F32, tag="idxtr")
                nc.tensor.matmul(
                    out=trp[:], lhsT=pe_sb[:2, ct * P:(ct + 1) * P],
                    rhs=id_f32[:2, :2],
                    is_transpose=True, start=True, stop=True)
                idx_i = scr.tile([P, 1], I32, tag="idxi")
                nc.vector.tensor_copy(out=idx_i[:], in_=trp[:, 0:1])
                xg = xg_pool.tile([P, H], BF16, tag="xg")
                nc.gpsimd.indirect_dma_start(
                    out=xg[:],
                    out_offset=None,
                    in_=xbf[:],
                    in_offset=bass.IndirectOffsetOnAxis(ap=idx_i[:, 0:1], axis=0))
                for k in range(NKH):
                    tps = psd.tile([P, P], BF16, tag="xtr")
                    nc.tensor.transpose(
                        out=tps[:], in_=xg[:, k * P:(k + 1) * P],
                        identity=id_bf[:])
                    nc.vector.tensor_copy(
                        out=xgt[k][:, ct * P:(ct + 1) * P], in_=tps[:])

        # ---- expert weights (resident in SBUF) ----
        from concourse.bass import _add_dep_helper
        last_x = x_dma_insts[-1]
        w1_sb = [wpool.tile([P, I_DIM], BF16, tag=f"w1_{k}", name=f"w1sb{k}")
                 for k in range(NKH)]
        w3_sb = [wpool.tile([P, I_DIM], BF16, tag=f"w3_{k}", name=f"w3sb{k}")
                 for k in range(NKH)]
        w_dmas = []
        for k in range(NKH):
            w_dmas.append(
                nc.scalar.dma_start(out=w1_sb[k][:], in_=w1[k * P:(k + 1) * P, :]))
            w_dmas.append(
                nc.scalar.dma_start(out=w3_sb[k][:], in_=w3[k * P:(k + 1) * P, :]))
        w2_sb = [wpool.tile([P, H], BF16, tag=f"w2_{k}", name=f"w2sb{k}")
                 for k in range(NKI)]
        for k in range(NKI):
            w_dmas.append(
                nc.scalar.dma_start(out=w2_sb[k][:], in_=w2[k * P:(k + 1) * P, :]))
        for wd in w_dmas:
            _add_dep_helper(wd.ins, last_x.ins, True,
                            "weights stream after xt (routing DMA priority)")

        # ---- expert FFN: gate/up + SwiGLU -> hT, down -> yT ----
        with tc.tile_pool(name="psm", bufs=2, space="PSUM") as psm:
            hts = [ht_pool.tile([P, CAP], BF16, tag=f"ht{i}", name=f"ht{i}")
                   for i in range(NKI)]
            for it in range(NKI):
                i0 = it * P
                for c0, n in CHUNKS:
                    gps = psm.tile([P, 512], F32, tag="gate")
                    ups = psm.tile([P, 512], F32, tag="up")
                    for k in range(NKH):
                        nc.tensor.matmul(
                            out=gps[:, :n], lhsT=w1_sb[k][:, i0:i0 + P],
                            rhs=xgt[k][:, c0:c0 + n],
                            start=(k == 0), stop=(k == NKH - 1))
                    for k in range(NKH):
                        nc.tensor.matmul(
                            out=ups[:, :n], lhsT=w3_sb[k][:, i0:i0 + P],
                            rhs=xgt[k][:, c0:c0 + n],
                            start=(k == 0), stop=(k == NKH - 1))
                    sl = mm_pool.tile([P, 512], BF16, tag="silu")
                    nc.scalar.activation(out=sl[:, :n], in_=gps[:, :n],
                                         func=AF.Sigmoid)
                    tmp = mm_pool.tile([P, 512], BF16, tag="sgate")
                    nc.vector.tensor_tensor(
                        out=tmp[:, :n], in0=sl[:, :n], in1=gps[:, :n],
                        op=OP.mult)
                    nc.vector.tensor_tensor(
                        out=hts[it][:, c0:c0 + n], in0=tmp[:, :n],
                        in1=ups[:, :n], op=OP.mult)
            for ht_i in range(NKH):
                h0 = ht_i * P
                for c0, n in CHUNKS:
                    yps = psm.tile([P, 512], F32, tag="y")
                    for k in range(NKI):
                        nc.tensor.matmul(
                            out=yps[:, :n], lhsT=w2_sb[k][:, h0:h0 + P],
                            rhs=hts[k][:, c0:c0 + n],
                            start=(k == 0), stop=(k == NKI - 1))
                    ysb = mm_pool.tile([P, 512], F32, tag="ysb")
                    nc.vector.tensor_tensor(
                        out=ysb[:, :n], in0=yps[:, :n],
                        in1=wbc_sb[:, c0:c0 + n], op=OP.mult)
                    nc.sync.dma_start(
                        out=yt[h0:h0 + P, c0:c0 + n], in_=ysb[:, :n])

    nc.compile()
    return nc


_NC_CACHE = None


def _get_program():
    global _NC_CACHE
    if _NC_CACHE is None:
        _NC_CACHE = build_program()
    return _NC_CACHE


def _prepare_in_maps(x, Wr, br, W1, W3, W2):
    x2d = np.ascontiguousarray(np.asarray(x, dtype=np.float32).reshape(S, H))
    xt = np.ascontiguousarray(x2d.T)
    xbf = x2d.astype(ml_dtypes.bfloat16)
    wr_np = np.ascontiguousarray(np.asarray(Wr, dtype=np.float32))
    br_np = np.asarray(br, dtype=np.float32).reshape(1, E)
    W1 = np.asarray(W1, dtype=np.float32)
    W3 = np.asarray(W3, dtype=np.float32)
    W2 = np.asarray(W2, dtype=np.float32)
    in_maps = []
    for e in range(N_CORES):
        oh_np = np.zeros((1, E), np.float32)
        oh_np[0, e] = 1.0
        in_maps.append({
            "xt": xt,
            "xbf": xbf,
            "wr": wr_np,
            "br": br_np,
            "oh": oh_np,
            "w1": W1[e].astype(ml_dtypes.bfloat16),
            "w3": W3[e].astype(ml_dtypes.bfloat16),
            "w2": W2[e].astype(ml_dtypes.bfloat16),
        })
    return in_maps


def _combine(results):
    out = np.zeros((S, H), np.float32)
    for e in range(N_CORES):
        idxw = np.asarray(results[e]["idxw"])
        yt = np.asarray(results[e]["yt"])
        idx = idxw[0, :].astype(np.int64)
        np.add.at(out, idx, yt[:, :CAP].T)
    return out.reshape(B, S, H)


def run_on_device(inputs, trace=False, trace_cores=None):
    """Run the SPMD program; returns (full_output, BassKernelResults)."""
    nc = _get_program()
    in_maps = _prepare_in_maps(**inputs)
    kwargs = {}
    if trace:
        try:
            import types

            if "antenv.axon_hooks" not in sys.modules:
                from trn_agent_boot.trn_boot import _ntff_profile_via_ctypes

                hook = _ntff_profile_via_ctypes("/opt/axon/libaxon_pjrt.so")
                mod = types.ModuleType("antenv.axon_hooks")
                mod._hook = hook
                mod.get_axon_ntff_profile_hook = lambda: mod._hook

                def _set(h):
                    mod._hook = h

                mod.set_axon_ntff_profile_hook = _set
                sys.modules["antenv.axon_hooks"] = mod
                import antenv

                antenv.axon_hooks = mod
        except Exception as exc:  # profiling unavailable -> run untraced
            print(f"trace hook install failed: {exc}", file=sys.stderr)
        kwargs = dict(trace=True,
                      trace_cores=trace_cores or list(range(N_CORES)))
    res = run_bass_kernel_spmd(nc, in_maps, list(range(N_CORES)), **kwargs)
    return _combine(res.results), res


def kernel(x, Wr, br, W1, W3, W2):
    out, _ = run_on_device(dict(x=x, Wr=Wr, br=br, W1=W1, W3=W3, W2=W2))
    return out


# revision 16
# speedup vs baseline: 2.0101x; 1.0038x over previous
"""Expert-parallel MoE (top-2 of 8 experts, SwiGLU) on 8 Trainium2 NeuronCores.

Sharding: one expert per core (W1/W3/W2 sharded on the expert axis), router
replicated. Each core, fully on-device:
  1. Routing: logitsT = Wr.T @ x.T (fp32 PE matmul), PE-transpose to [tok, 8],
     top-2 + softmax -> per-token combine weight c for this core's expert.
  2. Compaction: cross-partition prefix sum (strictly-upper-triangular ones
     matmul) assigns every selected token a dense slot; an indirect-DMA
     scatter writes (token_id, c) pairs into a DRAM slot table.
  3. Dispatch: read the token ids back, indirect-DMA gather the selected rows
     of x (bf16), PE-transpose them to put H on partitions.
  4. Expert FFN: gate/up/down matmuls in bf16 with fp32 PSUM accumulation,
     SwiGLU, scale by c, emit yT [H, CAP] fp32 plus the slot table.
Host: out[idx_e] += yt_e.T accumulated over the 8 cores (the unshard step for
expert-parallel sharding). Unfilled slots carry c = 0 so they contribute 0.
"""
import sys

sys.path.insert(0, "/opt/trn_rl_repo")

from contextlib import ExitStack

import ml_dtypes
import numpy as np

import concourse.bacc as bacc
import concourse.bass as bass
import concourse.mybir as mybir
from concourse.bass_utils import run_bass_kernel_spmd
from concourse.masks import make_identity, make_upper_triangular
from concourse.tile import TileContext

F32 = mybir.dt.float32
BF16 = mybir.dt.bfloat16
I32 = mybir.dt.int32
AF = mybir.ActivationFunctionType
OP = mybir.AluOpType

P = 128
B, S, H, I_DIM, E, TOP_K = 1, 2048, 1024, 2048, 8, 2
NTT = S // P        # 16 token tiles
NKH = H // P        # 8 k-tiles over H
NKI = I_DIM // P    # 16 i-tiles
CAP = 640           # per-expert token capacity (multiple of 128)
NCT = CAP // P
TRASH = CAP         # trash row of the (CAP+1)-row slot table
BIG = 3.0e38
N_CORES = 8

CHUNKS = [(0, 512), (512, 128)]   # token chunks for the expert matmuls
ROUTE_CHUNK = 256
N_ROUTE_CHUNKS = S // ROUTE_CHUNK


def build_program():
    nc = bacc.Bacc("TRN2", target_bir_lowering=False, debug=False,
                   num_devices=N_CORES)

    xt = nc.dram_tensor("xt", [N_ROUTE_CHUNKS * NKH * P, ROUTE_CHUNK], F32,
                    kind="ExternalInput")
    xbf = nc.dram_tensor("xbf", [S, H], BF16, kind="ExternalInput")
    wr = nc.dram_tensor("wr", [H, E], F32, kind="ExternalInput")
    br = nc.dram_tensor("br", [1, E], F32, kind="ExternalInput")
    oh = nc.dram_tensor("oh", [1, E], F32, kind="ExternalInput")
    w1 = nc.dram_tensor("w1", [H, I_DIM], BF16, kind="ExternalInput")
    w3 = nc.dram_tensor("w3", [H, I_DIM], BF16, kind="ExternalInput")
    w2 = nc.dram_tensor("w2", [I_DIM, H], BF16, kind="ExternalInput")
    # slot table: row 0 = token ids, row 1 = combine weights, per slot
    idxw = nc.dram_tensor("idxw", [4, CAP], F32, kind="ExternalOutput")
    yt = nc.dram_tensor("yt", [H, CAP], F32, kind="ExternalOutput")

    with TileContext(nc) as tc, ExitStack() as ctx:
        const = ctx.enter_context(tc.tile_pool(name="const", bufs=1))
        route = ctx.enter_context(tc.tile_pool(name="route", bufs=1))
        xtch_pool = ctx.enter_context(tc.tile_pool(name="xtch", bufs=2))
        scr = ctx.enter_context(tc.tile_pool(name="scr", bufs=4))
        disp = ctx.enter_context(tc.tile_pool(name="disp", bufs=1))
        wpool = ctx.enter_context(tc.tile_pool(name="wpool", bufs=1))
        xgt_pool = ctx.enter_context(tc.tile_pool(name="xgt", bufs=1))
        xg_pool = ctx.enter_context(tc.tile_pool(name="xg", bufs=3))
        ht_pool = ctx.enter_context(tc.tile_pool(name="ht", bufs=1))
        mm_pool = ctx.enter_context(tc.tile_pool(name="mm", bufs=2))

        # ---- constants ----
        id_f32 = const.tile([P, P], F32, tag="idf")
        make_identity(nc, id_f32[:])
        id_bf = const.tile([P, P], BF16, tag="idb")
        make_identity(nc, id_bf[:])
        u128 = const.tile([P, P], F32, tag="u128")  # strictly-upper ones
        make_upper_triangular(nc, u128[:], val=1.0, diag=False)
        ones_col = const.tile([1, P], F32, tag="ones")
        nc.vector.memset(ones_col[:], 1.0)
        ones128 = const.tile([P, 1], F32, tag="ones128")
        nc.vector.memset(ones128[:], 1.0)
        zeros16 = const.tile([1, NTT], F32, tag="z16")
        nc.vector.memset(zeros16[:], 0.0)
        iota640 = const.tile([P, CAP], F32, tag="iota640")
        ii = mm_pool.tile([P, CAP], I32, tag="iotai", bufs=1)
        nc.gpsimd.iota(ii[:], pattern=[[1, CAP]], base=0, channel_multiplier=0)
        nc.vector.tensor_copy(out=iota640[:], in_=ii[:])
        # rowsel[p, j] = 1 for p >= 2 (sums the c_hi + c_lo payload rows)
        rowsel = const.tile([4, P], F32, tag="rowsel")
        nc.gpsimd.memset(rowsel[:], 0.0)
        nc.gpsimd.affine_select(
            out=rowsel[:], in_=rowsel[:], pattern=[[0, P]],
            compare_op=OP.is_ge, fill=1.0, base=1, channel_multiplier=-1)
        br_bc = const.tile([P, E], F32, tag="brbc")
        nc.sync.dma_start(out=br_bc[:], in_=br[:].to_broadcast((P, E)))
        oh_bc = const.tile([P, E], F32, tag="ohbc")
        nc.sync.dma_start(out=oh_bc[:], in_=oh[:].to_broadcast((P, E)))
        wr_sb = [const.tile([P, E], F32, tag=f"wr{k}", name=f"wr_sb{k}")
                 for k in range(NKH)]
        for k in range(NKH):
            nc.sync.dma_start(out=wr_sb[k][:], in_=wr[k * P:(k + 1) * P, :])

        x_dma_insts = []
        with tc.tile_pool(name="psr", bufs=2, space="PSUM") as psr:
            # PE warmup: keep TensorE busy from t=0 so HAM un-throttles before
            # the routing matmuls start (they wait ~15us for the first DMAs).
            warm_ps = psr.tile([P, P], BF16, tag="excl", bufs=1)
            for _ in range(160):
                nc.tensor.transpose(
                    out=warm_ps[:], in_=id_bf[:], identity=id_bf[:])

            # ---- routing: logitsT [E, S] = Wr.T @ xT (Wr stationary: the
            # 8-column weight load is nearly free; fp32 cost rides the moving
            # rows) ----
            logT = route.tile([E, S], F32, tag="logT")
            for ch in range(N_ROUTE_CHUNKS):
                c0 = ch * ROUTE_CHUNK
                lps = psr.tile([E, ROUTE_CHUNK], F32, tag="lps")
                xts = []
                for k in range(NKH):
                    t = xtch_pool.tile([P, ROUTE_CHUNK], F32, tag=f"xtch{k}",
                                       name=f"xtch_{ch}_{k}")
                    r0 = (ch * NKH + k) * P
                    xdma = nc.sync.dma_start(
                        out=t[:], in_=xt[r0:r0 + P, :])
                    x_dma_insts.append(xdma)
                    xts.append(t)
                for k in range(NKH):
                    nc.tensor.matmul(
                        out=lps[:], lhsT=wr_sb[k][:], rhs=xts[k][:],
                        start=(k == 0), stop=(k == NKH - 1))
                nc.vector.tensor_copy(out=logT[:, c0:c0 + ROUTE_CHUNK], in_=lps[:])

            # ---- top-2 + softmax, all 16 token tiles at once ----
            # transpose each [E, 128] logit block into one [P, NTT*E] PSUM
            # bank (disjoint columns; single-shot groups, safe in order)
            trps = psr.tile([P, NTT * E], F32, tag="trps", bufs=1)
            for t in range(NTT):
                nc.tensor.matmul(
                    out=trps[:, t * E:(t + 1) * E],
                    lhsT=logT[:E, t * P:(t + 1) * P],
                    rhs=id_f32[:E, :E],
                    is_transpose=True, start=True, stop=True,
                    skip_group_check=True)
            l_all = disp.tile([P, NTT * E], F32, tag="lall")
            nc.vector.tensor_tensor(
                out=l_all[:].rearrange("p (t e) -> p t e", e=E),
                in0=trps[:].rearrange("p (t e) -> p t e", e=E),
                in1=br_bc[:].rearrange("p e -> p () e").to_broadcast((P, NTT, E)),
                op=OP.add)

            cm_all = disp.tile([P, NTT], F32, tag="cm")
            sel_all = disp.tile([P, NTT], F32, tag="sel")
            tokf = disp.tile([P, NTT], F32, tag="tokf")
            toki = scr.tile([P, NTT], I32, tag="toki")
            nc.gpsimd.iota(toki[:], pattern=[[P, NTT]], base=0,
                           channel_multiplier=1)
            nc.vector.tensor_copy(out=tokf[:], in_=toki[:])

            def bcast(ap):  # [P, NTT] -> [P, NTT, E] stride-0 view
                return ap.to_broadcast((P, NTT, E))

            l3 = l_all[:].rearrange("p (t e) -> p t e", e=E)
            m1 = scr.tile([P, NTT], F32, tag="m1")
            nc.vector.tensor_reduce(
                out=m1[:], in_=l3, axis=mybir.AxisListType.X, op=OP.max)
            mask1 = scr.tile([P, NTT * E], F32, tag="mask1")
            mask1_3 = mask1[:].rearrange("p (t e) -> p t e", e=E)
            nc.vector.tensor_tensor(
                out=mask1_3, in0=l3, in1=bcast(m1[:]), op=OP.is_equal)
            l2 = scr.tile([P, NTT * E], F32, tag="l2")
            l2_3 = l2[:].rearrange("p (t e) -> p t e", e=E)
            nc.vector.tensor_scalar(
                out=l2[:], in0=mask1[:], scalar1=-BIG, scalar2=None,
                op0=OP.mult)
            nc.vector.tensor_add(l2[:], l2[:], l_all[:])
            m2 = scr.tile([P, NTT], F32, tag="m2")
            nc.vector.tensor_reduce(
                out=m2[:], in_=l2_3, axis=mybir.AxisListType.X, op=OP.max)
            mask2 = scr.tile([P, NTT * E], F32, tag="mask2")
            mask2_3 = mask2[:].rearrange("p (t e) -> p t e", e=E)
            nc.vector.tensor_tensor(
                out=mask2_3, in0=l2_3, in1=bcast(m2[:]), op=OP.is_equal)
            d = scr.tile([P, NTT], F32, tag="d")
            nc.vector.tensor_sub(d[:], m2[:], m1[:])
            ed = scr.tile([P, NTT], F32, tag="ed")
            nc.scalar.activation(out=ed[:], in_=d[:], func=AF.Exp)
            den = scr.tile([P, NTT], F32, tag="den")
            nc.vector.tensor_scalar_add(den[:], ed[:], 1.0)
            w1c = scr.tile([P, NTT], F32, tag="w1c")
            nc.vector.reciprocal(w1c[:], den[:])
            w2c = scr.tile([P, NTT], F32, tag="w2c")
            nc.vector.tensor_mul(w2c[:], ed[:], w1c[:])
            # c[p,t,e] = mask1*w1 + mask2*w2; then pick this core's expert
            call = scr.tile([P, NTT * E], F32, tag="call")
            call_3 = call[:].rearrange("p (t e) -> p t e", e=E)
            nc.vector.tensor_tensor(
                out=call_3, in0=mask1_3, in1=bcast(w1c[:]), op=OP.mult)
            c2t = scr.tile([P, NTT * E], F32, tag="c2t")
            c2_3 = c2t[:].rearrange("p (t e) -> p t e", e=E)
            nc.vector.tensor_tensor(
                out=c2_3, in0=mask2_3, in1=bcast(w2c[:]), op=OP.mult)
            nc.vector.tensor_add(call[:], call[:], c2t[:])
            cm8 = scr.tile([P, NTT * E], F32, tag="cm8")
            cm8_3 = cm8[:].rearrange("p (t e) -> p t e", e=E)
            nc.vector.tensor_tensor(
                out=cm8_3, in0=call_3,
                in1=oh_bc[:].rearrange("p e -> p () e").to_broadcast((P, NTT, E)),
                op=OP.mult)
            nc.vector.tensor_reduce(
                out=cm_all[:], in_=cm8_3, axis=mybir.AxisListType.X, op=OP.add)
            nc.vector.tensor_scalar(
                out=sel_all[:], in0=cm_all[:], scalar1=0.0, scalar2=None,
                op0=OP.is_gt)

            # ---- compaction: dense slot per selected token ----
            excl_ps = psr.tile([P, NTT], F32, tag="excl", bufs=1)
            nc.tensor.matmul(
                out=excl_ps[:], lhsT=u128[:], rhs=sel_all[:], start=True,
                stop=True)
            excl = disp.tile([P, NTT], F32, tag="exclsb")
            nc.vector.tensor_copy(out=excl[:], in_=excl_ps[:])
            tot_ps = psr.tile([1, NTT], F32, tag="totps", bufs=1)
            nc.tensor.matmul(
                out=tot_ps[:], lhsT=ones128[:], rhs=sel_all[:], start=True,
                stop=True)
            incl = disp.tile([1, NTT], F32, tag="incl")
            nc.vector.tensor_tensor_scan(
                out=incl[:], data0=tot_ps[:], data1=zeros16[:], initial=0.0,
                op0=OP.add, op1=OP.add)
            offs = disp.tile([1, NTT], F32, tag="offs")
            nc.vector.tensor_sub(offs[:], incl[:], tot_ps[:])
            offs_ps = psr.tile([P, NTT], F32, tag="offsps", bufs=1)
            nc.tensor.matmul(
                out=offs_ps[:], lhsT=ones_col[:], rhs=offs[:], start=True,
                stop=True)
            slot = disp.tile([P, NTT], F32, tag="slot")
            nc.vector.tensor_tensor(
                out=slot[:], in0=excl[:], in1=offs_ps[:], op=OP.add)
            # unselected tokens -> far past any real slot
            nc.vector.tensor_scalar_sub(slot[:], slot[:], float(TRASH))
            nc.vector.tensor_mul(slot[:], slot[:], sel_all[:])
            nc.vector.tensor_scalar_add(slot[:], slot[:], float(TRASH))

            # payload rows per token: tile idx, partition idx, c split into
            # bf16 hi+lo halves (hi+lo is fp32-exact to ~1.5e-5)
            tcol = scr.tile([P, 1], F32, tag="tcol")
            chi = scr.tile([P, NTT], F32, tag="chi")
            clo = scr.tile([P, NTT], F32, tag="clo")
            chib = scr.tile([P, NTT], BF16, tag="chib")
            nc.vector.tensor_copy(out=chib[:], in_=cm_all[:])
            nc.vector.tensor_copy(out=chi[:], in_=chib[:])
            nc.vector.tensor_sub(clo[:], cm_all[:], chi[:])
            tvals = scr.tile([P, NTT], F32, tag="tvals")
            ti = scr.tile([P, NTT], I32, tag="ti")
            nc.gpsimd.iota(ti[:], pattern=[[1, NTT]], base=0,
                           channel_multiplier=0)
            nc.vector.tensor_copy(out=tvals[:], in_=ti[:])
            pvals = scr.tile([P, 1], I32, tag="pvals")
            nc.gpsimd.iota(pvals[:], pattern=[[1, 1]], base=0,
                           channel_multiplier=1)
            pvf = scr.tile([P, 1], F32, tag="pvf")
            nc.vector.tensor_copy(out=pvf[:], in_=pvals[:])

            pairb = disp.tile([P, 4 * NTT], BF16, tag="pairb")
            pb4 = pairb[:].rearrange("p (t four) -> p t four", four=4)
            nc.vector.tensor_copy(
                out=pb4[:, :, 0:1], in_=tvals[:].rearrange("p t -> p t ()"))
            nc.vector.tensor_copy(
                out=pb4[:, :, 1:2],
                in_=pvf[:].rearrange("p o -> p () o").to_broadcast((P, NTT, 1)))
            nc.vector.tensor_copy(
                out=pb4[:, :, 2:3], in_=chi[:].rearrange("p t -> p t ()"))
            nc.vector.tensor_copy(
                out=pb4[:, :, 3:4], in_=clo[:].rearrange("p t -> p t ()"))

            # ---- on-chip inverse permutation via one-hot matmuls ----
            # cmp_t[p, s] = (slot[p, t] == s); pe[4, s] += pairb[:,t].T @ cmp_t
            # Exactly one token matches each filled slot, so the sums are
            # single-term; ids are small ints, exact in bf16.
            pe_parts = []
            for c0, n in CHUNKS:
                pe_parts.append(psr.tile(
                    [4, 512], F32, tag=f"pe{c0}", bufs=1, name=f"pe_ps{c0}"))
            for t in range(NTT):
                cmp = scr.tile([P, CAP], BF16, tag="cmp", bufs=2)
                nc.vector.tensor_tensor(
                    out=cmp[:], in0=slot[:, t:t + 1].to_broadcast((P, CAP)),
                    in1=iota640[:], op=OP.is_equal)
                for ci, (c0, n) in enumerate(CHUNKS):
                    nc.tensor.matmul(
                        out=pe_parts[ci][:, :n], lhsT=pairb[:, 4 * t:4 * t + 4],
                        rhs=cmp[:, c0:c0 + n],
                        start=(t == 0), stop=(t == NTT - 1))
            pe_sb = disp.tile([4, CAP], F32, tag="pesb")
            for ci, (c0, n) in enumerate(CHUNKS):
                nc.vector.tensor_copy(
                    out=pe_sb[:, c0:c0 + n], in_=pe_parts[ci][:, :n])
            # ship the slot table to the host (host: idx = 128*row0 + row1)
            nc.sync.dma_start(out=idxw[:], in_=pe_sb[:])

        # ---- dispatch: gather selected x rows, transpose to [H, CAP] ----
        with tc.tile_pool(name="psd", bufs=2, space="PSUM") as psd:
            # broadcast c over partitions: wbc[p, s] = c_hi[s] + c_lo[s]
            wbc_sb = disp.tile([P, CAP], F32, tag="wbc")
            for c0, n in CHUNKS:
                wps = psd.tile([P, 512], F32, tag="wbcps", bufs=1)
                nc.tensor.matmul(
                    out=wps[:, :n], lhsT=rowsel[:], rhs=pe_sb[:, c0:c0 + n],
                    start=True, stop=True)
                nc.vector.tensor_copy(out=wbc_sb[:, c0:c0 + n], in_=wps[:, :n])

            xgt = [xgt_pool.tile([P, CAP], BF16, tag=f"xgt{k}", name=f"xgt{k}")
                   for k in range(NKH)]
            for ct in range(NCT):
                # idx per capacity tile: transpose pe_sb[:, ct*P:+P] -> [P, 4]
                trp = psd.tile([P, 4], F32, tag="idxtr")
                nc.tensor.matmul(
                    out=trp[:], lhsT=pe_sb[:4, ct * P:(ct + 1) * P],
                    rhs=id_f32[:4, :4],
                    is_transpose=True, start=True, stop=True)
                idx_f = scr.tile([P, 1], F32, tag="idxf")
                nc.vector.tensor_scalar(
                    out=idx_f[:], in0=trp[:, 0:1], scalar1=float(P),
                    scalar2=trp[:, 1:2], op0=OP.mult, op1=OP.add)
                idx_i = scr.tile([P, 1], I32, tag="idxi")
                nc.vector.tensor_copy(out=idx_i[:], in_=idx_f[:])
                xg = xg_pool.tile([P, H], BF16, tag="xg")
                nc.gpsimd.indirect_dma_start(
                    out=xg[:],
                    out_offset=None,
                    in_=xbf[:],
                    in_offset=bass.IndirectOffsetOnAxis(ap=idx_i[:, 0:1], axis=0))
                for k in range(NKH):
                    tps = psd.tile([P, P], BF16, tag="xtr")
                    nc.tensor.transpose(
                        out=tps[:], in_=xg[:, k * P:(k + 1) * P],
                        identity=id_bf[:])
                    nc.vector.tensor_copy(
                        out=xgt[k][:, ct * P:(ct + 1) * P], in_=tps[:])

        # ---- expert weights (resident in SBUF) ----
        from concourse.bass import _add_dep_helper
        last_x = x_dma_insts[-1]
        w1_sb = [wpool.tile([P, I_DIM], BF16, tag=f"w1_{k}", name=f"w1sb{k}")
                 for k in range(NKH)]
        w3_sb = [wpool.tile([P, I_DIM], BF16, tag=f"w3_{k}", name=f"w3sb{k}")
                 for k in range(NKH)]
        w_dmas = []
        for k in range(NKH):
            w_dmas.append(
                nc.scalar.dma_start(out=w1_sb[k][:], in_=w1[k * P:(k + 1) * P, :]))
            w_dmas.append(
                nc.scalar.dma_start(out=w3_sb[k][:], in_=w3[k * P:(k + 1) * P, :]))
        w2_sb = [wpool.tile([P, H], BF16, tag=f"w2_{k}", name=f"w2sb{k}")
                 for k in range(NKI)]
        for k in range(NKI):
            w_dmas.append(
                nc.scalar.dma_start(out=w2_sb[k][:], in_=w2[k * P:(k + 1) * P, :]))
        for wd in w_dmas:
            _add_dep_helper(wd.ins, last_x.ins, True,
                            "weights stream after xt (routing DMA priority)")

        # ---- expert FFN: gate/up + SwiGLU -> hT, down -> yT ----
        with tc.tile_pool(name="psm", bufs=2, space="PSUM") as psm:
            hts = [ht_pool.tile([P, CAP], BF16, tag=f"ht{i}", name=f"ht{i}")
                   for i in range(NKI)]
            for it in range(NKI):
                i0 = it * P
                for c0, n in CHUNKS:
                    gps = psm.tile([P, 512], F32, tag="gate")
                    ups = psm.tile([P, 512], F32, tag="up")
                    for k in range(NKH):
                        nc.tensor.matmul(
                            out=gps[:, :n], lhsT=w1_sb[k][:, i0:i0 + P],
                            rhs=xgt[k][:, c0:c0 + n],
                            start=(k == 0), stop=(k == NKH - 1))
                    for k in range(NKH):
                        nc.tensor.matmul(
                            out=ups[:, :n], lhsT=w3_sb[k][:, i0:i0 + P],
                            rhs=xgt[k][:, c0:c0 + n],
                            start=(k == 0), stop=(k == NKH - 1))
                    sl = mm_pool.tile([P, 512], BF16, tag="silu")
                    nc.scalar.activation(out=sl[:, :n], in_=gps[:, :n],
                                         func=AF.Sigmoid)
                    tmp = mm_pool.tile([P, 512], BF16, tag="sgate")
                    nc.vector.tensor_tensor(
                        out=tmp[:, :n], in0=sl[:, :n], in1=gps[:, :n],
                        op=OP.mult)
                    nc.vector.tensor_tensor(
                        out=hts[it][:, c0:c0 + n], in0=tmp[:, :n],
                        in1=ups[:, :n], op=OP.mult)
            for ht_i in range(NKH):
                h0 = ht_i * P
                for c0, n in CHUNKS:
                    yps = psm.tile([P, 512], F32, tag="y")
                    for k in range(NKI):
                        nc.tensor.matmul(
                            out=yps[:, :n], lhsT=w2_sb[k][:, h0:h0 + P],
                            rhs=hts[k][:, c0:c0 + n],
                            start=(k == 0), stop=(k == NKI - 1))
                    ysb = mm_pool.tile([P, 512], F32, tag="ysb")
                    nc.vector.tensor_tensor(
                        out=ysb[:, :n], in0=yps[:, :n],
                        in1=wbc_sb[:, c0:c0 + n], op=OP.mult)
                    nc.sync.dma_start(
                        out=yt[h0:h0 + P, c0:c0 + n], in_=ysb[:, :n])

    nc.compile()
    return nc


_NC_CACHE = None


def _get_program():
    global _NC_CACHE
    if _NC_CACHE is None:
        _NC_CACHE = build_program()
    return _NC_CACHE


def _prepare_in_maps(x, Wr, br, W1, W3, W2):
    x2d = np.ascontiguousarray(np.asarray(x, dtype=np.float32).reshape(S, H))
    # (k, p, ch, c) -> (ch, k, p, c): each routing chunk DMA is contiguous
    xt = np.ascontiguousarray(
        x2d.T.reshape(NKH, P, N_ROUTE_CHUNKS, ROUTE_CHUNK)
        .transpose(2, 0, 1, 3)
        .reshape(N_ROUTE_CHUNKS * NKH * P, ROUTE_CHUNK))
    xbf = x2d.astype(ml_dtypes.bfloat16)
    wr_np = np.ascontiguousarray(np.asarray(Wr, dtype=np.float32))
    br_np = np.asarray(br, dtype=np.float32).reshape(1, E)
    W1 = np.asarray(W1, dtype=np.float32)
    W3 = np.asarray(W3, dtype=np.float32)
    W2 = np.asarray(W2, dtype=np.float32)
    in_maps = []
    for e in range(N_CORES):
        oh_np = np.zeros((1, E), np.float32)
        oh_np[0, e] = 1.0
        in_maps.append({
            "xt": xt,
            "xbf": xbf,
            "wr": wr_np,
            "br": br_np,
            "oh": oh_np,
            "w1": W1[e].astype(ml_dtypes.bfloat16),
            "w3": W3[e].astype(ml_dtypes.bfloat16),
            "w2": W2[e].astype(ml_dtypes.bfloat16),
        })
    return in_maps


def _combine(results):
    out = np.zeros((S, H), np.float32)
    for e in range(N_CORES):
        idxw = np.asarray(results[e]["idxw"])
        yt = np.asarray(results[e]["yt"])
        idx = np.rint(idxw[0, :] * P + idxw[1, :]).astype(np.int64)
        np.add.at(out, idx, yt[:, :CAP].T)
    return out.reshape(B, S, H)


def run_on_device(inputs, trace=False, trace_cores=None):
    """Run the SPMD program; returns (full_output, BassKernelResults)."""
    nc = _get_program()
    in_maps = _prepare_in_maps(**inputs)
    kwargs = {}
    if trace:
        try:
            import types

            if "antenv.axon_hooks" not in sys.modules:
                from trn_agent_boot.trn_boot import _ntff_profile_via_ctypes

                hook = _ntff_profile_via_ctypes("/opt/axon/libaxon_pjrt.so")
                mod = types.ModuleType("antenv.axon_hooks")
                mod._hook = hook
                mod.get_axon_ntff_profile_hook = lambda: mod._hook

                def _set(h):
                    mod._hook = h

                mod.set_axon_ntff_profile_hook = _set
                sys.modules["antenv.axon_hooks"] = mod
                import antenv

                antenv.axon_hooks = mod
        except Exception as exc:  # profiling unavailable -> run untraced
            print(f"trace hook install failed: {exc}", file=sys.stderr)
        kwargs = dict(trace=True,
                      trace_cores=trace_cores or list(range(N_CORES)))
    res = run_bass_kernel_spmd(nc, in_maps, list(range(N_CORES)), **kwargs)
    return _combine(res.results), res


def kernel(x, Wr, br, W1, W3, W2):
    out, _ = run_on_device(dict(x=x, Wr=Wr, br=br, W1=W1, W3=W3, W2=W2))
    return out


# revision 18
# speedup vs baseline: 2.0539x; 1.0218x over previous
"""Expert-parallel MoE (top-2 of 8 experts, SwiGLU) on 8 Trainium2 NeuronCores.

Sharding: one expert per core (W1/W3/W2 sharded on the expert axis), router
replicated. Each core, fully on-device:
  1. Routing: logitsT = Wr.T @ x.T (fp32 PE matmul), PE-transpose to [tok, 8],
     top-2 + softmax -> per-token combine weight c for this core's expert.
  2. Compaction: cross-partition prefix sum (strictly-upper-triangular ones
     matmul) assigns every selected token a dense slot; an indirect-DMA
     scatter writes (token_id, c) pairs into a DRAM slot table.
  3. Dispatch: read the token ids back, indirect-DMA gather the selected rows
     of x (bf16), PE-transpose them to put H on partitions.
  4. Expert FFN: gate/up/down matmuls in bf16 with fp32 PSUM accumulation,
     SwiGLU, scale by c, emit yT [H, CAP] fp32 plus the slot table.
Host: out[idx_e] += yt_e.T accumulated over the 8 cores (the unshard step for
expert-parallel sharding). Unfilled slots carry c = 0 so they contribute 0.
"""
import sys

sys.path.insert(0, "/opt/trn_rl_repo")

from contextlib import ExitStack

import ml_dtypes
import numpy as np

import concourse.bacc as bacc
import concourse.bass as bass
import concourse.mybir as mybir
from concourse.bass_utils import run_bass_kernel_spmd
from concourse.masks import make_identity, make_upper_triangular
from concourse.tile import TileContext

F32 = mybir.dt.float32
BF16 = mybir.dt.bfloat16
I32 = mybir.dt.int32
AF = mybir.ActivationFunctionType
OP = mybir.AluOpType

P = 128
B, S, H, I_DIM, E, TOP_K = 1, 2048, 1024, 2048, 8, 2
NTT = S // P        # 16 token tiles
NKH = H // P        # 8 k-tiles over H
NKI = I_DIM // P    # 16 i-tiles
CAP = 640           # per-expert token capacity (multiple of 128)
NCT = CAP // P
TRASH = CAP         # trash row of the (CAP+1)-row slot table
BIG = 3.0e38
N_CORES = 8

CHUNKS = [(0, 512), (512, 128)]   # token chunks for the expert matmuls
ROUTE_CHUNK = 256
N_ROUTE_CHUNKS = S // ROUTE_CHUNK


def build_program():
    nc = bacc.Bacc("TRN2", target_bir_lowering=False, debug=False,
                   num_devices=N_CORES)

    xt = nc.dram_tensor("xt", [N_ROUTE_CHUNKS * NKH * P, ROUTE_CHUNK], F32,
                    kind="ExternalInput")
    xbf = nc.dram_tensor("xbf", [S, H], BF16, kind="ExternalInput")
    wr = nc.dram_tensor("wr", [H, E], F32, kind="ExternalInput")
    br = nc.dram_tensor("br", [1, E], F32, kind="ExternalInput")
    oh = nc.dram_tensor("oh", [1, E], F32, kind="ExternalInput")
    w1 = nc.dram_tensor("w1", [H, I_DIM], BF16, kind="ExternalInput")
    w3 = nc.dram_tensor("w3", [H, I_DIM], BF16, kind="ExternalInput")
    w2 = nc.dram_tensor("w2", [I_DIM, H], BF16, kind="ExternalInput")
    # slot table: row 0 = token ids, row 1 = combine weights, per slot
    idxw = nc.dram_tensor("idxw", [4, CAP], F32, kind="ExternalOutput")
    yt = nc.dram_tensor("yt", [H, CAP], F32, kind="ExternalOutput")

    with TileContext(nc) as tc, ExitStack() as ctx:
        const = ctx.enter_context(tc.tile_pool(name="const", bufs=1))
        route = ctx.enter_context(tc.tile_pool(name="route", bufs=1))
        xtch_pool = ctx.enter_context(tc.tile_pool(name="xtch", bufs=2))
        scr = ctx.enter_context(tc.tile_pool(name="scr", bufs=4))
        disp = ctx.enter_context(tc.tile_pool(name="disp", bufs=1))
        wpool = ctx.enter_context(tc.tile_pool(name="wpool", bufs=1))
        xgt_pool = ctx.enter_context(tc.tile_pool(name="xgt", bufs=1))
        xg_pool = ctx.enter_context(tc.tile_pool(name="xg", bufs=3))
        ht_pool = ctx.enter_context(tc.tile_pool(name="ht", bufs=1))
        mm_pool = ctx.enter_context(tc.tile_pool(name="mm", bufs=2))

        # ---- constants ----
        id_f32 = const.tile([P, P], F32, tag="idf")
        make_identity(nc, id_f32[:])
        id_bf = const.tile([P, P], BF16, tag="idb")
        make_identity(nc, id_bf[:])
        u128 = const.tile([P, P], F32, tag="u128")  # strictly-upper ones
        make_upper_triangular(nc, u128[:], val=1.0, diag=False)
        ones_col = const.tile([1, P], F32, tag="ones")
        nc.vector.memset(ones_col[:], 1.0)
        ones128 = const.tile([P, 1], F32, tag="ones128")
        nc.vector.memset(ones128[:], 1.0)
        zeros16 = const.tile([1, NTT], F32, tag="z16")
        nc.vector.memset(zeros16[:], 0.0)
        iota640 = const.tile([P, CAP], F32, tag="iota640")
        ii = mm_pool.tile([P, CAP], I32, tag="iotai", bufs=1)
        nc.gpsimd.iota(ii[:], pattern=[[1, CAP]], base=0, channel_multiplier=0)
        nc.vector.tensor_copy(out=iota640[:], in_=ii[:])
        # rowsel[p, j] = 1 for p >= 2 (sums the c_hi + c_lo payload rows)
        rowsel = const.tile([4, P], F32, tag="rowsel")
        nc.gpsimd.memset(rowsel[:], 0.0)
        nc.gpsimd.affine_select(
            out=rowsel[:], in_=rowsel[:], pattern=[[0, P]],
            compare_op=OP.is_ge, fill=1.0, base=1, channel_multiplier=-1)
        br_bc = const.tile([P, E], F32, tag="brbc")
        nc.sync.dma_start(out=br_bc[:], in_=br[:].to_broadcast((P, E)))
        oh_bc = const.tile([P, E], F32, tag="ohbc")
        nc.sync.dma_start(out=oh_bc[:], in_=oh[:].to_broadcast((P, E)))
        wr_sb = [const.tile([P, E], F32, tag=f"wr{k}", name=f"wr_sb{k}")
                 for k in range(NKH)]
        for k in range(NKH):
            nc.sync.dma_start(out=wr_sb[k][:], in_=wr[k * P:(k + 1) * P, :])

        x_dma_insts = []
        with tc.tile_pool(name="psr", bufs=2, space="PSUM") as psr:
            # PE warmup: keep TensorE busy from t=0 so HAM un-throttles before
            # the routing matmuls start (they wait ~15us for the first DMAs).
            warm_ps = psr.tile([P, P], BF16, tag="excl", bufs=1)
            for _ in range(160):
                nc.tensor.transpose(
                    out=warm_ps[:], in_=id_bf[:], identity=id_bf[:])

            # ---- routing: logitsT [E, S] = Wr.T @ xT (Wr stationary: the
            # 8-column weight load is nearly free; fp32 cost rides the moving
            # rows) ----
            logT = route.tile([E, S], F32, tag="logT")
            xt4 = xt[:].rearrange("(ch k p) c -> ch k p c", k=NKH, p=P)
            for ch in range(N_ROUTE_CHUNKS):
                c0 = ch * ROUTE_CHUNK
                lps = psr.tile([E, ROUTE_CHUNK], F32, tag="lps")
                xts = xtch_pool.tile([P, NKH, ROUTE_CHUNK], F32, tag="xtch",
                                     name=f"xtch_{ch}")
                xdma = nc.sync.dma_start(
                    out=xts[:],
                    in_=xt4[ch].rearrange("k p c -> p k c"))
                x_dma_insts.append(xdma)
                for k in range(NKH):
                    nc.tensor.matmul(
                        out=lps[:], lhsT=wr_sb[k][:],
                        rhs=xts[:, k, :],
                        start=(k == 0), stop=(k == NKH - 1))
                nc.vector.tensor_copy(out=logT[:, c0:c0 + ROUTE_CHUNK], in_=lps[:])

            # ---- top-2 + softmax, all 16 token tiles at once ----
            # transpose each [E, 128] logit block into one [P, NTT*E] PSUM
            # bank (disjoint columns; single-shot groups, safe in order)
            trps = psr.tile([P, NTT * E], F32, tag="trps", bufs=1)
            for t in range(NTT):
                nc.tensor.matmul(
                    out=trps[:, t * E:(t + 1) * E],
                    lhsT=logT[:E, t * P:(t + 1) * P],
                    rhs=id_f32[:E, :E],
                    is_transpose=True, start=True, stop=True,
                    skip_group_check=True)
            l_all = disp.tile([P, NTT * E], F32, tag="lall")
            nc.vector.tensor_tensor(
                out=l_all[:].rearrange("p (t e) -> p t e", e=E),
                in0=trps[:].rearrange("p (t e) -> p t e", e=E),
                in1=br_bc[:].rearrange("p e -> p () e").to_broadcast((P, NTT, E)),
                op=OP.add)

            cm_all = disp.tile([P, NTT], F32, tag="cm")
            sel_all = disp.tile([P, NTT], F32, tag="sel")
            tokf = disp.tile([P, NTT], F32, tag="tokf")
            toki = scr.tile([P, NTT], I32, tag="toki")
            nc.gpsimd.iota(toki[:], pattern=[[P, NTT]], base=0,
                           channel_multiplier=1)
            nc.vector.tensor_copy(out=tokf[:], in_=toki[:])

            def bcast(ap):  # [P, NTT] -> [P, NTT, E] stride-0 view
                return ap.to_broadcast((P, NTT, E))

            l3 = l_all[:].rearrange("p (t e) -> p t e", e=E)
            m1 = scr.tile([P, NTT], F32, tag="m1")
            nc.vector.tensor_reduce(
                out=m1[:], in_=l3, axis=mybir.AxisListType.X, op=OP.max)
            mask1 = scr.tile([P, NTT * E], F32, tag="mask1")
            mask1_3 = mask1[:].rearrange("p (t e) -> p t e", e=E)
            nc.vector.tensor_tensor(
                out=mask1_3, in0=l3, in1=bcast(m1[:]), op=OP.is_equal)
            l2 = scr.tile([P, NTT * E], F32, tag="l2")
            l2_3 = l2[:].rearrange("p (t e) -> p t e", e=E)
            nc.vector.tensor_scalar(
                out=l2[:], in0=mask1[:], scalar1=-BIG, scalar2=None,
                op0=OP.mult)
            nc.vector.tensor_add(l2[:], l2[:], l_all[:])
            m2 = scr.tile([P, NTT], F32, tag="m2")
            nc.vector.tensor_reduce(
                out=m2[:], in_=l2_3, axis=mybir.AxisListType.X, op=OP.max)
            mask2 = scr.tile([P, NTT * E], F32, tag="mask2")
            mask2_3 = mask2[:].rearrange("p (t e) -> p t e", e=E)
            nc.vector.tensor_tensor(
                out=mask2_3, in0=l2_3, in1=bcast(m2[:]), op=OP.is_equal)
            d = scr.tile([P, NTT], F32, tag="d")
            nc.vector.tensor_sub(d[:], m2[:], m1[:])
            ed = scr.tile([P, NTT], F32, tag="ed")
            nc.scalar.activation(out=ed[:], in_=d[:], func=AF.Exp)
            den = scr.tile([P, NTT], F32, tag="den")
            nc.vector.tensor_scalar_add(den[:], ed[:], 1.0)
            w1c = scr.tile([P, NTT], F32, tag="w1c")
            nc.vector.reciprocal(w1c[:], den[:])
            w2c = scr.tile([P, NTT], F32, tag="w2c")
            nc.vector.tensor_mul(w2c[:], ed[:], w1c[:])
            # c[p,t,e] = mask1*w1 + mask2*w2; then pick this core's expert
            call = scr.tile([P, NTT * E], F32, tag="call")
            call_3 = call[:].rearrange("p (t e) -> p t e", e=E)
            nc.vector.tensor_tensor(
                out=call_3, in0=mask1_3, in1=bcast(w1c[:]), op=OP.mult)
            c2t = scr.tile([P, NTT * E], F32, tag="c2t")
            c2_3 = c2t[:].rearrange("p (t e) -> p t e", e=E)
            nc.vector.tensor_tensor(
                out=c2_3, in0=mask2_3, in1=bcast(w2c[:]), op=OP.mult)
            nc.vector.tensor_add(call[:], call[:], c2t[:])
            cm8 = scr.tile([P, NTT * E], F32, tag="cm8")
            cm8_3 = cm8[:].rearrange("p (t e) -> p t e", e=E)
            nc.vector.tensor_tensor(
                out=cm8_3, in0=call_3,
                in1=oh_bc[:].rearrange("p e -> p () e").to_broadcast((P, NTT, E)),
                op=OP.mult)
            nc.vector.tensor_reduce(
                out=cm_all[:], in_=cm8_3, axis=mybir.AxisListType.X, op=OP.add)
            nc.vector.tensor_scalar(
                out=sel_all[:], in0=cm_all[:], scalar1=0.0, scalar2=None,
                op0=OP.is_gt)

            # ---- compaction: dense slot per selected token ----
            excl_ps = psr.tile([P, NTT], F32, tag="excl", bufs=1)
            nc.tensor.matmul(
                out=excl_ps[:], lhsT=u128[:], rhs=sel_all[:], start=True,
                stop=True)
            excl = disp.tile([P, NTT], F32, tag="exclsb")
            nc.vector.tensor_copy(out=excl[:], in_=excl_ps[:])
            tot_ps = psr.tile([1, NTT], F32, tag="totps", bufs=1)
            nc.tensor.matmul(
                out=tot_ps[:], lhsT=ones128[:], rhs=sel_all[:], start=True,
                stop=True)
            incl = disp.tile([1, NTT], F32, tag="incl")
            nc.vector.tensor_tensor_scan(
                out=incl[:], data0=tot_ps[:], data1=zeros16[:], initial=0.0,
                op0=OP.add, op1=OP.add)
            offs = disp.tile([1, NTT], F32, tag="offs")
            nc.vector.tensor_sub(offs[:], incl[:], tot_ps[:])
            offs_ps = psr.tile([P, NTT], F32, tag="offsps", bufs=1)
            nc.tensor.matmul(
                out=offs_ps[:], lhsT=ones_col[:], rhs=offs[:], start=True,
                stop=True)
            slot = disp.tile([P, NTT], F32, tag="slot")
            nc.vector.tensor_tensor(
                out=slot[:], in0=excl[:], in1=offs_ps[:], op=OP.add)
            # unselected tokens -> far past any real slot
            nc.vector.tensor_scalar_sub(slot[:], slot[:], float(TRASH))
            nc.vector.tensor_mul(slot[:], slot[:], sel_all[:])
            nc.vector.tensor_scalar_add(slot[:], slot[:], float(TRASH))

            # payload rows per token: tile idx, partition idx, c split into
            # bf16 hi+lo halves (hi+lo is fp32-exact to ~1.5e-5)
            tcol = scr.tile([P, 1], F32, tag="tcol")
            chi = scr.tile([P, NTT], F32, tag="chi")
            clo = scr.tile([P, NTT], F32, tag="clo")
            chib = scr.tile([P, NTT], BF16, tag="chib")
            nc.vector.tensor_copy(out=chib[:], in_=cm_all[:])
            nc.vector.tensor_copy(out=chi[:], in_=chib[:])
            nc.vector.tensor_sub(clo[:], cm_all[:], chi[:])
            tvals = scr.tile([P, NTT], F32, tag="tvals")
            ti = scr.tile([P, NTT], I32, tag="ti")
            nc.gpsimd.iota(ti[:], pattern=[[1, NTT]], base=0,
                           channel_multiplier=0)
            nc.vector.tensor_copy(out=tvals[:], in_=ti[:])
            pvals = scr.tile([P, 1], I32, tag="pvals")
            nc.gpsimd.iota(pvals[:], pattern=[[1, 1]], base=0,
                           channel_multiplier=1)
            pvf = scr.tile([P, 1], F32, tag="pvf")
            nc.vector.tensor_copy(out=pvf[:], in_=pvals[:])

            pairb = disp.tile([P, 4 * NTT], BF16, tag="pairb")
            pb4 = pairb[:].rearrange("p (t four) -> p t four", four=4)
            nc.vector.tensor_copy(
                out=pb4[:, :, 0:1], in_=tvals[:].rearrange("p t -> p t ()"))
            nc.vector.tensor_copy(
                out=pb4[:, :, 1:2],
                in_=pvf[:].rearrange("p o -> p () o").to_broadcast((P, NTT, 1)))
            nc.vector.tensor_copy(
                out=pb4[:, :, 2:3], in_=chi[:].rearrange("p t -> p t ()"))
            nc.vector.tensor_copy(
                out=pb4[:, :, 3:4], in_=clo[:].rearrange("p t -> p t ()"))

            # ---- on-chip inverse permutation via one-hot matmuls ----
            # cmp_t[p, s] = (slot[p, t] == s); pe[4, s] += pairb[:,t].T @ cmp_t
            # Exactly one token matches each filled slot, so the sums are
            # single-term; ids are small ints, exact in bf16.
            pe_parts = []
            for c0, n in CHUNKS:
                pe_parts.append(psr.tile(
                    [4, 512], F32, tag=f"pe{c0}", bufs=1, name=f"pe_ps{c0}"))
            for t in range(NTT):
                cmp = scr.tile([P, CAP], BF16, tag="cmp", bufs=2)
                nc.vector.tensor_tensor(
                    out=cmp[:], in0=slot[:, t:t + 1].to_broadcast((P, CAP)),
                    in1=iota640[:], op=OP.is_equal)
                for ci, (c0, n) in enumerate(CHUNKS):
                    nc.tensor.matmul(
                        out=pe_parts[ci][:, :n], lhsT=pairb[:, 4 * t:4 * t + 4],
                        rhs=cmp[:, c0:c0 + n],
                        start=(t == 0), stop=(t == NTT - 1))
            pe_sb = disp.tile([4, CAP], F32, tag="pesb")
            for ci, (c0, n) in enumerate(CHUNKS):
                nc.vector.tensor_copy(
                    out=pe_sb[:, c0:c0 + n], in_=pe_parts[ci][:, :n])
            # ship the slot table to the host (host: idx = 128*row0 + row1)
            nc.sync.dma_start(out=idxw[:], in_=pe_sb[:])

        # ---- dispatch: gather selected x rows, transpose to [H, CAP] ----
        with tc.tile_pool(name="psd", bufs=2, space="PSUM") as psd:
            # broadcast c over partitions: wbc[p, s] = c_hi[s] + c_lo[s]
            wbc_sb = disp.tile([P, CAP], F32, tag="wbc")
            for c0, n in CHUNKS:
                wps = psd.tile([P, 512], F32, tag="wbcps", bufs=1)
                nc.tensor.matmul(
                    out=wps[:, :n], lhsT=rowsel[:], rhs=pe_sb[:, c0:c0 + n],
                    start=True, stop=True)
                nc.vector.tensor_copy(out=wbc_sb[:, c0:c0 + n], in_=wps[:, :n])

            xgt = [xgt_pool.tile([P, CAP], BF16, tag=f"xgt{k}", name=f"xgt{k}")
                   for k in range(NKH)]
            for ct in range(NCT):
                # idx per capacity tile: transpose pe_sb[:, ct*P:+P] -> [P, 4]
                trp = psd.tile([P, 4], F32, tag="idxtr")
                nc.tensor.matmul(
                    out=trp[:], lhsT=pe_sb[:4, ct * P:(ct + 1) * P],
                    rhs=id_f32[:4, :4],
                    is_transpose=True, start=True, stop=True)
                idx_f = scr.tile([P, 1], F32, tag="idxf")
                nc.vector.tensor_scalar(
                    out=idx_f[:], in0=trp[:, 0:1], scalar1=float(P),
                    scalar2=trp[:, 1:2], op0=OP.mult, op1=OP.add)
                idx_i = scr.tile([P, 1], I32, tag="idxi")
                nc.vector.tensor_copy(out=idx_i[:], in_=idx_f[:])
                xg = xg_pool.tile([P, H], BF16, tag="xg")
                nc.gpsimd.indirect_dma_start(
                    out=xg[:],
                    out_offset=None,
                    in_=xbf[:],
                    in_offset=bass.IndirectOffsetOnAxis(ap=idx_i[:, 0:1], axis=0))
                for k in range(NKH):
                    tps = psd.tile([P, P], BF16, tag="xtr")
                    nc.tensor.transpose(
                        out=tps[:], in_=xg[:, k * P:(k + 1) * P],
                        identity=id_bf[:])
                    nc.vector.tensor_copy(
                        out=xgt[k][:, ct * P:(ct + 1) * P], in_=tps[:])

        # ---- expert weights (resident in SBUF) ----
        from concourse.bass import _add_dep_helper
        last_x = x_dma_insts[-1]
        w1_all = wpool.tile([P, NKH, I_DIM], BF16, tag="w1a")
        w3_all = wpool.tile([P, NKH, I_DIM], BF16, tag="w3a")
        w2_all = wpool.tile([P, NKI, H], BF16, tag="w2a")
        w_dmas = []
        w1v = w1[:].rearrange("(k p) i -> p k i", p=P)
        w3v = w3[:].rearrange("(k p) i -> p k i", p=P)
        w2v = w2[:].rearrange("(k p) h -> p k h", p=P)
        for half in range(2):
            k0, k1 = half * NKH // 2, (half + 1) * NKH // 2
            w_dmas.append(nc.scalar.dma_start(
                out=w1_all[:, k0:k1, :], in_=w1v[:, k0:k1, :]))
            w_dmas.append(nc.scalar.dma_start(
                out=w3_all[:, k0:k1, :], in_=w3v[:, k0:k1, :]))
        for half in range(2):
            k0, k1 = half * NKI // 2, (half + 1) * NKI // 2
            w_dmas.append(nc.scalar.dma_start(
                out=w2_all[:, k0:k1, :], in_=w2v[:, k0:k1, :]))

        for wd in w_dmas:
            _add_dep_helper(wd.ins, last_x.ins, True,
                            "weights stream after xt (routing DMA priority)")

        # ---- expert FFN: gate/up + SwiGLU -> hT, down -> yT ----
        with tc.tile_pool(name="psm", bufs=2, space="PSUM") as psm:
            hts = [ht_pool.tile([P, CAP], BF16, tag=f"ht{i}", name=f"ht{i}")
                   for i in range(NKI)]
            for it in range(NKI):
                i0 = it * P
                for c0, n in CHUNKS:
                    gps = psm.tile([P, 512], F32, tag="gate")
                    ups = psm.tile([P, 512], F32, tag="up")
                    for k in range(NKH):
                        nc.tensor.matmul(
                            out=gps[:, :n], lhsT=w1_all[:, k, i0:i0 + P],
                            rhs=xgt[k][:, c0:c0 + n],
                            start=(k == 0), stop=(k == NKH - 1))
                    for k in range(NKH):
                        nc.tensor.matmul(
                            out=ups[:, :n], lhsT=w3_all[:, k, i0:i0 + P],
                            rhs=xgt[k][:, c0:c0 + n],
                            start=(k == 0), stop=(k == NKH - 1))
                    sl = mm_pool.tile([P, 512], BF16, tag="silu")
                    nc.scalar.activation(out=sl[:, :n], in_=gps[:, :n],
                                         func=AF.Sigmoid)
                    tmp = mm_pool.tile([P, 512], BF16, tag="sgate")
                    nc.vector.tensor_tensor(
                        out=tmp[:, :n], in0=sl[:, :n], in1=gps[:, :n],
                        op=OP.mult)
                    nc.vector.tensor_tensor(
                        out=hts[it][:, c0:c0 + n], in0=tmp[:, :n],
                        in1=ups[:, :n], op=OP.mult)
            for ht_i in range(NKH):
                h0 = ht_i * P
                for c0, n in CHUNKS:
                    yps = psm.tile([P, 512], F32, tag="y")
                    for k in range(NKI):
                        nc.tensor.matmul(
                            out=yps[:, :n], lhsT=w2_all[:, k, h0:h0 + P],
                            rhs=hts[k][:, c0:c0 + n],
                            start=(k == 0), stop=(k == NKI - 1))
                    ysb = mm_pool.tile([P, 512], F32, tag="ysb")
                    nc.vector.tensor_tensor(
                        out=ysb[:, :n], in0=yps[:, :n],
                        in1=wbc_sb[:, c0:c0 + n], op=OP.mult)
                    nc.sync.dma_start(
                        out=yt[h0:h0 + P, c0:c0 + n], in_=ysb[:, :n])

    nc.compile()
    return nc


_NC_CACHE = None


def _get_program():
    global _NC_CACHE
    if _NC_CACHE is None:
        _NC_CACHE = build_program()
    return _NC_CACHE


def _prepare_in_maps(x, Wr, br, W1, W3, W2):
    x2d = np.ascontiguousarray(np.asarray(x, dtype=np.float32).reshape(S, H))
    # (k, p, ch, c) -> (ch, k, p, c): each routing chunk DMA is contiguous
    xt = np.ascontiguousarray(
        x2d.T.reshape(NKH, P, N_ROUTE_CHUNKS, ROUTE_CHUNK)
        .transpose(2, 0, 1, 3)
        .reshape(N_ROUTE_CHUNKS * NKH * P, ROUTE_CHUNK))
    xbf = x2d.astype(ml_dtypes.bfloat16)
    wr_np = np.ascontiguousarray(np.asarray(Wr, dtype=np.float32))
    br_np = np.asarray(br, dtype=np.float32).reshape(1, E)
    W1 = np.asarray(W1, dtype=np.float32)
    W3 = np.asarray(W3, dtype=np.float32)
    W2 = np.asarray(W2, dtype=np.float32)
    in_maps = []
    for e in range(N_CORES):
        oh_np = np.zeros((1, E), np.float32)
        oh_np[0, e] = 1.0
        in_maps.append({
            "xt": xt,
            "xbf": xbf,
            "wr": wr_np,
            "br": br_np,
            "oh": oh_np,
            "w1": W1[e].astype(ml_dtypes.bfloat16),
            "w3": W3[e].astype(ml_dtypes.bfloat16),
            "w2": W2[e].astype(ml_dtypes.bfloat16),
        })
    return in_maps


def _combine(results):
    out = np.zeros((S, H), np.float32)
    for e in range(N_CORES):
        idxw = np.asarray(results[e]["idxw"])
        yt = np.asarray(results[e]["yt"])
        idx = np.rint(idxw[0, :] * P + idxw[1, :]).astype(np.int64)
        np.add.at(out, idx, yt[:, :CAP].T)
    return out.reshape(B, S, H)


def run_on_device(inputs, trace=False, trace_cores=None):
    """Run the SPMD program; returns (full_output, BassKernelResults)."""
    nc = _get_program()
    in_maps = _prepare_in_maps(**inputs)
    kwargs = {}
    if trace:
        try:
            import types

            if "antenv.axon_hooks" not in sys.modules:
                from trn_agent_boot.trn_boot import _ntff_profile_via_ctypes

                hook = _ntff_profile_via_ctypes("/opt/axon/libaxon_pjrt.so")
                mod = types.ModuleType("antenv.axon_hooks")
                mod._hook = hook
                mod.get_axon_ntff_profile_hook = lambda: mod._hook

                def _set(h):
                    mod._hook = h

                mod.set_axon_ntff_profile_hook = _set
                sys.modules["antenv.axon_hooks"] = mod
                import antenv

                antenv.axon_hooks = mod
        except Exception as exc:  # profiling unavailable -> run untraced
            print(f"trace hook install failed: {exc}", file=sys.stderr)
        kwargs = dict(trace=True,
                      trace_cores=trace_cores or list(range(N_CORES)))
    res = run_bass_kernel_spmd(nc, in_maps, list(range(N_CORES)), **kwargs)
    return _combine(res.results), res


def kernel(x, Wr, br, W1, W3, W2):
    out, _ = run_on_device(dict(x=x, Wr=Wr, br=br, W1=W1, W3=W3, W2=W2))
    return out


# revision 19
# speedup vs baseline: 2.1193x; 1.0318x over previous
"""Expert-parallel MoE (top-2 of 8 experts, SwiGLU) on 8 Trainium2 NeuronCores.

Sharding: one expert per core (W1/W3/W2 sharded on the expert axis), router
replicated. Each core, fully on-device:
  1. Routing: logitsT = Wr.T @ x.T (fp32 PE matmul), PE-transpose to [tok, 8],
     top-2 + softmax -> per-token combine weight c for this core's expert.
  2. Compaction: cross-partition prefix sum (strictly-upper-triangular ones
     matmul) assigns every selected token a dense slot; an indirect-DMA
     scatter writes (token_id, c) pairs into a DRAM slot table.
  3. Dispatch: read the token ids back, indirect-DMA gather the selected rows
     of x (bf16), PE-transpose them to put H on partitions.
  4. Expert FFN: gate/up/down matmuls in bf16 with fp32 PSUM accumulation,
     SwiGLU, scale by c, emit yT [H, CAP] fp32 plus the slot table.
Host: out[idx_e] += yt_e.T accumulated over the 8 cores (the unshard step for
expert-parallel sharding). Unfilled slots carry c = 0 so they contribute 0.
"""
import sys

sys.path.insert(0, "/opt/trn_rl_repo")

from contextlib import ExitStack

import ml_dtypes
import numpy as np

import concourse.bacc as bacc
import concourse.bass as bass
import concourse.mybir as mybir
from concourse.bass_utils import run_bass_kernel_spmd
from concourse.masks import make_identity, make_upper_triangular
from concourse.tile import TileContext

F32 = mybir.dt.float32
BF16 = mybir.dt.bfloat16
I32 = mybir.dt.int32
AF = mybir.ActivationFunctionType
OP = mybir.AluOpType

P = 128
B, S, H, I_DIM, E, TOP_K = 1, 2048, 1024, 2048, 8, 2
NTT = S // P        # 16 token tiles
NKH = H // P        # 8 k-tiles over H
NKI = I_DIM // P    # 16 i-tiles
CAP = 640           # per-expert token capacity (multiple of 128)
NCT = CAP // P
TRASH = CAP         # trash row of the (CAP+1)-row slot table
BIG = 3.0e38
N_CORES = 8

CHUNKS = [(0, 512), (512, 128)]   # token chunks for the expert matmuls
ROUTE_CHUNK = 256
N_ROUTE_CHUNKS = S // ROUTE_CHUNK


def build_program():
    nc = bacc.Bacc("TRN2", target_bir_lowering=False, debug=False,
                   num_devices=N_CORES)

    xt = nc.dram_tensor("xt", [N_ROUTE_CHUNKS * NKH * P, ROUTE_CHUNK], F32,
                    kind="ExternalInput")
    xbf = nc.dram_tensor("xbf", [S, H], BF16, kind="ExternalInput")
    wr = nc.dram_tensor("wr", [H, E], F32, kind="ExternalInput")
    br = nc.dram_tensor("br", [1, E], F32, kind="ExternalInput")
    oh = nc.dram_tensor("oh", [1, E], F32, kind="ExternalInput")
    w1 = nc.dram_tensor("w1", [H, I_DIM], BF16, kind="ExternalInput")
    w3 = nc.dram_tensor("w3", [H, I_DIM], BF16, kind="ExternalInput")
    w2 = nc.dram_tensor("w2", [I_DIM, H], BF16, kind="ExternalInput")
    # slot table: row 0 = token ids, row 1 = combine weights, per slot
    idxw = nc.dram_tensor("idxw", [4, CAP], F32, kind="ExternalOutput")
    yt = nc.dram_tensor("yt", [H, CAP], F32, kind="ExternalOutput")

    with TileContext(nc) as tc, ExitStack() as ctx:
        const = ctx.enter_context(tc.tile_pool(name="const", bufs=1))
        route = ctx.enter_context(tc.tile_pool(name="route", bufs=1))
        xtch_pool = ctx.enter_context(tc.tile_pool(name="xtch", bufs=3))
        scr = ctx.enter_context(tc.tile_pool(name="scr", bufs=4))
        disp = ctx.enter_context(tc.tile_pool(name="disp", bufs=1))
        wpool = ctx.enter_context(tc.tile_pool(name="wpool", bufs=1))
        xgt_pool = ctx.enter_context(tc.tile_pool(name="xgt", bufs=1))
        xg_pool = ctx.enter_context(tc.tile_pool(name="xg", bufs=3))
        ht_pool = ctx.enter_context(tc.tile_pool(name="ht", bufs=1))
        mm_pool = ctx.enter_context(tc.tile_pool(name="mm", bufs=2))

        # ---- constants ----
        id_f32 = const.tile([P, P], F32, tag="idf")
        make_identity(nc, id_f32[:])
        id_bf = const.tile([P, P], BF16, tag="idb")
        make_identity(nc, id_bf[:])
        u128 = const.tile([P, P], F32, tag="u128")  # strictly-upper ones
        make_upper_triangular(nc, u128[:], val=1.0, diag=False)
        ones_col = const.tile([1, P], F32, tag="ones")
        nc.vector.memset(ones_col[:], 1.0)
        ones128 = const.tile([P, 1], F32, tag="ones128")
        nc.vector.memset(ones128[:], 1.0)
        zeros16 = const.tile([1, NTT], F32, tag="z16")
        nc.vector.memset(zeros16[:], 0.0)
        iota640 = const.tile([P, CAP], F32, tag="iota640")
        ii = mm_pool.tile([P, CAP], I32, tag="iotai", bufs=1)
        nc.gpsimd.iota(ii[:], pattern=[[1, CAP]], base=0, channel_multiplier=0)
        nc.vector.tensor_copy(out=iota640[:], in_=ii[:])
        # rowsel[p, j] = 1 for p >= 2 (sums the c_hi + c_lo payload rows)
        rowsel = const.tile([4, P], F32, tag="rowsel")
        nc.gpsimd.memset(rowsel[:], 0.0)
        nc.gpsimd.affine_select(
            out=rowsel[:], in_=rowsel[:], pattern=[[0, P]],
            compare_op=OP.is_ge, fill=1.0, base=1, channel_multiplier=-1)
        br_bc = const.tile([P, E], F32, tag="brbc")
        nc.sync.dma_start(out=br_bc[:], in_=br[:].to_broadcast((P, E)))
        oh_bc = const.tile([P, E], F32, tag="ohbc")
        nc.sync.dma_start(out=oh_bc[:], in_=oh[:].to_broadcast((P, E)))
        wr_sb = [const.tile([P, E], F32, tag=f"wr{k}", name=f"wr_sb{k}")
                 for k in range(NKH)]
        for k in range(NKH):
            nc.sync.dma_start(out=wr_sb[k][:], in_=wr[k * P:(k + 1) * P, :])

        x_dma_insts = []
        with tc.tile_pool(name="psr", bufs=2, space="PSUM") as psr:
            # PE warmup: keep TensorE busy from t=0 so HAM un-throttles before
            # the routing matmuls start (they wait ~15us for the first DMAs).
            warm_ps = psr.tile([P, P], BF16, tag="excl", bufs=1)
            for _ in range(160):
                nc.tensor.transpose(
                    out=warm_ps[:], in_=id_bf[:], identity=id_bf[:])

            # ---- routing: logitsT [E, S] = Wr.T @ xT (Wr stationary: the
            # 8-column weight load is nearly free; fp32 cost rides the moving
            # rows) ----
            logT = route.tile([E, S], F32, tag="logT")
            xt4 = xt[:].rearrange("(ch k p) c -> ch k p c", k=NKH, p=P)
            for ch in range(N_ROUTE_CHUNKS):
                c0 = ch * ROUTE_CHUNK
                lps = psr.tile([E, ROUTE_CHUNK], F32, tag="lps")
                xts = xtch_pool.tile([P, NKH, ROUTE_CHUNK], F32, tag="xtch",
                                     name=f"xtch_{ch}")
                xdma = nc.sync.dma_start(
                    out=xts[:],
                    in_=xt4[ch].rearrange("k p c -> p k c"))
                x_dma_insts.append(xdma)
                for k in range(NKH):
                    nc.tensor.matmul(
                        out=lps[:], lhsT=wr_sb[k][:],
                        rhs=xts[:, k, :],
                        start=(k == 0), stop=(k == NKH - 1))
                nc.vector.tensor_copy(out=logT[:, c0:c0 + ROUTE_CHUNK], in_=lps[:])

            # ---- top-2 + softmax, all 16 token tiles at once ----
            # transpose each [E, 128] logit block into one [P, NTT*E] PSUM
            # bank (disjoint columns; single-shot groups, safe in order)
            trps = psr.tile([P, NTT * E], F32, tag="trps", bufs=1)
            for t in range(NTT):
                nc.tensor.matmul(
                    out=trps[:, t * E:(t + 1) * E],
                    lhsT=logT[:E, t * P:(t + 1) * P],
                    rhs=id_f32[:E, :E],
                    is_transpose=True, start=True, stop=True,
                    skip_group_check=True)
            l_all = disp.tile([P, NTT * E], F32, tag="lall")
            nc.vector.tensor_tensor(
                out=l_all[:].rearrange("p (t e) -> p t e", e=E),
                in0=trps[:].rearrange("p (t e) -> p t e", e=E),
                in1=br_bc[:].rearrange("p e -> p () e").to_broadcast((P, NTT, E)),
                op=OP.add)

            cm_all = disp.tile([P, NTT], F32, tag="cm")
            sel_all = disp.tile([P, NTT], F32, tag="sel")
            tokf = disp.tile([P, NTT], F32, tag="tokf")
            toki = scr.tile([P, NTT], I32, tag="toki")
            nc.gpsimd.iota(toki[:], pattern=[[P, NTT]], base=0,
                           channel_multiplier=1)
            nc.vector.tensor_copy(out=tokf[:], in_=toki[:])

            def bcast(ap):  # [P, NTT] -> [P, NTT, E] stride-0 view
                return ap.to_broadcast((P, NTT, E))

            l3 = l_all[:].rearrange("p (t e) -> p t e", e=E)
            m1 = scr.tile([P, NTT], F32, tag="m1")
            nc.vector.tensor_reduce(
                out=m1[:], in_=l3, axis=mybir.AxisListType.X, op=OP.max)
            mask1 = scr.tile([P, NTT * E], F32, tag="mask1")
            mask1_3 = mask1[:].rearrange("p (t e) -> p t e", e=E)
            nc.vector.tensor_tensor(
                out=mask1_3, in0=l3, in1=bcast(m1[:]), op=OP.is_equal)
            l2 = scr.tile([P, NTT * E], F32, tag="l2")
            l2_3 = l2[:].rearrange("p (t e) -> p t e", e=E)
            nc.vector.tensor_scalar(
                out=l2[:], in0=mask1[:], scalar1=-BIG, scalar2=None,
                op0=OP.mult)
            nc.vector.tensor_add(l2[:], l2[:], l_all[:])
            m2 = scr.tile([P, NTT], F32, tag="m2")
            nc.vector.tensor_reduce(
                out=m2[:], in_=l2_3, axis=mybir.AxisListType.X, op=OP.max)
            mask2 = scr.tile([P, NTT * E], F32, tag="mask2")
            mask2_3 = mask2[:].rearrange("p (t e) -> p t e", e=E)
            nc.vector.tensor_tensor(
                out=mask2_3, in0=l2_3, in1=bcast(m2[:]), op=OP.is_equal)
            d = scr.tile([P, NTT], F32, tag="d")
            nc.vector.tensor_sub(d[:], m2[:], m1[:])
            ed = scr.tile([P, NTT], F32, tag="ed")
            nc.scalar.activation(out=ed[:], in_=d[:], func=AF.Exp)
            den = scr.tile([P, NTT], F32, tag="den")
            nc.vector.tensor_scalar_add(den[:], ed[:], 1.0)
            w1c = scr.tile([P, NTT], F32, tag="w1c")
            nc.vector.reciprocal(w1c[:], den[:])
            w2c = scr.tile([P, NTT], F32, tag="w2c")
            nc.vector.tensor_mul(w2c[:], ed[:], w1c[:])
            # c[p,t,e] = mask1*w1 + mask2*w2; then pick this core's expert
            call = scr.tile([P, NTT * E], F32, tag="call")
            call_3 = call[:].rearrange("p (t e) -> p t e", e=E)
            nc.vector.tensor_tensor(
                out=call_3, in0=mask1_3, in1=bcast(w1c[:]), op=OP.mult)
            c2t = scr.tile([P, NTT * E], F32, tag="c2t")
            c2_3 = c2t[:].rearrange("p (t e) -> p t e", e=E)
            nc.vector.tensor_tensor(
                out=c2_3, in0=mask2_3, in1=bcast(w2c[:]), op=OP.mult)
            nc.vector.tensor_add(call[:], call[:], c2t[:])
            cm8 = scr.tile([P, NTT * E], F32, tag="cm8")
            cm8_3 = cm8[:].rearrange("p (t e) -> p t e", e=E)
            nc.vector.tensor_tensor(
                out=cm8_3, in0=call_3,
                in1=oh_bc[:].rearrange("p e -> p () e").to_broadcast((P, NTT, E)),
                op=OP.mult)
            nc.vector.tensor_reduce(
                out=cm_all[:], in_=cm8_3, axis=mybir.AxisListType.X, op=OP.add)
            nc.vector.tensor_scalar(
                out=sel_all[:], in0=cm_all[:], scalar1=0.0, scalar2=None,
                op0=OP.is_gt)

            # ---- compaction: dense slot per selected token ----
            excl_ps = psr.tile([P, NTT], F32, tag="excl", bufs=1)
            nc.tensor.matmul(
                out=excl_ps[:], lhsT=u128[:], rhs=sel_all[:], start=True,
                stop=True)
            excl = disp.tile([P, NTT], F32, tag="exclsb")
            nc.vector.tensor_copy(out=excl[:], in_=excl_ps[:])
            tot_ps = psr.tile([1, NTT], F32, tag="totps", bufs=1)
            nc.tensor.matmul(
                out=tot_ps[:], lhsT=ones128[:], rhs=sel_all[:], start=True,
                stop=True)
            incl = disp.tile([1, NTT], F32, tag="incl")
            nc.vector.tensor_tensor_scan(
                out=incl[:], data0=tot_ps[:], data1=zeros16[:], initial=0.0,
                op0=OP.add, op1=OP.add)
            offs = disp.tile([1, NTT], F32, tag="offs")
            nc.vector.tensor_sub(offs[:], incl[:], tot_ps[:])
            offs_ps = psr.tile([P, NTT], F32, tag="offsps", bufs=1)
            nc.tensor.matmul(
                out=offs_ps[:], lhsT=ones_col[:], rhs=offs[:], start=True,
                stop=True)
            slot = disp.tile([P, NTT], F32, tag="slot")
            nc.vector.tensor_tensor(
                out=slot[:], in0=excl[:], in1=offs_ps[:], op=OP.add)
            # unselected tokens -> far past any real slot
            nc.vector.tensor_scalar_sub(slot[:], slot[:], float(TRASH))
            nc.vector.tensor_mul(slot[:], slot[:], sel_all[:])
            nc.vector.tensor_scalar_add(slot[:], slot[:], float(TRASH))

            # payload rows per token: tile idx, partition idx, c split into
            # bf16 hi+lo halves (hi+lo is fp32-exact to ~1.5e-5)
            tcol = scr.tile([P, 1], F32, tag="tcol")
            chi = scr.tile([P, NTT], F32, tag="chi")
            clo = scr.tile([P, NTT], F32, tag="clo")
            chib = scr.tile([P, NTT], BF16, tag="chib")
            nc.vector.tensor_copy(out=chib[:], in_=cm_all[:])
            nc.vector.tensor_copy(out=chi[:], in_=chib[:])
            nc.vector.tensor_sub(clo[:], cm_all[:], chi[:])
            tvals = scr.tile([P, NTT], F32, tag="tvals")
            ti = scr.tile([P, NTT], I32, tag="ti")
            nc.gpsimd.iota(ti[:], pattern=[[1, NTT]], base=0,
                           channel_multiplier=0)
            nc.vector.tensor_copy(out=tvals[:], in_=ti[:])
            pvals = scr.tile([P, 1], I32, tag="pvals")
            nc.gpsimd.iota(pvals[:], pattern=[[1, 1]], base=0,
                           channel_multiplier=1)
            pvf = scr.tile([P, 1], F32, tag="pvf")
            nc.vector.tensor_copy(out=pvf[:], in_=pvals[:])

            pairb = disp.tile([P, 4 * NTT], BF16, tag="pairb")
            pb4 = pairb[:].rearrange("p (t four) -> p t four", four=4)
            nc.vector.tensor_copy(
                out=pb4[:, :, 0:1], in_=tvals[:].rearrange("p t -> p t ()"))
            nc.vector.tensor_copy(
                out=pb4[:, :, 1:2],
                in_=pvf[:].rearrange("p o -> p () o").to_broadcast((P, NTT, 1)))
            nc.vector.tensor_copy(
                out=pb4[:, :, 2:3], in_=chi[:].rearrange("p t -> p t ()"))
            nc.vector.tensor_copy(
                out=pb4[:, :, 3:4], in_=clo[:].rearrange("p t -> p t ()"))

            # ---- on-chip inverse permutation via one-hot matmuls ----
            # cmp_t[p, s] = (slot[p, t] == s); pe[4, s] += pairb[:,t].T @ cmp_t
            # Exactly one token matches each filled slot, so the sums are
            # single-term; ids are small ints, exact in bf16.
            pe_parts = []
            for c0, n in CHUNKS:
                pe_parts.append(psr.tile(
                    [4, 512], F32, tag=f"pe{c0}", bufs=1, name=f"pe_ps{c0}"))
            for t in range(NTT):
                cmp = scr.tile([P, CAP], BF16, tag="cmp", bufs=2)
                nc.vector.tensor_tensor(
                    out=cmp[:], in0=slot[:, t:t + 1].to_broadcast((P, CAP)),
                    in1=iota640[:], op=OP.is_equal)
                for ci, (c0, n) in enumerate(CHUNKS):
                    nc.tensor.matmul(
                        out=pe_parts[ci][:, :n], lhsT=pairb[:, 4 * t:4 * t + 4],
                        rhs=cmp[:, c0:c0 + n],
                        start=(t == 0), stop=(t == NTT - 1))
            pe_sb = disp.tile([4, CAP], F32, tag="pesb")
            for ci, (c0, n) in enumerate(CHUNKS):
                nc.vector.tensor_copy(
                    out=pe_sb[:, c0:c0 + n], in_=pe_parts[ci][:, :n])
            # ship the slot table to the host (host: idx = 128*row0 + row1)
            nc.sync.dma_start(out=idxw[:], in_=pe_sb[:])

        # ---- dispatch: gather selected x rows, transpose to [H, CAP] ----
        with tc.tile_pool(name="psd", bufs=2, space="PSUM") as psd:
            # broadcast c over partitions: wbc[p, s] = c_hi[s] + c_lo[s]
            wbc_sb = disp.tile([P, CAP], F32, tag="wbc")
            for c0, n in CHUNKS:
                wps = psd.tile([P, 512], F32, tag="wbcps", bufs=1)
                nc.tensor.matmul(
                    out=wps[:, :n], lhsT=rowsel[:], rhs=pe_sb[:, c0:c0 + n],
                    start=True, stop=True)
                nc.vector.tensor_copy(out=wbc_sb[:, c0:c0 + n], in_=wps[:, :n])

            xgt = [xgt_pool.tile([P, CAP], BF16, tag=f"xgt{k}", name=f"xgt{k}")
                   for k in range(NKH)]
            for ct in range(NCT):
                # idx per capacity tile: transpose pe_sb[:, ct*P:+P] -> [P, 4]
                trp = psd.tile([P, 4], F32, tag="idxtr")
                nc.tensor.matmul(
                    out=trp[:], lhsT=pe_sb[:4, ct * P:(ct + 1) * P],
                    rhs=id_f32[:4, :4],
                    is_transpose=True, start=True, stop=True)
                idx_f = scr.tile([P, 1], F32, tag="idxf")
                nc.vector.tensor_scalar(
                    out=idx_f[:], in0=trp[:, 0:1], scalar1=float(P),
                    scalar2=trp[:, 1:2], op0=OP.mult, op1=OP.add)
                idx_i = scr.tile([P, 1], I32, tag="idxi")
                nc.vector.tensor_copy(out=idx_i[:], in_=idx_f[:])
                xg = xg_pool.tile([P, H], BF16, tag="xg")
                nc.gpsimd.indirect_dma_start(
                    out=xg[:],
                    out_offset=None,
                    in_=xbf[:],
                    in_offset=bass.IndirectOffsetOnAxis(ap=idx_i[:, 0:1], axis=0))
                for k in range(NKH):
                    tps = psd.tile([P, P], BF16, tag="xtr")
                    nc.tensor.transpose(
                        out=tps[:], in_=xg[:, k * P:(k + 1) * P],
                        identity=id_bf[:])
                    nc.vector.tensor_copy(
                        out=xgt[k][:, ct * P:(ct + 1) * P], in_=tps[:])

        # ---- expert weights (resident in SBUF) ----
        from concourse.bass import _add_dep_helper
        last_x = x_dma_insts[-1]
        w1_all = wpool.tile([P, NKH, I_DIM], BF16, tag="w1a")
        w3_all = wpool.tile([P, NKH, I_DIM], BF16, tag="w3a")
        w2_all = wpool.tile([P, NKI, H], BF16, tag="w2a")
        w_dmas = []
        w1v = w1[:].rearrange("(k p) i -> p k i", p=P)
        w3v = w3[:].rearrange("(k p) i -> p k i", p=P)
        w2v = w2[:].rearrange("(k p) h -> p k h", p=P)
        for half in range(2):
            k0, k1 = half * NKH // 2, (half + 1) * NKH // 2
            w_dmas.append(nc.scalar.dma_start(
                out=w1_all[:, k0:k1, :], in_=w1v[:, k0:k1, :]))
            w_dmas.append(nc.scalar.dma_start(
                out=w3_all[:, k0:k1, :], in_=w3v[:, k0:k1, :]))
        for half in range(2):
            k0, k1 = half * NKI // 2, (half + 1) * NKI // 2
            w_dmas.append(nc.scalar.dma_start(
                out=w2_all[:, k0:k1, :], in_=w2v[:, k0:k1, :]))

        for wd in w_dmas:
            _add_dep_helper(wd.ins, last_x.ins, True,
                            "weights stream after xt (routing DMA priority)")

        # ---- expert FFN: gate/up + SwiGLU -> hT, down -> yT ----
        with tc.tile_pool(name="psm", bufs=2, space="PSUM") as psm:
            hts = [ht_pool.tile([P, CAP], BF16, tag=f"ht{i}", name=f"ht{i}")
                   for i in range(NKI)]
            for it in range(NKI):
                i0 = it * P
                for c0, n in CHUNKS:
                    gps = psm.tile([P, 512], F32, tag="gate")
                    ups = psm.tile([P, 512], F32, tag="up")
                    for k in range(NKH):
                        nc.tensor.matmul(
                            out=gps[:, :n], lhsT=w1_all[:, k, i0:i0 + P],
                            rhs=xgt[k][:, c0:c0 + n],
                            start=(k == 0), stop=(k == NKH - 1))
                    for k in range(NKH):
                        nc.tensor.matmul(
                            out=ups[:, :n], lhsT=w3_all[:, k, i0:i0 + P],
                            rhs=xgt[k][:, c0:c0 + n],
                            start=(k == 0), stop=(k == NKH - 1))
                    sl = mm_pool.tile([P, 512], BF16, tag="silu")
                    nc.scalar.activation(out=sl[:, :n], in_=gps[:, :n],
                                         func=AF.Sigmoid)
                    tmp = mm_pool.tile([P, 512], BF16, tag="sgate")
                    nc.vector.tensor_tensor(
                        out=tmp[:, :n], in0=sl[:, :n], in1=gps[:, :n],
                        op=OP.mult)
                    nc.vector.tensor_tensor(
                        out=hts[it][:, c0:c0 + n], in0=tmp[:, :n],
                        in1=ups[:, :n], op=OP.mult)
            for ht_i in range(NKH):
                h0 = ht_i * P
                for c0, n in CHUNKS:
                    yps = psm.tile([P, 512], F32, tag="y")
                    for k in range(NKI):
                        nc.tensor.matmul(
                            out=yps[:, :n], lhsT=w2_all[:, k, h0:h0 + P],
                            rhs=hts[k][:, c0:c0 + n],
                            start=(k == 0), stop=(k == NKI - 1))
                    ysb = mm_pool.tile([P, 512], F32, tag="ysb")
                    nc.vector.tensor_tensor(
                        out=ysb[:, :n], in0=yps[:, :n],
                        in1=wbc_sb[:, c0:c0 + n], op=OP.mult)
                    nc.sync.dma_start(
                        out=yt[h0:h0 + P, c0:c0 + n], in_=ysb[:, :n])

    nc.compile()
    return nc


_NC_CACHE = None


def _get_program():
    global _NC_CACHE
    if _NC_CACHE is None:
        _NC_CACHE = build_program()
    return _NC_CACHE


def _prepare_in_maps(x, Wr, br, W1, W3, W2):
    x2d = np.ascontiguousarray(np.asarray(x, dtype=np.float32).reshape(S, H))
    # (k, p, ch, c) -> (ch, k, p, c): each routing chunk DMA is contiguous
    xt = np.ascontiguousarray(
        x2d.T.reshape(NKH, P, N_ROUTE_CHUNKS, ROUTE_CHUNK)
        .transpose(2, 0, 1, 3)
        .reshape(N_ROUTE_CHUNKS * NKH * P, ROUTE_CHUNK))
    xbf = x2d.astype(ml_dtypes.bfloat16)
    wr_np = np.ascontiguousarray(np.asarray(Wr, dtype=np.float32))
    br_np = np.asarray(br, dtype=np.float32).reshape(1, E)
    W1 = np.asarray(W1, dtype=np.float32)
    W3 = np.asarray(W3, dtype=np.float32)
    W2 = np.asarray(W2, dtype=np.float32)
    in_maps = []
    for e in range(N_CORES):
        oh_np = np.zeros((1, E), np.float32)
        oh_np[0, e] = 1.0
        in_maps.append({
            "xt": xt,
            "xbf": xbf,
            "wr": wr_np,
            "br": br_np,
            "oh": oh_np,
            "w1": W1[e].astype(ml_dtypes.bfloat16),
            "w3": W3[e].astype(ml_dtypes.bfloat16),
            "w2": W2[e].astype(ml_dtypes.bfloat16),
        })
    return in_maps


def _combine(results):
    out = np.zeros((S, H), np.float32)
    for e in range(N_CORES):
        idxw = np.asarray(results[e]["idxw"])
        yt = np.asarray(results[e]["yt"])
        idx = np.rint(idxw[0, :] * P + idxw[1, :]).astype(np.int64)
        np.add.at(out, idx, yt[:, :CAP].T)
    return out.reshape(B, S, H)


def run_on_device(inputs, trace=False, trace_cores=None):
    """Run the SPMD program; returns (full_output, BassKernelResults)."""
    nc = _get_program()
    in_maps = _prepare_in_maps(**inputs)
    kwargs = {}
    if trace:
        try:
            import types

            if "antenv.axon_hooks" not in sys.modules:
                from trn_agent_boot.trn_boot import _ntff_profile_via_ctypes

                hook = _ntff_profile_via_ctypes("/opt/axon/libaxon_pjrt.so")
                mod = types.ModuleType("antenv.axon_hooks")
                mod._hook = hook
                mod.get_axon_ntff_profile_hook = lambda: mod._hook

                def _set(h):
                    mod._hook = h

                mod.set_axon_ntff_profile_hook = _set
                sys.modules["antenv.axon_hooks"] = mod
                import antenv

                antenv.axon_hooks = mod
        except Exception as exc:  # profiling unavailable -> run untraced
            print(f"trace hook install failed: {exc}", file=sys.stderr)
        kwargs = dict(trace=True,
                      trace_cores=trace_cores or list(range(N_CORES)))
    res = run_bass_kernel_spmd(nc, in_maps, list(range(N_CORES)), **kwargs)
    return _combine(res.results), res


def kernel(x, Wr, br, W1, W3, W2):
    out, _ = run_on_device(dict(x=x, Wr=Wr, br=br, W1=W1, W3=W3, W2=W2))
    return out


# revision 20
# speedup vs baseline: 2.1230x; 1.0018x over previous
"""Expert-parallel MoE (top-2 of 8 experts, SwiGLU) on 8 Trainium2 NeuronCores.

Sharding: one expert per core (W1/W3/W2 sharded on the expert axis), router
replicated. Each core, fully on-device:
  1. Routing: logitsT = Wr.T @ x.T (fp32 PE matmul), PE-transpose to [tok, 8],
     top-2 + softmax -> per-token combine weight c for this core's expert.
  2. Compaction: cross-partition prefix sum (strictly-upper-triangular ones
     matmul) assigns every selected token a dense slot; an indirect-DMA
     scatter writes (token_id, c) pairs into a DRAM slot table.
  3. Dispatch: read the token ids back, indirect-DMA gather the selected rows
     of x (bf16), PE-transpose them to put H on partitions.
  4. Expert FFN: gate/up/down matmuls in bf16 with fp32 PSUM accumulation,
     SwiGLU, scale by c, emit yT [H, CAP] fp32 plus the slot table.
Host: out[idx_e] += yt_e.T accumulated over the 8 cores (the unshard step for
expert-parallel sharding). Unfilled slots carry c = 0 so they contribute 0.
"""
import sys

sys.path.insert(0, "/opt/trn_rl_repo")

from contextlib import ExitStack

import ml_dtypes
import numpy as np

import concourse.bacc as bacc
import concourse.bass as bass
import concourse.mybir as mybir
from concourse.bass_utils import run_bass_kernel_spmd
from concourse.masks import make_identity, make_upper_triangular
from concourse.tile import TileContext

F32 = mybir.dt.float32
BF16 = mybir.dt.bfloat16
I32 = mybir.dt.int32
AF = mybir.ActivationFunctionType
OP = mybir.AluOpType

P = 128
B, S, H, I_DIM, E, TOP_K = 1, 2048, 1024, 2048, 8, 2
NTT = S // P        # 16 token tiles
NKH = H // P        # 8 k-tiles over H
NKI = I_DIM // P    # 16 i-tiles
CAP = 640           # per-expert token capacity (multiple of 128)
NCT = CAP // P
TRASH = CAP         # trash row of the (CAP+1)-row slot table
BIG = 3.0e38
N_CORES = 8

CHUNKS = [(0, 512), (512, 128)]   # token chunks for the expert matmuls
ROUTE_CHUNK = 256
N_ROUTE_CHUNKS = S // ROUTE_CHUNK


def build_program():
    nc = bacc.Bacc("TRN2", target_bir_lowering=False, debug=False,
                   num_devices=N_CORES)

    xth = nc.dram_tensor("xth", [N_ROUTE_CHUNKS * NKH * P, ROUTE_CHUNK], BF16,
                         kind="ExternalInput")
    xtl = nc.dram_tensor("xtl", [N_ROUTE_CHUNKS * NKH * P, ROUTE_CHUNK], BF16,
                         kind="ExternalInput")
    xbf = nc.dram_tensor("xbf", [S, H], BF16, kind="ExternalInput")
    wrh = nc.dram_tensor("wrh", [H, E], BF16, kind="ExternalInput")
    wrl = nc.dram_tensor("wrl", [H, E], BF16, kind="ExternalInput")
    br = nc.dram_tensor("br", [1, E], F32, kind="ExternalInput")
    oh = nc.dram_tensor("oh", [1, E], F32, kind="ExternalInput")
    w1 = nc.dram_tensor("w1", [H, I_DIM], BF16, kind="ExternalInput")
    w3 = nc.dram_tensor("w3", [H, I_DIM], BF16, kind="ExternalInput")
    w2 = nc.dram_tensor("w2", [I_DIM, H], BF16, kind="ExternalInput")
    # slot table: row 0 = token ids, row 1 = combine weights, per slot
    idxw = nc.dram_tensor("idxw", [4, CAP], F32, kind="ExternalOutput")
    yt = nc.dram_tensor("yt", [H, CAP], F32, kind="ExternalOutput")

    with TileContext(nc) as tc, ExitStack() as ctx:
        const = ctx.enter_context(tc.tile_pool(name="const", bufs=1))
        route = ctx.enter_context(tc.tile_pool(name="route", bufs=1))
        xtch_pool = ctx.enter_context(tc.tile_pool(name="xtch", bufs=3))
        scr = ctx.enter_context(tc.tile_pool(name="scr", bufs=4))
        disp = ctx.enter_context(tc.tile_pool(name="disp", bufs=1))
        wpool = ctx.enter_context(tc.tile_pool(name="wpool", bufs=1))
        xgt_pool = ctx.enter_context(tc.tile_pool(name="xgt", bufs=1))
        xg_pool = ctx.enter_context(tc.tile_pool(name="xg", bufs=3))
        ht_pool = ctx.enter_context(tc.tile_pool(name="ht", bufs=1))
        mm_pool = ctx.enter_context(tc.tile_pool(name="mm", bufs=2))

        # ---- constants ----
        id_f32 = const.tile([P, P], F32, tag="idf")
        make_identity(nc, id_f32[:])
        id_bf = const.tile([P, P], BF16, tag="idb")
        make_identity(nc, id_bf[:])
        u128 = const.tile([P, P], F32, tag="u128")  # strictly-upper ones
        make_upper_triangular(nc, u128[:], val=1.0, diag=False)
        ones_col = const.tile([1, P], F32, tag="ones")
        nc.vector.memset(ones_col[:], 1.0)
        ones128 = const.tile([P, 1], F32, tag="ones128")
        nc.vector.memset(ones128[:], 1.0)
        zeros16 = const.tile([1, NTT], F32, tag="z16")
        nc.vector.memset(zeros16[:], 0.0)
        iota640 = const.tile([P, CAP], F32, tag="iota640")
        ii = mm_pool.tile([P, CAP], I32, tag="iotai", bufs=1)
        nc.gpsimd.iota(ii[:], pattern=[[1, CAP]], base=0, channel_multiplier=0)
        nc.vector.tensor_copy(out=iota640[:], in_=ii[:])
        # rowsel[p, j] = 1 for p >= 2 (sums the c_hi + c_lo payload rows)
        rowsel = const.tile([4, P], F32, tag="rowsel")
        nc.gpsimd.memset(rowsel[:], 0.0)
        nc.gpsimd.affine_select(
            out=rowsel[:], in_=rowsel[:], pattern=[[0, P]],
            compare_op=OP.is_ge, fill=1.0, base=1, channel_multiplier=-1)
        br_bc = const.tile([P, E], F32, tag="brbc")
        nc.sync.dma_start(out=br_bc[:], in_=br[:].to_broadcast((P, E)))
        oh_bc = const.tile([P, E], F32, tag="ohbc")
        nc.sync.dma_start(out=oh_bc[:], in_=oh[:].to_broadcast((P, E)))
        wrh_sb = [const.tile([P, E], BF16, tag=f"wrh{k}", name=f"wrh_sb{k}")
                  for k in range(NKH)]
        wrl_sb = [const.tile([P, E], BF16, tag=f"wrl{k}", name=f"wrl_sb{k}")
                  for k in range(NKH)]
        for k in range(NKH):
            nc.sync.dma_start(out=wrh_sb[k][:], in_=wrh[k * P:(k + 1) * P, :])
            nc.sync.dma_start(out=wrl_sb[k][:], in_=wrl[k * P:(k + 1) * P, :])

        x_dma_insts = []
        with tc.tile_pool(name="psr", bufs=2, space="PSUM") as psr:
            # PE warmup: keep TensorE busy from t=0 so HAM un-throttles before
            # the routing matmuls start (they wait ~15us for the first DMAs).
            warm_ps = psr.tile([P, P], BF16, tag="excl", bufs=1)
            for _ in range(90):
                nc.tensor.transpose(
                    out=warm_ps[:], in_=id_bf[:], identity=id_bf[:])

            # ---- routing: logitsT [E, S] = Wr.T @ xT (Wr stationary: the
            # 8-column weight load is nearly free; fp32 cost rides the moving
            # rows) ----
            logT = route.tile([E, S], F32, tag="logT")
            xth4 = xth[:].rearrange("(ch k p) c -> ch k p c", k=NKH, p=P)
            xtl4 = xtl[:].rearrange("(ch k p) c -> ch k p c", k=NKH, p=P)
            for ch in range(N_ROUTE_CHUNKS):
                c0 = ch * ROUTE_CHUNK
                lps = psr.tile([E, ROUTE_CHUNK], F32, tag="lps")
                xtsh = xtch_pool.tile([P, NKH, ROUTE_CHUNK], BF16, tag="xtch",
                                      name=f"xtch_{ch}")
                xtsl = xtch_pool.tile([P, NKH, ROUTE_CHUNK], BF16, tag="xtcl",
                                      name=f"xtcl_{ch}")
                xdma = nc.sync.dma_start(
                    out=xtsh[:], in_=xth4[ch].rearrange("k p c -> p k c"))
                x_dma_insts.append(xdma)
                xdma = nc.sync.dma_start(
                    out=xtsl[:], in_=xtl4[ch].rearrange("k p c -> p k c"))
                x_dma_insts.append(xdma)
                # exact-enough fp32: hi*hi + hi*lo + lo*hi (lo*lo ~ 2^-16)
                n_mm = 3 * NKH
                mi = 0
                for k in range(NKH):
                    for lhs, rhs in ((wrh_sb[k][:], xtsh[:, k, :]),
                                     (wrl_sb[k][:], xtsh[:, k, :]),
                                     (wrh_sb[k][:], xtsl[:, k, :])):
                        nc.tensor.matmul(
                            out=lps[:], lhsT=lhs, rhs=rhs,
                            start=(mi == 0), stop=(mi == n_mm - 1))
                        mi += 1
                nc.vector.tensor_copy(out=logT[:, c0:c0 + ROUTE_CHUNK], in_=lps[:])

            # ---- top-2 + softmax, all 16 token tiles at once ----
            # transpose each [E, 128] logit block into one [P, NTT*E] PSUM
            # bank (disjoint columns; single-shot groups, safe in order)
            trps = psr.tile([P, NTT * E], F32, tag="trps", bufs=1)
            for t in range(NTT):
                nc.tensor.matmul(
                    out=trps[:, t * E:(t + 1) * E],
                    lhsT=logT[:E, t * P:(t + 1) * P],
                    rhs=id_f32[:E, :E],
                    is_transpose=True, start=True, stop=True,
                    skip_group_check=True)
            l_all = disp.tile([P, NTT * E], F32, tag="lall")
            nc.vector.tensor_tensor(
                out=l_all[:].rearrange("p (t e) -> p t e", e=E),
                in0=trps[:].rearrange("p (t e) -> p t e", e=E),
                in1=br_bc[:].rearrange("p e -> p () e").to_broadcast((P, NTT, E)),
                op=OP.add)

            cm_all = disp.tile([P, NTT], F32, tag="cm")
            sel_all = disp.tile([P, NTT], F32, tag="sel")
            tokf = disp.tile([P, NTT], F32, tag="tokf")
            toki = scr.tile([P, NTT], I32, tag="toki")
            nc.gpsimd.iota(toki[:], pattern=[[P, NTT]], base=0,
                           channel_multiplier=1)
            nc.vector.tensor_copy(out=tokf[:], in_=toki[:])

            def bcast(ap):  # [P, NTT] -> [P, NTT, E] stride-0 view
                return ap.to_broadcast((P, NTT, E))

            l3 = l_all[:].rearrange("p (t e) -> p t e", e=E)
            m1 = scr.tile([P, NTT], F32, tag="m1")
            nc.vector.tensor_reduce(
                out=m1[:], in_=l3, axis=mybir.AxisListType.X, op=OP.max)
            mask1 = scr.tile([P, NTT * E], F32, tag="mask1")
            mask1_3 = mask1[:].rearrange("p (t e) -> p t e", e=E)
            nc.vector.tensor_tensor(
                out=mask1_3, in0=l3, in1=bcast(m1[:]), op=OP.is_equal)
            l2 = scr.tile([P, NTT * E], F32, tag="l2")
            l2_3 = l2[:].rearrange("p (t e) -> p t e", e=E)
            nc.vector.tensor_scalar(
                out=l2[:], in0=mask1[:], scalar1=-BIG, scalar2=None,
                op0=OP.mult)
            nc.vector.tensor_add(l2[:], l2[:], l_all[:])
            m2 = scr.tile([P, NTT], F32, tag="m2")
            nc.vector.tensor_reduce(
                out=m2[:], in_=l2_3, axis=mybir.AxisListType.X, op=OP.max)
            mask2 = scr.tile([P, NTT * E], F32, tag="mask2")
            mask2_3 = mask2[:].rearrange("p (t e) -> p t e", e=E)
            nc.vector.tensor_tensor(
                out=mask2_3, in0=l2_3, in1=bcast(m2[:]), op=OP.is_equal)
            d = scr.tile([P, NTT], F32, tag="d")
            nc.vector.tensor_sub(d[:], m2[:], m1[:])
            ed = scr.tile([P, NTT], F32, tag="ed")
            nc.scalar.activation(out=ed[:], in_=d[:], func=AF.Exp)
            den = scr.tile([P, NTT], F32, tag="den")
            nc.vector.tensor_scalar_add(den[:], ed[:], 1.0)
            w1c = scr.tile([P, NTT], F32, tag="w1c")
            nc.vector.reciprocal(w1c[:], den[:])
            w2c = scr.tile([P, NTT], F32, tag="w2c")
            nc.vector.tensor_mul(w2c[:], ed[:], w1c[:])
            # c[p,t,e] = mask1*w1 + mask2*w2; then pick this core's expert
            call = scr.tile([P, NTT * E], F32, tag="call")
            call_3 = call[:].rearrange("p (t e) -> p t e", e=E)
            nc.vector.tensor_tensor(
                out=call_3, in0=mask1_3, in1=bcast(w1c[:]), op=OP.mult)
            c2t = scr.tile([P, NTT * E], F32, tag="c2t")
            c2_3 = c2t[:].rearrange("p (t e) -> p t e", e=E)
            nc.vector.tensor_tensor(
                out=c2_3, in0=mask2_3, in1=bcast(w2c[:]), op=OP.mult)
            nc.vector.tensor_add(call[:], call[:], c2t[:])
            cm8 = scr.tile([P, NTT * E], F32, tag="cm8")
            cm8_3 = cm8[:].rearrange("p (t e) -> p t e", e=E)
            nc.vector.tensor_tensor(
                out=cm8_3, in0=call_3,
                in1=oh_bc[:].rearrange("p e -> p () e").to_broadcast((P, NTT, E)),
                op=OP.mult)
            nc.vector.tensor_reduce(
                out=cm_all[:], in_=cm8_3, axis=mybir.AxisListType.X, op=OP.add)
            nc.vector.tensor_scalar(
                out=sel_all[:], in0=cm_all[:], scalar1=0.0, scalar2=None,
                op0=OP.is_gt)

            # ---- compaction: dense slot per selected token ----
            excl_ps = psr.tile([P, NTT], F32, tag="excl", bufs=1)
            nc.tensor.matmul(
                out=excl_ps[:], lhsT=u128[:], rhs=sel_all[:], start=True,
                stop=True)
            excl = disp.tile([P, NTT], F32, tag="exclsb")
            nc.vector.tensor_copy(out=excl[:], in_=excl_ps[:])
            tot_ps = psr.tile([1, NTT], F32, tag="totps", bufs=1)
            nc.tensor.matmul(
                out=tot_ps[:], lhsT=ones128[:], rhs=sel_all[:], start=True,
                stop=True)
            incl = disp.tile([1, NTT], F32, tag="incl")
            nc.vector.tensor_tensor_scan(
                out=incl[:], data0=tot_ps[:], data1=zeros16[:], initial=0.0,
                op0=OP.add, op1=OP.add)
            offs = disp.tile([1, NTT], F32, tag="offs")
            nc.vector.tensor_sub(offs[:], incl[:], tot_ps[:])
            offs_ps = psr.tile([P, NTT], F32, tag="offsps", bufs=1)
            nc.tensor.matmul(
                out=offs_ps[:], lhsT=ones_col[:], rhs=offs[:], start=True,
                stop=True)
            slot = disp.tile([P, NTT], F32, tag="slot")
            nc.vector.tensor_tensor(
                out=slot[:], in0=excl[:], in1=offs_ps[:], op=OP.add)
            # unselected tokens -> far past any real slot
            nc.vector.tensor_scalar_sub(slot[:], slot[:], float(TRASH))
            nc.vector.tensor_mul(slot[:], slot[:], sel_all[:])
            nc.vector.tensor_scalar_add(slot[:], slot[:], float(TRASH))

            # payload rows per token: tile idx, partition idx, c split into
            # bf16 hi+lo halves (hi+lo is fp32-exact to ~1.5e-5)
            tcol = scr.tile([P, 1], F32, tag="tcol")
            chi = scr.tile([P, NTT], F32, tag="chi")
            clo = scr.tile([P, NTT], F32, tag="clo")
            chib = scr.tile([P, NTT], BF16, tag="chib")
            nc.vector.tensor_copy(out=chib[:], in_=cm_all[:])
            nc.vector.tensor_copy(out=chi[:], in_=chib[:])
            nc.vector.tensor_sub(clo[:], cm_all[:], chi[:])
            tvals = scr.tile([P, NTT], F32, tag="tvals")
            ti = scr.tile([P, NTT], I32, tag="ti")
            nc.gpsimd.iota(ti[:], pattern=[[1, NTT]], base=0,
                           channel_multiplier=0)
            nc.vector.tensor_copy(out=tvals[:], in_=ti[:])
            pvals = scr.tile([P, 1], I32, tag="pvals")
            nc.gpsimd.iota(pvals[:], pattern=[[1, 1]], base=0,
                           channel_multiplier=1)
            pvf = scr.tile([P, 1], F32, tag="pvf")
            nc.vector.tensor_copy(out=pvf[:], in_=pvals[:])

            pairb = disp.tile([P, 4 * NTT], BF16, tag="pairb")
            pb4 = pairb[:].rearrange("p (t four) -> p t four", four=4)
            nc.vector.tensor_copy(
                out=pb4[:, :, 0:1], in_=tvals[:].rearrange("p t -> p t ()"))
            nc.vector.tensor_copy(
                out=pb4[:, :, 1:2],
                in_=pvf[:].rearrange("p o -> p () o").to_broadcast((P, NTT, 1)))
            nc.vector.tensor_copy(
                out=pb4[:, :, 2:3], in_=chi[:].rearrange("p t -> p t ()"))
            nc.vector.tensor_copy(
                out=pb4[:, :, 3:4], in_=clo[:].rearrange("p t -> p t ()"))

            # ---- on-chip inverse permutation via one-hot matmuls ----
            # cmp_t[p, s] = (slot[p, t] == s); pe[4, s] += pairb[:,t].T @ cmp_t
            # Exactly one token matches each filled slot, so the sums are
            # single-term; ids are small ints, exact in bf16.
            pe_parts = []
            for c0, n in CHUNKS:
                pe_parts.append(psr.tile(
                    [4, 512], F32, tag=f"pe{c0}", bufs=1, name=f"pe_ps{c0}"))
            for t in range(NTT):
                cmp = scr.tile([P, CAP], BF16, tag="cmp", bufs=2)
                nc.vector.tensor_tensor(
                    out=cmp[:], in0=slot[:, t:t + 1].to_broadcast((P, CAP)),
                    in1=iota640[:], op=OP.is_equal)
                for ci, (c0, n) in enumerate(CHUNKS):
                    nc.tensor.matmul(
                        out=pe_parts[ci][:, :n], lhsT=pairb[:, 4 * t:4 * t + 4],
                        rhs=cmp[:, c0:c0 + n],
                        start=(t == 0), stop=(t == NTT - 1))
            pe_sb = disp.tile([4, CAP], F32, tag="pesb")
            for ci, (c0, n) in enumerate(CHUNKS):
                nc.vector.tensor_copy(
                    out=pe_sb[:, c0:c0 + n], in_=pe_parts[ci][:, :n])
            # ship the slot table to the host (host: idx = 128*row0 + row1)
            nc.sync.dma_start(out=idxw[:], in_=pe_sb[:])

        # ---- dispatch: gather selected x rows, transpose to [H, CAP] ----
        with tc.tile_pool(name="psd", bufs=2, space="PSUM") as psd:
            # broadcast c over partitions: wbc[p, s] = c_hi[s] + c_lo[s]
            wbc_sb = disp.tile([P, CAP], F32, tag="wbc")
            for c0, n in CHUNKS:
                wps = psd.tile([P, 512], F32, tag="wbcps", bufs=1)
                nc.tensor.matmul(
                    out=wps[:, :n], lhsT=rowsel[:], rhs=pe_sb[:, c0:c0 + n],
                    start=True, stop=True)
                nc.vector.tensor_copy(out=wbc_sb[:, c0:c0 + n], in_=wps[:, :n])

            xgt = [xgt_pool.tile([P, CAP], BF16, tag=f"xgt{k}", name=f"xgt{k}")
                   for k in range(NKH)]
            for ct in range(NCT):
                # idx per capacity tile: transpose pe_sb[:, ct*P:+P] -> [P, 4]
                trp = psd.tile([P, 4], F32, tag="idxtr")
                nc.tensor.matmul(
                    out=trp[:], lhsT=pe_sb[:4, ct * P:(ct + 1) * P],
                    rhs=id_f32[:4, :4],
                    is_transpose=True, start=True, stop=True)
                idx_f = scr.tile([P, 1], F32, tag="idxf")
                nc.vector.tensor_scalar(
                    out=idx_f[:], in0=trp[:, 0:1], scalar1=float(P),
                    scalar2=trp[:, 1:2], op0=OP.mult, op1=OP.add)
                idx_i = scr.tile([P, 1], I32, tag="idxi")
                nc.vector.tensor_copy(out=idx_i[:], in_=idx_f[:])
                xg = xg_pool.tile([P, H], BF16, tag="xg")
                nc.gpsimd.indirect_dma_start(
                    out=xg[:],
                    out_offset=None,
                    in_=xbf[:],
                    in_offset=bass.IndirectOffsetOnAxis(ap=idx_i[:, 0:1], axis=0))
                for k in range(NKH):
                    tps = psd.tile([P, P], BF16, tag="xtr")
                    nc.tensor.transpose(
                        out=tps[:], in_=xg[:, k * P:(k + 1) * P],
                        identity=id_bf[:])
                    nc.vector.tensor_copy(
                        out=xgt[k][:, ct * P:(ct + 1) * P], in_=tps[:])

        # ---- expert weights (resident in SBUF) ----
        from concourse.bass import _add_dep_helper
        last_x = x_dma_insts[-1]
        w1_all = wpool.tile([P, NKH, I_DIM], BF16, tag="w1a")
        w3_all = wpool.tile([P, NKH, I_DIM], BF16, tag="w3a")
        w2_all = wpool.tile([P, NKI, H], BF16, tag="w2a")
        w_dmas = []
        w1v = w1[:].rearrange("(k p) i -> p k i", p=P)
        w3v = w3[:].rearrange("(k p) i -> p k i", p=P)
        w2v = w2[:].rearrange("(k p) h -> p k h", p=P)
        for half in range(2):
            k0, k1 = half * NKH // 2, (half + 1) * NKH // 2
            w_dmas.append(nc.scalar.dma_start(
                out=w1_all[:, k0:k1, :], in_=w1v[:, k0:k1, :]))
            w_dmas.append(nc.scalar.dma_start(
                out=w3_all[:, k0:k1, :], in_=w3v[:, k0:k1, :]))
        for half in range(2):
            k0, k1 = half * NKI // 2, (half + 1) * NKI // 2
            w_dmas.append(nc.scalar.dma_start(
                out=w2_all[:, k0:k1, :], in_=w2v[:, k0:k1, :]))

        for wd in w_dmas:
            _add_dep_helper(wd.ins, last_x.ins, True,
                            "weights stream after xt (routing DMA priority)")

        # ---- expert FFN: gate/up + SwiGLU -> hT, down -> yT ----
        with tc.tile_pool(name="psm", bufs=2, space="PSUM") as psm:
            hts = [ht_pool.tile([P, CAP], BF16, tag=f"ht{i}", name=f"ht{i}")
                   for i in range(NKI)]
            for it in range(NKI):
                i0 = it * P
                for c0, n in CHUNKS:
                    gps = psm.tile([P, 512], F32, tag="gate")
                    ups = psm.tile([P, 512], F32, tag="up")
                    for k in range(NKH):
                        nc.tensor.matmul(
                            out=gps[:, :n], lhsT=w1_all[:, k, i0:i0 + P],
                            rhs=xgt[k][:, c0:c0 + n],
                            start=(k == 0), stop=(k == NKH - 1))
                    for k in range(NKH):
                        nc.tensor.matmul(
                            out=ups[:, :n], lhsT=w3_all[:, k, i0:i0 + P],
                            rhs=xgt[k][:, c0:c0 + n],
                            start=(k == 0), stop=(k == NKH - 1))
                    sl = mm_pool.tile([P, 512], BF16, tag="silu")
                    nc.scalar.activation(out=sl[:, :n], in_=gps[:, :n],
                                         func=AF.Sigmoid)
                    tmp = mm_pool.tile([P, 512], BF16, tag="sgate")
                    nc.vector.tensor_tensor(
                        out=tmp[:, :n], in0=sl[:, :n], in1=gps[:, :n],
                        op=OP.mult)
                    nc.vector.tensor_tensor(
                        out=hts[it][:, c0:c0 + n], in0=tmp[:, :n],
                        in1=ups[:, :n], op=OP.mult)
            for ht_i in range(NKH):
                h0 = ht_i * P
                for c0, n in CHUNKS:
                    yps = psm.tile([P, 512], F32, tag="y")
                    for k in range(NKI):
                        nc.tensor.matmul(
                            out=yps[:, :n], lhsT=w2_all[:, k, h0:h0 + P],
                            rhs=hts[k][:, c0:c0 + n],
                            start=(k == 0), stop=(k == NKI - 1))
                    ysb = mm_pool.tile([P, 512], F32, tag="ysb")
                    nc.vector.tensor_tensor(
                        out=ysb[:, :n], in0=yps[:, :n],
                        in1=wbc_sb[:, c0:c0 + n], op=OP.mult)
                    nc.sync.dma_start(
                        out=yt[h0:h0 + P, c0:c0 + n], in_=ysb[:, :n])

    nc.compile()
    return nc


_NC_CACHE = None


def _get_program():
    global _NC_CACHE
    if _NC_CACHE is None:
        _NC_CACHE = build_program()
    return _NC_CACHE


def _prepare_in_maps(x, Wr, br, W1, W3, W2):
    x2d = np.ascontiguousarray(np.asarray(x, dtype=np.float32).reshape(S, H))
    # (k, p, ch, c) -> (ch, k, p, c): each routing chunk DMA is contiguous
    xt = np.ascontiguousarray(
        x2d.T.reshape(NKH, P, N_ROUTE_CHUNKS, ROUTE_CHUNK)
        .transpose(2, 0, 1, 3)
        .reshape(N_ROUTE_CHUNKS * NKH * P, ROUTE_CHUNK))
    xth = xt.astype(ml_dtypes.bfloat16)
    xtl = (xt - xth.astype(np.float32)).astype(ml_dtypes.bfloat16)
    xbf = x2d.astype(ml_dtypes.bfloat16)
    wr_np = np.ascontiguousarray(np.asarray(Wr, dtype=np.float32))
    wrh_np = wr_np.astype(ml_dtypes.bfloat16)
    wrl_np = (wr_np - wrh_np.astype(np.float32)).astype(ml_dtypes.bfloat16)
    br_np = np.asarray(br, dtype=np.float32).reshape(1, E)
    W1 = np.asarray(W1, dtype=np.float32)
    W3 = np.asarray(W3, dtype=np.float32)
    W2 = np.asarray(W2, dtype=np.float32)
    in_maps = []
    for e in range(N_CORES):
        oh_np = np.zeros((1, E), np.float32)
        oh_np[0, e] = 1.0
        in_maps.append({
            "xth": xth,
            "xtl": xtl,
            "xbf": xbf,
            "wrh": wrh_np,
            "wrl": wrl_np,
            "br": br_np,
            "oh": oh_np,
            "w1": W1[e].astype(ml_dtypes.bfloat16),
            "w3": W3[e].astype(ml_dtypes.bfloat16),
            "w2": W2[e].astype(ml_dtypes.bfloat16),
        })
    return in_maps


def _combine(results):
    out = np.zeros((S, H), np.float32)
    for e in range(N_CORES):
        idxw = np.asarray(results[e]["idxw"])
        yt = np.asarray(results[e]["yt"])
        idx = np.rint(idxw[0, :] * P + idxw[1, :]).astype(np.int64)
        np.add.at(out, idx, yt[:, :CAP].T)
    return out.reshape(B, S, H)


def run_on_device(inputs, trace=False, trace_cores=None):
    """Run the SPMD program; returns (full_output, BassKernelResults)."""
    nc = _get_program()
    in_maps = _prepare_in_maps(**inputs)
    kwargs = {}
    if trace:
        try:
            import types

            if "antenv.axon_hooks" not in sys.modules:
                from trn_agent_boot.trn_boot import _ntff_profile_via_ctypes

                hook = _ntff_profile_via_ctypes("/opt/axon/libaxon_pjrt.so")
                mod = types.ModuleType("antenv.axon_hooks")
                mod._hook = hook
                mod.get_axon_ntff_profile_hook = lambda: mod._hook

                def _set(h):
                    mod._hook = h

                mod.set_axon_ntff_profile_hook = _set
                sys.modules["antenv.axon_hooks"] = mod
                import antenv

                antenv.axon_hooks = mod
        except Exception as exc:  # profiling unavailable -> run untraced
            print(f"trace hook install failed: {exc}", file=sys.stderr)
        kwargs = dict(trace=True,
                      trace_cores=trace_cores or list(range(N_CORES)))
    res = run_bass_kernel_spmd(nc, in_maps, list(range(N_CORES)), **kwargs)
    return _combine(res.results), res


def kernel(x, Wr, br, W1, W3, W2):
    out, _ = run_on_device(dict(x=x, Wr=Wr, br=br, W1=W1, W3=W3, W2=W2))
    return out


# revision 21
# speedup vs baseline: 2.1952x; 1.0340x over previous
"""Expert-parallel MoE (top-2 of 8 experts, SwiGLU) on 8 Trainium2 NeuronCores.

Sharding: one expert per core (W1/W3/W2 sharded on the expert axis), router
replicated. Each core, fully on-device:
  1. Routing: logitsT = Wr.T @ x.T (fp32 PE matmul), PE-transpose to [tok, 8],
     top-2 + softmax -> per-token combine weight c for this core's expert.
  2. Compaction: cross-partition prefix sum (strictly-upper-triangular ones
     matmul) assigns every selected token a dense slot; an indirect-DMA
     scatter writes (token_id, c) pairs into a DRAM slot table.
  3. Dispatch: read the token ids back, indirect-DMA gather the selected rows
     of x (bf16), PE-transpose them to put H on partitions.
  4. Expert FFN: gate/up/down matmuls in bf16 with fp32 PSUM accumulation,
     SwiGLU, scale by c, emit yT [H, CAP] fp32 plus the slot table.
Host: out[idx_e] += yt_e.T accumulated over the 8 cores (the unshard step for
expert-parallel sharding). Unfilled slots carry c = 0 so they contribute 0.
"""
import sys

sys.path.insert(0, "/opt/trn_rl_repo")

from contextlib import ExitStack

import ml_dtypes
import numpy as np

import concourse.bacc as bacc
import concourse.bass as bass
import concourse.mybir as mybir
from concourse.bass_utils import run_bass_kernel_spmd
from concourse.masks import make_identity, make_upper_triangular
from concourse.tile import TileContext

F32 = mybir.dt.float32
BF16 = mybir.dt.bfloat16
I32 = mybir.dt.int32
AF = mybir.ActivationFunctionType
OP = mybir.AluOpType

P = 128
B, S, H, I_DIM, E, TOP_K = 1, 2048, 1024, 2048, 8, 2
NTT = S // P        # 16 token tiles
NKH = H // P        # 8 k-tiles over H
NKI = I_DIM // P    # 16 i-tiles
CAP = 640           # per-expert token capacity (multiple of 128)
NCT = CAP // P
TRASH = CAP         # trash row of the (CAP+1)-row slot table
BIG = 3.0e38
N_CORES = 8

CHUNKS = [(0, 512), (512, 128)]   # token chunks for the expert matmuls
ROUTE_CHUNK = 256
N_ROUTE_CHUNKS = S // ROUTE_CHUNK


def build_program():
    nc = bacc.Bacc("TRN2", target_bir_lowering=False, debug=False,
                   num_devices=N_CORES)

    xtc = nc.dram_tensor(
        "xtc", [N_ROUTE_CHUNKS * P, 2 * NKH * ROUTE_CHUNK], BF16,
        kind="ExternalInput")
    xbf = nc.dram_tensor("xbf", [S, H], BF16, kind="ExternalInput")
    wrc = nc.dram_tensor("wrc", [P, 2 * NKH * E], BF16, kind="ExternalInput")
    br = nc.dram_tensor("br", [1, E], F32, kind="ExternalInput")
    oh = nc.dram_tensor("oh", [1, E], F32, kind="ExternalInput")
    w1 = nc.dram_tensor("w1", [P, NKH * I_DIM], BF16, kind="ExternalInput")
    w3 = nc.dram_tensor("w3", [P, NKH * I_DIM], BF16, kind="ExternalInput")
    w2 = nc.dram_tensor("w2", [P, NKI * H], BF16, kind="ExternalInput")
    # slot table: row 0 = token ids, row 1 = combine weights, per slot
    idxw = nc.dram_tensor("idxw", [4, CAP], F32, kind="ExternalOutput")
    yt = nc.dram_tensor("yt", [H, CAP], F32, kind="ExternalOutput")

    with TileContext(nc) as tc, ExitStack() as ctx:
        const = ctx.enter_context(tc.tile_pool(name="const", bufs=1))
        route = ctx.enter_context(tc.tile_pool(name="route", bufs=1))
        xtch_pool = ctx.enter_context(tc.tile_pool(name="xtch", bufs=3))
        scr = ctx.enter_context(tc.tile_pool(name="scr", bufs=4))
        disp = ctx.enter_context(tc.tile_pool(name="disp", bufs=1))
        wpool = ctx.enter_context(tc.tile_pool(name="wpool", bufs=1))
        xgt_pool = ctx.enter_context(tc.tile_pool(name="xgt", bufs=1))
        xg_pool = ctx.enter_context(tc.tile_pool(name="xg", bufs=3))
        ht_pool = ctx.enter_context(tc.tile_pool(name="ht", bufs=1))
        mm_pool = ctx.enter_context(tc.tile_pool(name="mm", bufs=2))

        # ---- constants ----
        id_f32 = const.tile([P, P], F32, tag="idf")
        make_identity(nc, id_f32[:])
        id_bf = const.tile([P, P], BF16, tag="idb")
        make_identity(nc, id_bf[:])
        u128 = const.tile([P, P], F32, tag="u128")  # strictly-upper ones
        make_upper_triangular(nc, u128[:], val=1.0, diag=False)
        ones_col = const.tile([1, P], F32, tag="ones")
        nc.vector.memset(ones_col[:], 1.0)
        ones128 = const.tile([P, 1], F32, tag="ones128")
        nc.vector.memset(ones128[:], 1.0)
        zeros16 = const.tile([1, NTT], F32, tag="z16")
        nc.vector.memset(zeros16[:], 0.0)
        iota640 = const.tile([P, CAP], F32, tag="iota640")
        ii = mm_pool.tile([P, CAP], I32, tag="iotai", bufs=1)
        nc.gpsimd.iota(ii[:], pattern=[[1, CAP]], base=0, channel_multiplier=0)
        nc.vector.tensor_copy(out=iota640[:], in_=ii[:])
        # rowsel[p, j] = 1 for p >= 2 (sums the c_hi + c_lo payload rows)
        rowsel = const.tile([4, P], F32, tag="rowsel")
        nc.gpsimd.memset(rowsel[:], 0.0)
        nc.gpsimd.affine_select(
            out=rowsel[:], in_=rowsel[:], pattern=[[0, P]],
            compare_op=OP.is_ge, fill=1.0, base=1, channel_multiplier=-1)
        br_bc = const.tile([P, E], F32, tag="brbc")
        nc.sync.dma_start(out=br_bc[:], in_=br[:].to_broadcast((P, E)))
        oh_bc = const.tile([P, E], F32, tag="ohbc")
        nc.sync.dma_start(out=oh_bc[:], in_=oh[:].to_broadcast((P, E)))
        wr_sb = const.tile([P, 2, NKH, E], BF16, tag="wrc")
        nc.sync.dma_start(out=wr_sb[:], in_=wrc[:])

        x_dma_insts = []
        with tc.tile_pool(name="psr", bufs=2, space="PSUM") as psr:
            # PE warmup: keep TensorE busy from t=0 so HAM un-throttles before
            # the routing matmuls start (they wait ~15us for the first DMAs).
            warm_ps = psr.tile([P, P], BF16, tag="excl", bufs=1)
            for _ in range(90):
                nc.tensor.transpose(
                    out=warm_ps[:], in_=id_bf[:], identity=id_bf[:])

            # ---- routing: logitsT [E, S] = Wr.T @ xT (Wr stationary: the
            # 8-column weight load is nearly free; fp32 cost rides the moving
            # rows) ----
            logT = route.tile([E, S], F32, tag="logT")
            for ch in range(N_ROUTE_CHUNKS):
                c0 = ch * ROUTE_CHUNK
                lps = psr.tile([E, ROUTE_CHUNK], F32, tag="lps")
                xts = xtch_pool.tile([P, 2, NKH, ROUTE_CHUNK], BF16, tag="xtch",
                                     name=f"xtch_{ch}")
                xdma = nc.sync.dma_start(
                    out=xts[:], in_=xtc[ch * P:(ch + 1) * P, :])
                x_dma_insts.append(xdma)
                # exact-enough fp32: hi*hi + hi*lo + lo*hi (lo*lo ~ 2^-16)
                n_mm = 3 * NKH
                mi = 0
                for k in range(NKH):
                    for half, xhalf in ((0, 0), (1, 0), (0, 1)):
                        nc.tensor.matmul(
                            out=lps[:], lhsT=wr_sb[:, half, k, :],
                            rhs=xts[:, xhalf, k, :],
                            start=(mi == 0), stop=(mi == n_mm - 1))
                        mi += 1
                nc.vector.tensor_copy(out=logT[:, c0:c0 + ROUTE_CHUNK], in_=lps[:])

            # ---- top-2 + softmax, all 16 token tiles at once ----
            # transpose each [E, 128] logit block into one [P, NTT*E] PSUM
            # bank (disjoint columns; single-shot groups, safe in order)
            trps = psr.tile([P, NTT * E], F32, tag="trps", bufs=1)
            for t in range(NTT):
                nc.tensor.matmul(
                    out=trps[:, t * E:(t + 1) * E],
                    lhsT=logT[:E, t * P:(t + 1) * P],
                    rhs=id_f32[:E, :E],
                    is_transpose=True, start=True, stop=True,
                    skip_group_check=True)
            l_all = disp.tile([P, NTT * E], F32, tag="lall")
            nc.vector.tensor_tensor(
                out=l_all[:].rearrange("p (t e) -> p t e", e=E),
                in0=trps[:].rearrange("p (t e) -> p t e", e=E),
                in1=br_bc[:].rearrange("p e -> p () e").to_broadcast((P, NTT, E)),
                op=OP.add)

            cm_all = disp.tile([P, NTT], F32, tag="cm")
            sel_all = disp.tile([P, NTT], F32, tag="sel")
            tokf = disp.tile([P, NTT], F32, tag="tokf")
            toki = scr.tile([P, NTT], I32, tag="toki")
            nc.gpsimd.iota(toki[:], pattern=[[P, NTT]], base=0,
                           channel_multiplier=1)
            nc.vector.tensor_copy(out=tokf[:], in_=toki[:])

            def bcast(ap):  # [P, NTT] -> [P, NTT, E] stride-0 view
                return ap.to_broadcast((P, NTT, E))

            l3 = l_all[:].rearrange("p (t e) -> p t e", e=E)
            m1 = scr.tile([P, NTT], F32, tag="m1")
            nc.vector.tensor_reduce(
                out=m1[:], in_=l3, axis=mybir.AxisListType.X, op=OP.max)
            mask1 = scr.tile([P, NTT * E], F32, tag="mask1")
            mask1_3 = mask1[:].rearrange("p (t e) -> p t e", e=E)
            nc.vector.tensor_tensor(
                out=mask1_3, in0=l3, in1=bcast(m1[:]), op=OP.is_equal)
            l2 = scr.tile([P, NTT * E], F32, tag="l2")
            l2_3 = l2[:].rearrange("p (t e) -> p t e", e=E)
            nc.vector.tensor_scalar(
                out=l2[:], in0=mask1[:], scalar1=-BIG, scalar2=None,
                op0=OP.mult)
            nc.vector.tensor_add(l2[:], l2[:], l_all[:])
            m2 = scr.tile([P, NTT], F32, tag="m2")
            nc.vector.tensor_reduce(
                out=m2[:], in_=l2_3, axis=mybir.AxisListType.X, op=OP.max)
            mask2 = scr.tile([P, NTT * E], F32, tag="mask2")
            mask2_3 = mask2[:].rearrange("p (t e) -> p t e", e=E)
            nc.vector.tensor_tensor(
                out=mask2_3, in0=l2_3, in1=bcast(m2[:]), op=OP.is_equal)
            d = scr.tile([P, NTT], F32, tag="d")
            nc.vector.tensor_sub(d[:], m2[:], m1[:])
            ed = scr.tile([P, NTT], F32, tag="ed")
            nc.scalar.activation(out=ed[:], in_=d[:], func=AF.Exp)
            den = scr.tile([P, NTT], F32, tag="den")
            nc.vector.tensor_scalar_add(den[:], ed[:], 1.0)
            w1c = scr.tile([P, NTT], F32, tag="w1c")
            nc.vector.reciprocal(w1c[:], den[:])
            w2c = scr.tile([P, NTT], F32, tag="w2c")
            nc.vector.tensor_mul(w2c[:], ed[:], w1c[:])
            # c[p,t,e] = mask1*w1 + mask2*w2; then pick this core's expert
            call = scr.tile([P, NTT * E], F32, tag="call")
            call_3 = call[:].rearrange("p (t e) -> p t e", e=E)
            nc.vector.tensor_tensor(
                out=call_3, in0=mask1_3, in1=bcast(w1c[:]), op=OP.mult)
            c2t = scr.tile([P, NTT * E], F32, tag="c2t")
            c2_3 = c2t[:].rearrange("p (t e) -> p t e", e=E)
            nc.vector.tensor_tensor(
                out=c2_3, in0=mask2_3, in1=bcast(w2c[:]), op=OP.mult)
            nc.vector.tensor_add(call[:], call[:], c2t[:])
            cm8 = scr.tile([P, NTT * E], F32, tag="cm8")
            cm8_3 = cm8[:].rearrange("p (t e) -> p t e", e=E)
            nc.vector.tensor_tensor(
                out=cm8_3, in0=call_3,
                in1=oh_bc[:].rearrange("p e -> p () e").to_broadcast((P, NTT, E)),
                op=OP.mult)
            nc.vector.tensor_reduce(
                out=cm_all[:], in_=cm8_3, axis=mybir.AxisListType.X, op=OP.add)
            nc.vector.tensor_scalar(
                out=sel_all[:], in0=cm_all[:], scalar1=0.0, scalar2=None,
                op0=OP.is_gt)

            # ---- compaction: dense slot per selected token ----
            excl_ps = psr.tile([P, NTT], F32, tag="excl", bufs=1)
            nc.tensor.matmul(
                out=excl_ps[:], lhsT=u128[:], rhs=sel_all[:], start=True,
                stop=True)
            excl = disp.tile([P, NTT], F32, tag="exclsb")
            nc.vector.tensor_copy(out=excl[:], in_=excl_ps[:])
            tot_ps = psr.tile([1, NTT], F32, tag="totps", bufs=1)
            nc.tensor.matmul(
                out=tot_ps[:], lhsT=ones128[:], rhs=sel_all[:], start=True,
                stop=True)
            incl = disp.tile([1, NTT], F32, tag="incl")
            nc.vector.tensor_tensor_scan(
                out=incl[:], data0=tot_ps[:], data1=zeros16[:], initial=0.0,
                op0=OP.add, op1=OP.add)
            offs = disp.tile([1, NTT], F32, tag="offs")
            nc.vector.tensor_sub(offs[:], incl[:], tot_ps[:])
            offs_ps = psr.tile([P, NTT], F32, tag="offsps", bufs=1)
            nc.tensor.matmul(
                out=offs_ps[:], lhsT=ones_col[:], rhs=offs[:], start=True,
                stop=True)
            slot = disp.tile([P, NTT], F32, tag="slot")
            nc.vector.tensor_tensor(
                out=slot[:], in0=excl[:], in1=offs_ps[:], op=OP.add)
            # unselected tokens -> far past any real slot
            nc.vector.tensor_scalar_sub(slot[:], slot[:], float(TRASH))
            nc.vector.tensor_mul(slot[:], slot[:], sel_all[:])
            nc.vector.tensor_scalar_add(slot[:], slot[:], float(TRASH))

            # payload rows per token: tile idx, partition idx, c split into
            # bf16 hi+lo halves (hi+lo is fp32-exact to ~1.5e-5)
            tcol = scr.tile([P, 1], F32, tag="tcol")
            chi = scr.tile([P, NTT], F32, tag="chi")
            clo = scr.tile([P, NTT], F32, tag="clo")
            chib = scr.tile([P, NTT], BF16, tag="chib")
            nc.vector.tensor_copy(out=chib[:], in_=cm_all[:])
            nc.vector.tensor_copy(out=chi[:], in_=chib[:])
            nc.vector.tensor_sub(clo[:], cm_all[:], chi[:])
            tvals = scr.tile([P, NTT], F32, tag="tvals")
            ti = scr.tile([P, NTT], I32, tag="ti")
            nc.gpsimd.iota(ti[:], pattern=[[1, NTT]], base=0,
                           channel_multiplier=0)
            nc.vector.tensor_copy(out=tvals[:], in_=ti[:])
            pvals = scr.tile([P, 1], I32, tag="pvals")
            nc.gpsimd.iota(pvals[:], pattern=[[1, 1]], base=0,
                           channel_multiplier=1)
            pvf = scr.tile([P, 1], F32, tag="pvf")
            nc.vector.tensor_copy(out=pvf[:], in_=pvals[:])

            pairb = disp.tile([P, 4 * NTT], BF16, tag="pairb")
            pb4 = pairb[:].rearrange("p (t four) -> p t four", four=4)
            nc.vector.tensor_copy(
                out=pb4[:, :, 0:1], in_=tvals[:].rearrange("p t -> p t ()"))
            nc.vector.tensor_copy(
                out=pb4[:, :, 1:2],
                in_=pvf[:].rearrange("p o -> p () o").to_broadcast((P, NTT, 1)))
            nc.vector.tensor_copy(
                out=pb4[:, :, 2:3], in_=chi[:].rearrange("p t -> p t ()"))
            nc.vector.tensor_copy(
                out=pb4[:, :, 3:4], in_=clo[:].rearrange("p t -> p t ()"))

            # ---- on-chip inverse permutation via one-hot matmuls ----
            # cmp_t[p, s] = (slot[p, t] == s); pe[4, s] += pairb[:,t].T @ cmp_t
            # Exactly one token matches each filled slot, so the sums are
            # single-term; ids are small ints, exact in bf16.
            pe_parts = []
            for c0, n in CHUNKS:
                pe_parts.append(psr.tile(
                    [4, 512], F32, tag=f"pe{c0}", bufs=1, name=f"pe_ps{c0}"))
            for t in range(NTT):
                cmp = scr.tile([P, CAP], BF16, tag="cmp", bufs=3)
                nc.vector.tensor_tensor(
                    out=cmp[:], in0=slot[:, t:t + 1].to_broadcast((P, CAP)),
                    in1=iota640[:], op=OP.is_equal)
                for ci, (c0, n) in enumerate(CHUNKS):
                    nc.tensor.matmul(
                        out=pe_parts[ci][:, :n], lhsT=pairb[:, 4 * t:4 * t + 4],
                        rhs=cmp[:, c0:c0 + n],
                        start=(t == 0), stop=(t == NTT - 1))
            pe_sb = disp.tile([4, CAP], F32, tag="pesb")
            for ci, (c0, n) in enumerate(CHUNKS):
                nc.vector.tensor_copy(
                    out=pe_sb[:, c0:c0 + n], in_=pe_parts[ci][:, :n])
            # ship the slot table to the host (host: idx = 128*row0 + row1)
            nc.sync.dma_start(out=idxw[:], in_=pe_sb[:])

        # ---- dispatch: gather selected x rows, transpose to [H, CAP] ----
        with tc.tile_pool(name="psd", bufs=2, space="PSUM") as psd:
            # broadcast c over partitions: wbc[p, s] = c_hi[s] + c_lo[s]
            wbc_sb = disp.tile([P, CAP], F32, tag="wbc")
            for c0, n in CHUNKS:
                wps = psd.tile([P, 512], F32, tag="wbcps", bufs=1)
                nc.tensor.matmul(
                    out=wps[:, :n], lhsT=rowsel[:], rhs=pe_sb[:, c0:c0 + n],
                    start=True, stop=True)
                nc.vector.tensor_copy(out=wbc_sb[:, c0:c0 + n], in_=wps[:, :n])

            xgt = [xgt_pool.tile([P, CAP], BF16, tag=f"xgt{k}", name=f"xgt{k}")
                   for k in range(NKH)]
            for ct in range(NCT):
                # idx per capacity tile: transpose pe_sb[:, ct*P:+P] -> [P, 4]
                trp = psd.tile([P, 4], F32, tag="idxtr")
                nc.tensor.matmul(
                    out=trp[:], lhsT=pe_sb[:4, ct * P:(ct + 1) * P],
                    rhs=id_f32[:4, :4],
                    is_transpose=True, start=True, stop=True)
                idx_f = scr.tile([P, 1], F32, tag="idxf")
                nc.vector.tensor_scalar(
                    out=idx_f[:], in0=trp[:, 0:1], scalar1=float(P),
                    scalar2=trp[:, 1:2], op0=OP.mult, op1=OP.add)
                idx_i = scr.tile([P, 1], I32, tag="idxi")
                nc.vector.tensor_copy(out=idx_i[:], in_=idx_f[:])
                xg = xg_pool.tile([P, H], BF16, tag="xg")
                nc.gpsimd.indirect_dma_start(
                    out=xg[:],
                    out_offset=None,
                    in_=xbf[:],
                    in_offset=bass.IndirectOffsetOnAxis(ap=idx_i[:, 0:1], axis=0))
                for k in range(NKH):
                    tps = psd.tile([P, P], BF16, tag="xtr")
                    nc.tensor.transpose(
                        out=tps[:], in_=xg[:, k * P:(k + 1) * P],
                        identity=id_bf[:])
                    nc.vector.tensor_copy(
                        out=xgt[k][:, ct * P:(ct + 1) * P], in_=tps[:])

        # ---- expert weights (resident in SBUF) ----
        from concourse.bass import _add_dep_helper
        last_x = x_dma_insts[-1]
        w1_all = wpool.tile([P, NKH, I_DIM], BF16, tag="w1a")
        w3_all = wpool.tile([P, NKH, I_DIM], BF16, tag="w3a")
        w2_all = wpool.tile([P, NKI, H], BF16, tag="w2a")
        w_dmas = []
        for half in range(2):
            k0, k1 = half * NKH // 2, (half + 1) * NKH // 2
            w_dmas.append(nc.scalar.dma_start(
                out=w1_all[:, k0:k1, :],
                in_=w1[:, k0 * I_DIM:k1 * I_DIM]))
            w_dmas.append(nc.scalar.dma_start(
                out=w3_all[:, k0:k1, :],
                in_=w3[:, k0 * I_DIM:k1 * I_DIM]))
        for half in range(2):
            k0, k1 = half * NKI // 2, (half + 1) * NKI // 2
            w_dmas.append(nc.scalar.dma_start(
                out=w2_all[:, k0:k1, :],
                in_=w2[:, k0 * H:k1 * H]))

        for wd in w_dmas:
            _add_dep_helper(wd.ins, last_x.ins, True,
                            "weights stream after xt (routing DMA priority)")

        # ---- expert FFN: gate/up + SwiGLU -> hT, down -> yT ----
        with tc.tile_pool(name="psm", bufs=2, space="PSUM") as psm:
            hts = [ht_pool.tile([P, CAP], BF16, tag=f"ht{i}", name=f"ht{i}")
                   for i in range(NKI)]
            for it in range(NKI):
                i0 = it * P
                for c0, n in CHUNKS:
                    gps = psm.tile([P, 512], F32, tag="gate")
                    ups = psm.tile([P, 512], F32, tag="up")
                    for k in range(NKH):
                        nc.tensor.matmul(
                            out=gps[:, :n], lhsT=w1_all[:, k, i0:i0 + P],
                            rhs=xgt[k][:, c0:c0 + n],
                            start=(k == 0), stop=(k == NKH - 1))
                    for k in range(NKH):
                        nc.tensor.matmul(
                            out=ups[:, :n], lhsT=w3_all[:, k, i0:i0 + P],
                            rhs=xgt[k][:, c0:c0 + n],
                            start=(k == 0), stop=(k == NKH - 1))
                    sl = mm_pool.tile([P, 512], BF16, tag="silu")
                    nc.scalar.activation(out=sl[:, :n], in_=gps[:, :n],
                                         func=AF.Sigmoid)
                    tmp = mm_pool.tile([P, 512], BF16, tag="sgate")
                    nc.vector.tensor_tensor(
                        out=tmp[:, :n], in0=sl[:, :n], in1=gps[:, :n],
                        op=OP.mult)
                    nc.vector.tensor_tensor(
                        out=hts[it][:, c0:c0 + n], in0=tmp[:, :n],
                        in1=ups[:, :n], op=OP.mult)
            for ht_i in range(NKH):
                h0 = ht_i * P
                ysb = mm_pool.tile([P, CAP], F32, tag="ysb")
                for c0, n in CHUNKS:
                    yps = psm.tile([P, 512], F32, tag="y")
                    for k in range(NKI):
                        nc.tensor.matmul(
                            out=yps[:, :n], lhsT=w2_all[:, k, h0:h0 + P],
                            rhs=hts[k][:, c0:c0 + n],
                            start=(k == 0), stop=(k == NKI - 1))
                    nc.vector.tensor_tensor(
                        out=ysb[:, c0:c0 + n], in0=yps[:, :n],
                        in1=wbc_sb[:, c0:c0 + n], op=OP.mult)
                nc.sync.dma_start(out=yt[h0:h0 + P, :], in_=ysb[:])

    nc.compile()
    return nc


_NC_CACHE = None


def _get_program():
    global _NC_CACHE
    if _NC_CACHE is None:
        _NC_CACHE = build_program()
    return _NC_CACHE


def _prepare_in_maps(x, Wr, br, W1, W3, W2):
    x2d = np.ascontiguousarray(np.asarray(x, dtype=np.float32).reshape(S, H))
    # (k, p, ch, c) -> (ch, k, p, c): each routing chunk DMA is contiguous
    xt = np.ascontiguousarray(
        x2d.T.reshape(NKH, P, N_ROUTE_CHUNKS, ROUTE_CHUNK)
        .transpose(2, 0, 1, 3)
        .reshape(N_ROUTE_CHUNKS * NKH * P, ROUTE_CHUNK))
    xth = xt.astype(ml_dtypes.bfloat16)
    xtl = (xt - xth.astype(np.float32)).astype(ml_dtypes.bfloat16)
    # fused per-chunk layout [ch*P, 2*NKH*RC]: row p = [hi(k0..k7) | lo(k0..k7)]
    def _chunkify(a):
        return a.reshape(N_ROUTE_CHUNKS, NKH, P, ROUTE_CHUNK).transpose(0, 2, 1, 3)
    xtc = np.concatenate([_chunkify(xth), _chunkify(xtl)], axis=2)
    xtc = np.ascontiguousarray(
        xtc.reshape(N_ROUTE_CHUNKS, P, 2, NKH, ROUTE_CHUNK)
        .reshape(N_ROUTE_CHUNKS * P, 2 * NKH * ROUTE_CHUNK))
    xbf = x2d.astype(ml_dtypes.bfloat16)
    wr_np = np.ascontiguousarray(np.asarray(Wr, dtype=np.float32))
    wrh_np = wr_np.astype(ml_dtypes.bfloat16)
    wrl_np = (wr_np - wrh_np.astype(np.float32)).astype(ml_dtypes.bfloat16)
    # [P, 2*NKH*E]: row p = [hi(k0..k7) | lo(k0..k7)] of Wr[k*P+p, :]
    def _wrpack(a):
        return a.reshape(NKH, P, E).transpose(1, 0, 2)
    wrc_np = np.ascontiguousarray(
        np.stack([_wrpack(wrh_np), _wrpack(wrl_np)], axis=1)
        .reshape(P, 2 * NKH * E))
    br_np = np.asarray(br, dtype=np.float32).reshape(1, E)
    W1 = np.asarray(W1, dtype=np.float32)
    W3 = np.asarray(W3, dtype=np.float32)
    W2 = np.asarray(W2, dtype=np.float32)
    in_maps = []
    for e in range(N_CORES):
        oh_np = np.zeros((1, E), np.float32)
        oh_np[0, e] = 1.0
        def _wpack(a, nk):
            return np.ascontiguousarray(
                a.reshape(nk, P, -1).transpose(1, 0, 2).reshape(P, -1))
        in_maps.append({
            "xtc": xtc,
            "xbf": xbf,
            "wrc": wrc_np,
            "br": br_np,
            "oh": oh_np,
            "w1": _wpack(W1[e].astype(ml_dtypes.bfloat16), NKH),
            "w3": _wpack(W3[e].astype(ml_dtypes.bfloat16), NKH),
            "w2": _wpack(W2[e].astype(ml_dtypes.bfloat16), NKI),
        })
    return in_maps


def _combine(results):
    out = np.zeros((S, H), np.float32)
    for e in range(N_CORES):
        idxw = np.asarray(results[e]["idxw"])
        yt = np.asarray(results[e]["yt"])
        idx = np.rint(idxw[0, :] * P + idxw[1, :]).astype(np.int64)
        np.add.at(out, idx, yt[:, :CAP].T)
    return out.reshape(B, S, H)


def run_on_device(inputs, trace=False, trace_cores=None):
    """Run the SPMD program; returns (full_output, BassKernelResults)."""
    nc = _get_program()
    in_maps = _prepare_in_maps(**inputs)
    kwargs = {}
    if trace:
        try:
            import types

            if "antenv.axon_hooks" not in sys.modules:
                from trn_agent_boot.trn_boot import _ntff_profile_via_ctypes

                hook = _ntff_profile_via_ctypes("/opt/axon/libaxon_pjrt.so")
                mod = types.ModuleType("antenv.axon_hooks")
                mod._hook = hook
                mod.get_axon_ntff_profile_hook = lambda: mod._hook

                def _set(h):
                    mod._hook = h

                mod.set_axon_ntff_profile_hook = _set
                sys.modules["antenv.axon_hooks"] = mod
                import antenv

                antenv.axon_hooks = mod
        except Exception as exc:  # profiling unavailable -> run untraced
            print(f"trace hook install failed: {exc}", file=sys.stderr)
        kwargs = dict(trace=True,
                      trace_cores=trace_cores or list(range(N_CORES)))
    res = run_bass_kernel_spmd(nc, in_maps, list(range(N_CORES)), **kwargs)
    return _combine(res.results), res


def kernel(x, Wr, br, W1, W3, W2):
    out, _ = run_on_device(dict(x=x, Wr=Wr, br=br, W1=W1, W3=W3, W2=W2))
    return out


# revision 23
# speedup vs baseline: 2.2076x; 1.0056x over previous
"""Expert-parallel MoE (top-2 of 8 experts, SwiGLU) on 8 Trainium2 NeuronCores.

Sharding: one expert per core (W1/W3/W2 sharded on the expert axis), router
replicated. Each core, fully on-device:
  1. Routing: logitsT = Wr.T @ x.T (fp32 PE matmul), PE-transpose to [tok, 8],
     top-2 + softmax -> per-token combine weight c for this core's expert.
  2. Compaction: cross-partition prefix sum (strictly-upper-triangular ones
     matmul) assigns every selected token a dense slot; an indirect-DMA
     scatter writes (token_id, c) pairs into a DRAM slot table.
  3. Dispatch: read the token ids back, indirect-DMA gather the selected rows
     of x (bf16), PE-transpose them to put H on partitions.
  4. Expert FFN: gate/up/down matmuls in bf16 with fp32 PSUM accumulation,
     SwiGLU, scale by c, emit yT [H, CAP] fp32 plus the slot table.
Host: out[idx_e] += yt_e.T accumulated over the 8 cores (the unshard step for
expert-parallel sharding). Unfilled slots carry c = 0 so they contribute 0.
"""
import sys

sys.path.insert(0, "/opt/trn_rl_repo")

from contextlib import ExitStack

import ml_dtypes
import numpy as np

import concourse.bacc as bacc
import concourse.bass as bass
import concourse.mybir as mybir
from concourse.bass_utils import run_bass_kernel_spmd
from concourse.masks import make_identity, make_upper_triangular
from concourse.tile import TileContext

F32 = mybir.dt.float32
BF16 = mybir.dt.bfloat16
FP16 = mybir.dt.float16
I32 = mybir.dt.int32
AF = mybir.ActivationFunctionType
OP = mybir.AluOpType

P = 128
B, S, H, I_DIM, E, TOP_K = 1, 2048, 1024, 2048, 8, 2
NTT = S // P        # 16 token tiles
NKH = H // P        # 8 k-tiles over H
NKI = I_DIM // P    # 16 i-tiles
CAP = 640           # per-expert token capacity (multiple of 128)
NCT = CAP // P
TRASH = CAP         # trash row of the (CAP+1)-row slot table
BIG = 3.0e38
N_CORES = 8

CHUNKS = [(0, 512), (512, 128)]   # token chunks for the expert matmuls
ROUTE_CHUNK = 256
N_ROUTE_CHUNKS = S // ROUTE_CHUNK


def build_program():
    nc = bacc.Bacc("TRN2", target_bir_lowering=False, debug=False,
                   num_devices=N_CORES)

    xtc = nc.dram_tensor(
        "xtc", [N_ROUTE_CHUNKS * P, 2 * NKH * ROUTE_CHUNK], BF16,
        kind="ExternalInput")
    xbf = nc.dram_tensor("xbf", [S, H], BF16, kind="ExternalInput")
    wrc = nc.dram_tensor("wrc", [P, 2 * NKH * E], BF16, kind="ExternalInput")
    br = nc.dram_tensor("br", [1, E], F32, kind="ExternalInput")
    oh = nc.dram_tensor("oh", [1, E], F32, kind="ExternalInput")
    w1 = nc.dram_tensor("w1", [P, NKH * I_DIM], BF16, kind="ExternalInput")
    w3 = nc.dram_tensor("w3", [P, NKH * I_DIM], BF16, kind="ExternalInput")
    w2 = nc.dram_tensor("w2", [P, NKI * H], BF16, kind="ExternalInput")
    # slot table: row 0 = token ids, row 1 = combine weights, per slot
    idxw = nc.dram_tensor("idxw", [4, CAP], F32, kind="ExternalOutput")
    yt = nc.dram_tensor("yt", [H, CAP], F32, kind="ExternalOutput")

    with TileContext(nc) as tc, ExitStack() as ctx:
        const = ctx.enter_context(tc.tile_pool(name="const", bufs=1))
        route = ctx.enter_context(tc.tile_pool(name="route", bufs=1))
        xtch_pool = ctx.enter_context(tc.tile_pool(name="xtch", bufs=3))
        scr = ctx.enter_context(tc.tile_pool(name="scr", bufs=4))
        disp = ctx.enter_context(tc.tile_pool(name="disp", bufs=1))
        wpool = ctx.enter_context(tc.tile_pool(name="wpool", bufs=1))
        xgt_pool = ctx.enter_context(tc.tile_pool(name="xgt", bufs=1))
        xg_pool = ctx.enter_context(tc.tile_pool(name="xg", bufs=3))
        ht_pool = ctx.enter_context(tc.tile_pool(name="ht", bufs=1))
        mm_pool = ctx.enter_context(tc.tile_pool(name="mm", bufs=2))

        # ---- constants ----
        id_f32 = const.tile([P, P], F32, tag="idf")
        make_identity(nc, id_f32[:])
        id_bf = const.tile([P, P], BF16, tag="idb")
        make_identity(nc, id_bf[:])
        u128 = const.tile([P, P], F32, tag="u128")  # strictly-upper ones
        make_upper_triangular(nc, u128[:], val=1.0, diag=False)
        ones_col = const.tile([1, P], F32, tag="ones")
        nc.vector.memset(ones_col[:], 1.0)
        ones128 = const.tile([P, 1], F32, tag="ones128")
        nc.vector.memset(ones128[:], 1.0)
        zeros16 = const.tile([1, NTT], F32, tag="z16")
        nc.vector.memset(zeros16[:], 0.0)
        iota640 = const.tile([P, CAP], FP16, tag="iota640")
        ii = mm_pool.tile([P, CAP], I32, tag="iotai", bufs=1)
        nc.gpsimd.iota(ii[:], pattern=[[1, CAP]], base=0, channel_multiplier=0)
        nc.vector.tensor_copy(out=iota640[:], in_=ii[:])
        # rowsel[p, j] = 1 for p >= 2 (sums the c_hi + c_lo payload rows)
        rowsel = const.tile([4, P], FP16, tag="rowsel")
        nc.gpsimd.memset(rowsel[:], 0.0)
        nc.gpsimd.affine_select(
            out=rowsel[:], in_=rowsel[:], pattern=[[0, P]],
            compare_op=OP.is_ge, fill=1.0, base=1, channel_multiplier=-1)
        br_bc = const.tile([P, E], F32, tag="brbc")
        nc.sync.dma_start(out=br_bc[:], in_=br[:].to_broadcast((P, E)))
        oh_bc = const.tile([P, E], F32, tag="ohbc")
        nc.sync.dma_start(out=oh_bc[:], in_=oh[:].to_broadcast((P, E)))
        wr_sb = const.tile([P, 2, NKH, E], BF16, tag="wrc")
        nc.sync.dma_start(out=wr_sb[:], in_=wrc[:])

        x_dma_insts = []
        with tc.tile_pool(name="psr", bufs=2, space="PSUM") as psr:
            # PE warmup: keep TensorE busy from t=0 so HAM un-throttles before
            # the routing matmuls start (they wait ~15us for the first DMAs).
            warm_ps = psr.tile([P, P], BF16, tag="excl", bufs=1)
            for _ in range(90):
                nc.tensor.transpose(
                    out=warm_ps[:], in_=id_bf[:], identity=id_bf[:])

            # ---- routing: logitsT [E, S] = Wr.T @ xT (Wr stationary: the
            # 8-column weight load is nearly free; fp32 cost rides the moving
            # rows) ----
            # per-chunk logitsT tiles; transpose into the shared trps bank
            # right after each chunk (disjoint columns; groups close in order)
            trps = psr.tile([P, NTT * E], F32, tag="trps", bufs=1)
            tiles_per_chunk = ROUTE_CHUNK // P
            for ch in range(N_ROUTE_CHUNKS):
                lps = psr.tile([E, ROUTE_CHUNK], F32, tag="lps")
                xts = xtch_pool.tile([P, 2, NKH, ROUTE_CHUNK], BF16, tag="xtch",
                                     name=f"xtch_{ch}")
                xdma = nc.sync.dma_start(
                    out=xts[:], in_=xtc[ch * P:(ch + 1) * P, :])
                x_dma_insts.append(xdma)
                # exact-enough fp32: hi*hi + hi*lo + lo*hi (lo*lo ~ 2^-16)
                n_mm = 3 * NKH
                mi = 0
                for k in range(NKH):
                    for half, xhalf in ((0, 0), (1, 0), (0, 1)):
                        nc.tensor.matmul(
                            out=lps[:], lhsT=wr_sb[:, half, k, :],
                            rhs=xts[:, xhalf, k, :],
                            start=(mi == 0), stop=(mi == n_mm - 1))
                        mi += 1
                lsb = route.tile([E, ROUTE_CHUNK], F32, tag="lsb", bufs=3,
                                 name=f"lsb{ch}")
                nc.vector.tensor_copy(out=lsb[:], in_=lps[:])
                for tt in range(tiles_per_chunk):
                    t = ch * tiles_per_chunk + tt
                    nc.tensor.matmul(
                        out=trps[:, t * E:(t + 1) * E],
                        lhsT=lsb[:E, tt * P:(tt + 1) * P],
                        rhs=id_f32[:E, :E],
                        is_transpose=True, start=True, stop=True,
                        skip_group_check=True)

            l_all = disp.tile([P, NTT * E], F32, tag="lall")
            nc.vector.tensor_tensor(
                out=l_all[:].rearrange("p (t e) -> p t e", e=E),
                in0=trps[:].rearrange("p (t e) -> p t e", e=E),
                in1=br_bc[:].rearrange("p e -> p () e").to_broadcast((P, NTT, E)),
                op=OP.add)

            cm_all = disp.tile([P, NTT], F32, tag="cm")
            sel_all = disp.tile([P, NTT], F32, tag="sel")
            tokf = disp.tile([P, NTT], F32, tag="tokf")
            toki = scr.tile([P, NTT], I32, tag="toki")
            nc.gpsimd.iota(toki[:], pattern=[[P, NTT]], base=0,
                           channel_multiplier=1)
            nc.vector.tensor_copy(out=tokf[:], in_=toki[:])

            def bcast(ap):  # [P, NTT] -> [P, NTT, E] stride-0 view
                return ap.to_broadcast((P, NTT, E))

            l3 = l_all[:].rearrange("p (t e) -> p t e", e=E)
            m1 = scr.tile([P, NTT], F32, tag="m1")
            nc.vector.tensor_reduce(
                out=m1[:], in_=l3, axis=mybir.AxisListType.X, op=OP.max)
            mask1 = scr.tile([P, NTT * E], F32, tag="mask1")
            mask1_3 = mask1[:].rearrange("p (t e) -> p t e", e=E)
            nc.vector.tensor_tensor(
                out=mask1_3, in0=l3, in1=bcast(m1[:]), op=OP.is_equal)
            l2 = scr.tile([P, NTT * E], F32, tag="l2")
            l2_3 = l2[:].rearrange("p (t e) -> p t e", e=E)
            nc.vector.tensor_scalar(
                out=l2[:], in0=mask1[:], scalar1=-BIG, scalar2=None,
                op0=OP.mult)
            nc.vector.tensor_add(l2[:], l2[:], l_all[:])
            m2 = scr.tile([P, NTT], F32, tag="m2")
            nc.vector.tensor_reduce(
                out=m2[:], in_=l2_3, axis=mybir.AxisListType.X, op=OP.max)
            mask2 = scr.tile([P, NTT * E], F32, tag="mask2")
            mask2_3 = mask2[:].rearrange("p (t e) -> p t e", e=E)
            nc.vector.tensor_tensor(
                out=mask2_3, in0=l2_3, in1=bcast(m2[:]), op=OP.is_equal)
            d = scr.tile([P, NTT], F32, tag="d")
            nc.vector.tensor_sub(d[:], m2[:], m1[:])
            ed = scr.tile([P, NTT], F32, tag="ed")
            nc.scalar.activation(out=ed[:], in_=d[:], func=AF.Exp)
            den = scr.tile([P, NTT], F32, tag="den")
            nc.vector.tensor_scalar_add(den[:], ed[:], 1.0)
            w1c = scr.tile([P, NTT], F32, tag="w1c")
            nc.vector.reciprocal(w1c[:], den[:])
            w2c = scr.tile([P, NTT], F32, tag="w2c")
            nc.vector.tensor_mul(w2c[:], ed[:], w1c[:])
            # c[p,t,e] = mask1*w1 + mask2*w2; then pick this core's expert
            call = scr.tile([P, NTT * E], F32, tag="call")
            call_3 = call[:].rearrange("p (t e) -> p t e", e=E)
            nc.vector.tensor_tensor(
                out=call_3, in0=mask1_3, in1=bcast(w1c[:]), op=OP.mult)
            c2t = scr.tile([P, NTT * E], F32, tag="c2t")
            c2_3 = c2t[:].rearrange("p (t e) -> p t e", e=E)
            nc.vector.tensor_tensor(
                out=c2_3, in0=mask2_3, in1=bcast(w2c[:]), op=OP.mult)
            nc.vector.tensor_add(call[:], call[:], c2t[:])
            cm8 = scr.tile([P, NTT * E], F32, tag="cm8")
            cm8_3 = cm8[:].rearrange("p (t e) -> p t e", e=E)
            nc.vector.tensor_tensor(
                out=cm8_3, in0=call_3,
                in1=oh_bc[:].rearrange("p e -> p () e").to_broadcast((P, NTT, E)),
                op=OP.mult)
            nc.vector.tensor_reduce(
                out=cm_all[:], in_=cm8_3, axis=mybir.AxisListType.X, op=OP.add)
            nc.vector.tensor_scalar(
                out=sel_all[:], in0=cm_all[:], scalar1=0.0, scalar2=None,
                op0=OP.is_gt)

            # ---- compaction: dense slot per selected token ----
            excl_ps = psr.tile([P, NTT], F32, tag="excl", bufs=1)
            nc.tensor.matmul(
                out=excl_ps[:], lhsT=u128[:], rhs=sel_all[:], start=True,
                stop=True)
            excl = disp.tile([P, NTT], F32, tag="exclsb")
            nc.vector.tensor_copy(out=excl[:], in_=excl_ps[:])
            tot_ps = psr.tile([1, NTT], F32, tag="totps", bufs=1)
            nc.tensor.matmul(
                out=tot_ps[:], lhsT=ones128[:], rhs=sel_all[:], start=True,
                stop=True)
            incl = disp.tile([1, NTT], F32, tag="incl")
            nc.vector.tensor_tensor_scan(
                out=incl[:], data0=tot_ps[:], data1=zeros16[:], initial=0.0,
                op0=OP.add, op1=OP.add)
            offs = disp.tile([1, NTT], F32, tag="offs")
            nc.vector.tensor_sub(offs[:], incl[:], tot_ps[:])
            offs_ps = psr.tile([P, NTT], F32, tag="offsps", bufs=1)
            nc.tensor.matmul(
                out=offs_ps[:], lhsT=ones_col[:], rhs=offs[:], start=True,
                stop=True)
            slot = disp.tile([P, NTT], F32, tag="slot")
            nc.vector.tensor_tensor(
                out=slot[:], in0=excl[:], in1=offs_ps[:], op=OP.add)
            # unselected tokens -> far past any real slot
            nc.vector.tensor_scalar_sub(slot[:], slot[:], float(TRASH))
            nc.vector.tensor_mul(slot[:], slot[:], sel_all[:])
            nc.vector.tensor_scalar_add(slot[:], slot[:], float(TRASH))

            # payload rows per token: tile idx, partition idx, c split into
            # bf16 hi+lo halves (hi+lo is fp32-exact to ~1.5e-5)
            chi = scr.tile([P, NTT], F32, tag="chi")
            clo = scr.tile([P, NTT], F32, tag="clo")
            chib = scr.tile([P, NTT], FP16, tag="chib")
            nc.vector.tensor_copy(out=chib[:], in_=cm_all[:])
            nc.vector.tensor_copy(out=chi[:], in_=chib[:])
            nc.vector.tensor_sub(clo[:], cm_all[:], chi[:])
            sloth = scr.tile([P, NTT], FP16, tag="sloth")
            tvals = scr.tile([P, NTT], F32, tag="tvals")
            ti = scr.tile([P, NTT], I32, tag="ti")
            nc.gpsimd.iota(ti[:], pattern=[[1, NTT]], base=0,
                           channel_multiplier=0)
            nc.vector.tensor_copy(out=tvals[:], in_=ti[:])
            pvals = scr.tile([P, 1], I32, tag="pvals")
            nc.gpsimd.iota(pvals[:], pattern=[[1, 1]], base=0,
                           channel_multiplier=1)
            pvf = scr.tile([P, 1], F32, tag="pvf")
            nc.vector.tensor_copy(out=pvf[:], in_=pvals[:])

            pairb = disp.tile([P, 4 * NTT], FP16, tag="pairb")
            pb4 = pairb[:].rearrange("p (t four) -> p t four", four=4)
            nc.vector.tensor_copy(
                out=pb4[:, :, 0:1], in_=tvals[:].rearrange("p t -> p t ()"))
            nc.vector.tensor_copy(
                out=pb4[:, :, 1:2],
                in_=pvf[:].rearrange("p o -> p () o").to_broadcast((P, NTT, 1)))
            nc.vector.tensor_copy(
                out=pb4[:, :, 2:3], in_=chi[:].rearrange("p t -> p t ()"))
            nc.vector.tensor_copy(
                out=pb4[:, :, 3:4], in_=clo[:].rearrange("p t -> p t ()"))

            # ---- on-chip inverse permutation via one-hot matmuls ----
            # cmp_t[p, s] = (slot[p, t] == s); pe[4, s] += pairb[:,t].T @ cmp_t
            # Exactly one token matches each filled slot, so the sums are
            # single-term; ids are small ints, exact in bf16.
            pe_parts = []
            for c0, n in CHUNKS:
                pe_parts.append(psr.tile(
                    [4, 512], F32, tag=f"pe{c0}", bufs=1, name=f"pe_ps{c0}"))
            nc.vector.tensor_copy(out=sloth[:], in_=slot[:])
            for t in range(NTT):
                cmp = scr.tile([P, CAP], FP16, tag="cmp", bufs=3)
                nc.vector.tensor_tensor(
                    out=cmp[:], in0=sloth[:, t:t + 1].to_broadcast((P, CAP)),
                    in1=iota640[:], op=OP.is_equal)
                for ci, (c0, n) in enumerate(CHUNKS):
                    nc.tensor.matmul(
                        out=pe_parts[ci][:, :n], lhsT=pairb[:, 4 * t:4 * t + 4],
                        rhs=cmp[:, c0:c0 + n],
                        start=(t == 0), stop=(t == NTT - 1))
            pe_sb = disp.tile([4, CAP], F32, tag="pesb")
            pe_sbh = disp.tile([4, CAP], FP16, tag="pesbh")
            for ci, (c0, n) in enumerate(CHUNKS):
                nc.vector.tensor_copy(
                    out=pe_sb[:, c0:c0 + n], in_=pe_parts[ci][:, :n])
                nc.vector.tensor_copy(
                    out=pe_sbh[:, c0:c0 + n], in_=pe_parts[ci][:, :n])
            # ship the slot table to the host (host: idx = 128*row0 + row1)
            nc.sync.dma_start(out=idxw[:], in_=pe_sb[:])

        # ---- dispatch: gather selected x rows, transpose to [H, CAP] ----
        with tc.tile_pool(name="psd", bufs=2, space="PSUM") as psd:
            # broadcast c over partitions: wbc[p, s] = c_hi[s] + c_lo[s]
            wbc_sb = disp.tile([P, CAP], F32, tag="wbc")
            for c0, n in CHUNKS:
                wps = psd.tile([P, 512], F32, tag="wbcps", bufs=1)
                nc.tensor.matmul(
                    out=wps[:, :n], lhsT=rowsel[:], rhs=pe_sbh[:, c0:c0 + n],
                    start=True, stop=True)
                nc.vector.tensor_copy(out=wbc_sb[:, c0:c0 + n], in_=wps[:, :n])

            xgt = [xgt_pool.tile([P, CAP], BF16, tag=f"xgt{k}", name=f"xgt{k}")
                   for k in range(NKH)]
            idx_is = []
            for ct in range(NCT):
                # idx per capacity tile: transpose pe_sb[:, ct*P:+P] -> [P, 4]
                trp = psd.tile([P, 4], F32, tag="idxtr")
                nc.tensor.matmul(
                    out=trp[:], lhsT=pe_sb[:4, ct * P:(ct + 1) * P],
                    rhs=id_f32[:4, :4],
                    is_transpose=True, start=True, stop=True)
                idx_f = scr.tile([P, 1], F32, tag="idxf")
                nc.vector.tensor_scalar(
                    out=idx_f[:], in0=trp[:, 0:1], scalar1=float(P),
                    scalar2=trp[:, 1:2], op0=OP.mult, op1=OP.add)
                idx_i = scr.tile([P, 1], I32, tag="idxi", bufs=NCT,
                                 name=f"idx_i{ct}")
                nc.vector.tensor_copy(out=idx_i[:], in_=idx_f[:])
                idx_is.append(idx_i)
            xgs = []
            for ct in range(NCT):
                xg = xg_pool.tile([P, H], BF16, tag="xg", bufs=NCT,
                                  name=f"xg{ct}")
                nc.gpsimd.indirect_dma_start(
                    out=xg[:],
                    out_offset=None,
                    in_=xbf[:],
                    in_offset=bass.IndirectOffsetOnAxis(
                        ap=idx_is[ct][:, 0:1], axis=0))
                xgs.append(xg)
            for ct in range(NCT):
                for k in range(NKH):
                    tps = psd.tile([P, P], BF16, tag="xtr")
                    nc.tensor.transpose(
                        out=tps[:], in_=xgs[ct][:, k * P:(k + 1) * P],
                        identity=id_bf[:])
                    nc.vector.tensor_copy(
                        out=xgt[k][:, ct * P:(ct + 1) * P], in_=tps[:])

        # ---- expert weights (resident in SBUF) ----
        from concourse.bass import _add_dep_helper
        last_x = x_dma_insts[-1]
        w1_all = wpool.tile([P, NKH, I_DIM], BF16, tag="w1a")
        w3_all = wpool.tile([P, NKH, I_DIM], BF16, tag="w3a")
        w2_all = wpool.tile([P, NKI, H], BF16, tag="w2a")
        w_dmas = []
        for half in range(2):
            k0, k1 = half * NKH // 2, (half + 1) * NKH // 2
            w_dmas.append(nc.scalar.dma_start(
                out=w1_all[:, k0:k1, :],
                in_=w1[:, k0 * I_DIM:k1 * I_DIM]))
            w_dmas.append(nc.scalar.dma_start(
                out=w3_all[:, k0:k1, :],
                in_=w3[:, k0 * I_DIM:k1 * I_DIM]))
        for half in range(2):
            k0, k1 = half * NKI // 2, (half + 1) * NKI // 2
            w_dmas.append(nc.scalar.dma_start(
                out=w2_all[:, k0:k1, :],
                in_=w2[:, k0 * H:k1 * H]))

        for wd in w_dmas:
            _add_dep_helper(wd.ins, last_x.ins, True,
                            "weights stream after xt (routing DMA priority)")

        # ---- expert FFN: gate/up + SwiGLU -> hT, down -> yT ----
        with tc.tile_pool(name="psm", bufs=2, space="PSUM") as psm:
            hts = [ht_pool.tile([P, CAP], BF16, tag=f"ht{i}", name=f"ht{i}")
                   for i in range(NKI)]
            for it in range(NKI):
                i0 = it * P
                for c0, n in CHUNKS:
                    gps = psm.tile([P, 512], F32, tag="gate")
                    ups = psm.tile([P, 512], F32, tag="up")
                    for k in range(NKH):
                        nc.tensor.matmul(
                            out=gps[:, :n], lhsT=w1_all[:, k, i0:i0 + P],
                            rhs=xgt[k][:, c0:c0 + n],
                            start=(k == 0), stop=(k == NKH - 1))
                    for k in range(NKH):
                        nc.tensor.matmul(
                            out=ups[:, :n], lhsT=w3_all[:, k, i0:i0 + P],
                            rhs=xgt[k][:, c0:c0 + n],
                            start=(k == 0), stop=(k == NKH - 1))
                    sl = mm_pool.tile([P, 512], BF16, tag="silu")
                    nc.scalar.activation(out=sl[:, :n], in_=gps[:, :n],
                                         func=AF.Sigmoid)
                    tmp = mm_pool.tile([P, 512], BF16, tag="sgate")
                    nc.vector.tensor_tensor(
                        out=tmp[:, :n], in0=sl[:, :n], in1=gps[:, :n],
                        op=OP.mult)
                    nc.vector.tensor_tensor(
                        out=hts[it][:, c0:c0 + n], in0=tmp[:, :n],
                        in1=ups[:, :n], op=OP.mult)
            for ht_i in range(NKH):
                h0 = ht_i * P
                ysb = mm_pool.tile([P, CAP], F32, tag="ysb")
                for c0, n in CHUNKS:
                    yps = psm.tile([P, 512], F32, tag="y")
                    for k in range(NKI):
                        nc.tensor.matmul(
                            out=yps[:, :n], lhsT=w2_all[:, k, h0:h0 + P],
                            rhs=hts[k][:, c0:c0 + n],
                            start=(k == 0), stop=(k == NKI - 1))
                    nc.vector.tensor_tensor(
                        out=ysb[:, c0:c0 + n], in0=yps[:, :n],
                        in1=wbc_sb[:, c0:c0 + n], op=OP.mult)
                nc.sync.dma_start(out=yt[h0:h0 + P, :], in_=ysb[:])

    nc.compile()
    return nc


_NC_CACHE = None


def _get_program():
    global _NC_CACHE
    if _NC_CACHE is None:
        _NC_CACHE = build_program()
    return _NC_CACHE


def _prepare_in_maps(x, Wr, br, W1, W3, W2):
    x2d = np.ascontiguousarray(np.asarray(x, dtype=np.float32).reshape(S, H))
    # (k, p, ch, c) -> (ch, k, p, c): each routing chunk DMA is contiguous
    xt = np.ascontiguousarray(
        x2d.T.reshape(NKH, P, N_ROUTE_CHUNKS, ROUTE_CHUNK)
        .transpose(2, 0, 1, 3)
        .reshape(N_ROUTE_CHUNKS * NKH * P, ROUTE_CHUNK))
    xth = xt.astype(ml_dtypes.bfloat16)
    xtl = (xt - xth.astype(np.float32)).astype(ml_dtypes.bfloat16)
    # fused per-chunk layout [ch*P, 2*NKH*RC]: row p = [hi(k0..k7) | lo(k0..k7)]
    def _chunkify(a):
        return a.reshape(N_ROUTE_CHUNKS, NKH, P, ROUTE_CHUNK).transpose(0, 2, 1, 3)
    xtc = np.concatenate([_chunkify(xth), _chunkify(xtl)], axis=2)
    xtc = np.ascontiguousarray(
        xtc.reshape(N_ROUTE_CHUNKS, P, 2, NKH, ROUTE_CHUNK)
        .reshape(N_ROUTE_CHUNKS * P, 2 * NKH * ROUTE_CHUNK))
    xbf = x2d.astype(ml_dtypes.bfloat16)
    wr_np = np.ascontiguousarray(np.asarray(Wr, dtype=np.float32))
    wrh_np = wr_np.astype(ml_dtypes.bfloat16)
    wrl_np = (wr_np - wrh_np.astype(np.float32)).astype(ml_dtypes.bfloat16)
    # [P, 2*NKH*E]: row p = [hi(k0..k7) | lo(k0..k7)] of Wr[k*P+p, :]
    def _wrpack(a):
        return a.reshape(NKH, P, E).transpose(1, 0, 2)
    wrc_np = np.ascontiguousarray(
        np.stack([_wrpack(wrh_np), _wrpack(wrl_np)], axis=1)
        .reshape(P, 2 * NKH * E))
    br_np = np.asarray(br, dtype=np.float32).reshape(1, E)
    W1 = np.asarray(W1, dtype=np.float32)
    W3 = np.asarray(W3, dtype=np.float32)
    W2 = np.asarray(W2, dtype=np.float32)
    in_maps = []
    for e in range(N_CORES):
        oh_np = np.zeros((1, E), np.float32)
        oh_np[0, e] = 1.0
        def _wpack(a, nk):
            return np.ascontiguousarray(
                a.reshape(nk, P, -1).transpose(1, 0, 2).reshape(P, -1))
        in_maps.append({
            "xtc": xtc,
            "xbf": xbf,
            "wrc": wrc_np,
            "br": br_np,
            "oh": oh_np,
            "w1": _wpack(W1[e].astype(ml_dtypes.bfloat16), NKH),
            "w3": _wpack(W3[e].astype(ml_dtypes.bfloat16), NKH),
            "w2": _wpack(W2[e].astype(ml_dtypes.bfloat16), NKI),
        })
    return in_maps


def _combine(results):
    out = np.zeros((S, H), np.float32)
    for e in range(N_CORES):
        idxw = np.asarray(results[e]["idxw"])
        yt = np.asarray(results[e]["yt"])
        idx = np.rint(idxw[0, :] * P + idxw[1, :]).astype(np.int64)
        np.add.at(out, idx, yt[:, :CAP].T)
    return out.reshape(B, S, H)


def run_on_device(inputs, trace=False, trace_cores=None):
    """Run the SPMD program; returns (full_output, BassKernelResults)."""
    nc = _get_program()
    in_maps = _prepare_in_maps(**inputs)
    kwargs = {}
    if trace:
        try:
            import types

            if "antenv.axon_hooks" not in sys.modules:
                from trn_agent_boot.trn_boot import _ntff_profile_via_ctypes

                hook = _ntff_profile_via_ctypes("/opt/axon/libaxon_pjrt.so")
                mod = types.ModuleType("antenv.axon_hooks")
                mod._hook = hook
                mod.get_axon_ntff_profile_hook = lambda: mod._hook

                def _set(h):
                    mod._hook = h

                mod.set_axon_ntff_profile_hook = _set
                sys.modules["antenv.axon_hooks"] = mod
                import antenv

                antenv.axon_hooks = mod
        except Exception as exc:  # profiling unavailable -> run untraced
            print(f"trace hook install failed: {exc}", file=sys.stderr)
        kwargs = dict(trace=True,
                      trace_cores=trace_cores or list(range(N_CORES)))
    res = run_bass_kernel_spmd(nc, in_maps, list(range(N_CORES)), **kwargs)
    return _combine(res.results), res


def kernel(x, Wr, br, W1, W3, W2):
    out, _ = run_on_device(dict(x=x, Wr=Wr, br=br, W1=W1, W3=W3, W2=W2))
    return out
